# revision 1
# baseline (speedup 1.0000x reference)
"""Bass kernel for nn_AltFilter: dual-pass windowed transformer + conv.

Sharding: pass-1 data-parallel over w (8 chunks of W/8), pass-2 over h.
Between passes: AllGather of pass-1 output (bf16), conv read from gathered.

Layouts (per core, per b-half):
  xtok1   [C, (v, wl, u, h)]        pass-1 token input (host prepped, bf16)
  y_perm  [C, (u, hpad34, wl, v)]   pass-1 output staged for AG (bf16)
  x1_full [8*C, (u, hpad34, wl, v)] AG result, wc-major blocks
  conv1in [C, (u, 6, Wpad, v)]      conv window (bf16) ; w = 4*wc+wl
  sc2     [C, (u, hl, w, v)]        conv1+res (f32/bf16) == pass-2 tokens
  y2      [C, (u, hl, w, v)]        pass-2 out chunk (bf16) -> AG (10-block padded)
  out     [C, (b, u, hl, w, v)]     final (f32)
"""

import numpy as np
from dataclasses import dataclass
from concourse import bass, bacc, tile, mybir
from concourse.bass import ds

BF16 = mybir.dt.bfloat16
F32 = mybir.dt.float32
AF = mybir.ActivationFunctionType
OP = mybir.AluOpType


@dataclass
class Cfg:
    A: int = 5
    C: int = 128
    E: int = 256
    NH: int = 8
    B: int = 2
    H: int = 32
    W: int = 32
    ncores: int = 8
    ch_seqs: int = 6      # sequences per processing chunk
    win: int = 5          # attention half-window (KW//2)

    @property
    def HD(self):
        return self.E // self.NH

    @property
    def WL(self):
        return self.W // self.ncores

    @property
    def HL(self):
        return self.H // self.ncores

    @property
    def L1(self):
        return self.A * self.H       # pass-1 tokens per seq (u, h)

    @property
    def L2(self):
        return self.W * self.A       # pass-2 tokens per seq (w, v)

    @property
    def SEQ1(self):
        return self.A * self.WL      # per-b seqs pass 1 (v, wl)

    @property
    def SEQ2(self):
        return self.A * self.HL      # per-b seqs pass 2 (u, hl)

    @property
    def HPAD(self):
        return self.H + 2



_tname_ctr = [0]


def _tn(tag):
    _tname_ctr[0] += 1
    return f"{tag}_{_tname_ctr[0]}"

def mchunks(L):
    out = []
    o = 0
    while o < L:
        sz = min(128, L - o)
        out.append((o, sz))
        o += sz
    return out


# ---------------------------------------------------------------- host prep

def host_prep(cfg, inputs):
    """Build per-core in_maps from full inputs. Returns list of dicts."""
    import ml_dtypes
    bf = ml_dtypes.bfloat16
    A, C, E, B, H, W = cfg.A, cfg.C, cfg.E, cfg.B, cfg.H, cfg.W
    NC = cfg.ncores
    WL, HL = cfg.WL, cfg.HL

    buf = np.asarray(inputs["buffer"], np.float32)
    buf6 = buf.reshape(B, C, A, A, H, W)                    # b c u v h w

    # pass-1 tokens: [c, b, v, w, u, h] -> core k takes w slice
    xt = np.ascontiguousarray(buf6.transpose(1, 0, 3, 5, 2, 4))  # c b v w u h
    # conv1 residual: [c, b, u, h, w, v] -> core k takes h slice
    rs = np.ascontiguousarray(buf6.transpose(1, 0, 2, 4, 5, 3))  # c b u h w v

    ip = np.asarray(inputs["in_proj"], np.float32)
    sc = 1.0 / np.sqrt(cfg.HD)
    WqT = (ip[0:E].T * sc).astype(bf)
    WkT = ip[E:2 * E].T.astype(bf)
    WvT = ip[2 * E:3 * E].T.astype(bf)
    WinT = np.asarray(inputs["Win"], np.float32).T.astype(bf)       # (C, E)
    WoT = np.asarray(inputs["attn_out_w"], np.float32).T.astype(bf)  # (E, E)
    Wff1T = np.asarray(inputs["ff1"], np.float32).T.astype(bf)       # (E, 2E)
    Wff2T = np.asarray(inputs["ff2"], np.float32).T.astype(bf)       # (2E, E)
    WoutT = np.asarray(inputs["Wout"], np.float32).T.astype(bf)      # (E, C)
    cw = np.asarray(inputs["conv_w"], np.float32)[:, :, 0]           # (O,I,3,3)
    tapT = np.ascontiguousarray(cw.transpose(2, 3, 1, 0)).astype(bf)  # ky kx I O

    def band_mask(L, blk, n_outer, outer_major):
        # tokens: outer_major=True -> l = outer*blk_count... build via indices
        # pass1: l = u*H + h, band over h ; pass2: l = w*A + v, band over w
        l = np.arange(L)
        if outer_major:
            pos = l % blk          # h = l % H  (u-major, h inner)
        else:
            pos = l // n_outer     # w = l // A (w-major, v inner)
        d = np.abs(pos[:, None] - pos[None, :])
        m = np.where(d <= cfg.win, 0.0, -30000.0).astype(np.float32)
        return m.astype(bf)

    mask1 = band_mask(cfg.L1, cfg.H, cfg.A, True)
    mask2 = band_mask(cfg.L2, cfg.W, cfg.A, False)

    # rank-32 mask factorization: mask[m, l] = sum_r ind[r, m] * mrow[r, l]
    # (pos = h for pass1, w for pass2; both have 32 possible values)
    def ind_mrow(L, pos):
        r = np.arange(32)
        ind = (pos[None, :] == r[:, None]).astype(np.float32)
        mr = np.where(np.abs(r[:, None] - pos[None, :]) <= cfg.win,
                      0.0, -30000.0).astype(np.float32)
        return ind.astype(bf), mr.astype(bf)

    l1 = np.arange(cfg.L1)
    l2 = np.arange(cfg.L2)
    ind1, mrow1 = ind_mrow(cfg.L1, l1 % cfg.H)
    ind2, mrow2 = ind_mrow(cfg.L2, l2 // cfg.A)

    lnw1 = np.asarray(inputs["ln_w"], np.float32).reshape(E // 128, 128).T.copy()
    lnb1 = np.asarray(inputs["ln_b"], np.float32).reshape(E // 128, 128).T.copy()
    lnw2 = np.asarray(inputs["ffn_ln_w"], np.float32).reshape(E // 128, 128).T.copy()
    lnb2 = np.asarray(inputs["ffn_ln_b"], np.float32).reshape(E // 128, 128).T.copy()

    shared = dict(WinT=WinT, WqT=WqT, WkT=WkT, WvT=WvT, WoT=WoT,
                  Wff1T=Wff1T, Wff2T=Wff2T, WoutT=WoutT, tapT=tapT,
                  ind1=ind1, mrow1=mrow1, ind2=ind2, mrow2=mrow2,
                  lnw1=lnw1, lnb1=lnb1, lnw2=lnw2, lnb2=lnb2)

    maps = []
    for k in range(NC):
        xk = xt[:, :, :, k * WL:(k + 1) * WL]   # c b v wl u h
        xk = np.ascontiguousarray(xk).reshape(C, -1).astype(bf)
        rk = rs[:, :, :, k * HL:(k + 1) * HL]   # c b u hl w v
        rk = np.ascontiguousarray(rk).reshape(C, -1).astype(np.float32)
        m = dict(shared)
        m["xtok1"] = xk
        m["res1"] = rk
        maps.append(m)
    return maps


def host_unshard(cfg, outs):
    """outs: list of per-core 'out' arrays [C, B*A*HL*W*A] -> full output."""
    A, C, B, H, W = cfg.A, cfg.C, cfg.B, cfg.H, cfg.W
    o = np.stack(outs)  # j c b u hl w v
    o = o.reshape(cfg.ncores, C, B, A, cfg.HL, W, A)
    o = o.transpose(2, 1, 3, 6, 0, 4, 5)  # b c u v j hl w
    return np.ascontiguousarray(o).reshape(B, C, A * A, H, W)


# ---------------------------------------------------------------- builder

class Ker:
    """Holds nc + pools + weight tiles while building."""

    _tables_pinned = False

    @classmethod
    def _pin_act_tables(cls):
        import os as _os
        if cls._tables_pinned or _os.environ.get("KTAB", "pin") != "pin":
            return
        cls._tables_pinned = True
        import concourse.bacc as _baccmod
        from concourse.hw_specs import get_activation_tables as _gat

        def pinned(arch):
            tabs = _gat(arch)
            keep = "natural_log_exp_and_others"
            mine = {AF.Exp, AF.Ln, AF.Copy, AF.Identity}
            out = {}
            for name, s in tabs.items():
                out[name] = s if name == keep else (s - mine)
            return out

        _baccmod.get_activation_tables = pinned

    def __init__(self, cfg, stage=4, ablate=()):
        self._pin_act_tables()
        self.cfg = cfg
        self.stage = stage  # 1=pass1, 2=+conv1, 3=+pass2, 4=full
        self.ablate = set(ablate)
        self.nc = bacc.Bacc("TRN2", target_bir_lowering=False, debug=False,
                            num_devices=cfg.ncores)

    # -- weights to sbuf ---------------------------------------------------
    def load_weights(self, tc, pool):
        nc, cfg = self.nc, self.cfg
        E = cfg.E

        def wtile(name, K, M):
            p = self.params[name]
            t = pool.tile([128, (K // 128) * M], BF16, tag=name)
            for kt in range(K // 128):
                nc.sync.dma_start(t[:, kt * M:(kt + 1) * M],
                                  p[kt * 128:(kt + 1) * 128, :])
            return t

        self.WinT = wtile("WinT", cfg.C, E)
        self.WqT = wtile("WqT", E, E)
        self.WkT = wtile("WkT", E, E)
        self.WvT = wtile("WvT", E, E)
        self.WoT = wtile("WoT", E, E)
        self.Wff1T = wtile("Wff1T", E, 2 * E)
        self.Wff2T = wtile("Wff2T", 2 * E, E)
        self.WoutT = wtile("WoutT", E, cfg.C)

        tap = self.params["tapT"]  # (3,3,I,O)
        self.taps = pool.tile([128, 9 * 128], BF16, tag="taps", name=_tn("taps"))
        for ky in range(3):
            for kx in range(3):
                i = ky * 3 + kx
                nc.sync.dma_start(self.taps[:, i * 128:(i + 1) * 128],
                                  tap[ky, kx])


        def lnt(name):
            t = pool.tile([128, E // 128], F32, tag=name)
            nc.sync.dma_start(t, self.params[name])
            return t

        self.lnw1, self.lnb1 = lnt("lnw1"), lnt("lnb1")
        self.lnw2, self.lnb2 = lnt("lnw2"), lnt("lnb2")

        self.ones = pool.tile([128, 128], BF16, tag="ones", name=_tn("ones"))
        nc.vector.memset(self.ones[:], 1.0 / E)


    # -- generic GEMM over one token chunk --------------------------------
    def gemm(self, psum_pool, wt, K, M, rhs_fn, ntok, nt_sz, evict_fn,
             tag="gemm", name=_tn("gemm")):
        """out[mt] = sum_kt  wt[kt,mt].T @ rhs(kt, nt) ; evict per (mt, nt)."""
        nc = self.nc
        KT, MT = K // 128, M // 128
        for mt in range(MT):
            for nt0 in range(0, ntok, nt_sz):
                n = min(nt_sz, ntok - nt0)
                ps = psum_pool.tile([128, nt_sz], F32, tag=tag)
                for kt in range(KT):
                    nc.tensor.matmul(
                        ps[:, :n],
                        wt[:, kt * M + mt * 128: kt * M + mt * 128 + 128],
                        rhs_fn(kt, nt0, n),
                        start=(kt == 0), stop=(kt == KT - 1))
                evict_fn(mt, nt0, n, ps)

    # -- layernorm over one chunk -----------------------------------------
    def ln_chunk_v2(self, tc, pools, x_f32, lnw, lnb, out_bf, ntok, nt_sz,
                    also_sq_src=None):
        """Feature-major LN. Stats are computed via all-ones matmuls whose
        M=128 stationary replicates sum across all partitions, so no
        partition-broadcast is ever needed."""
        nc, cfg = self.nc, self.cfg
        FT = cfg.E // 128
        x_bf = also_sq_src
        sq = pools["scratch"].tile([128, FT * ntok], BF16, tag="lnsq", name=_tn("lnsq"))
        for ft in range(FT):
            nc.vector.tensor_tensor(
                out=sq[:, ft * ntok:(ft + 1) * ntok],
                in0=x_bf[:, ft * ntok:(ft + 1) * ntok],
                in1=x_bf[:, ft * ntok:(ft + 1) * ntok], op=OP.mult)
        mean = pools["scratch"].tile([128, ntok], F32, tag="lnmean", name=_tn("lnmean"))
        rstd = pools["scratch"].tile([128, ntok], F32, tag="lnrstd", name=_tn("lnrstd"))
        for nt0 in range(0, ntok, nt_sz):
            n = min(nt_sz, ntok - nt0)
            ps_s = pools["stat_psum"].tile([128, nt_sz], F32, tag="lnstat", name=_tn("lnstat"))
            ps_q = pools["stat_psum"].tile([128, nt_sz], F32, tag="lnstat", name=_tn("lnstat"))
            for kt in range(FT):
                nc.tensor.matmul(ps_s[:, :n], self.ones,
                                 x_bf[:, kt * ntok + nt0: kt * ntok + nt0 + n],
                                 start=(kt == 0), stop=(kt == FT - 1))
            for kt in range(FT):
                nc.tensor.matmul(ps_q[:, :n], self.ones,
                                 sq[:, kt * ntok + nt0: kt * ntok + nt0 + n],
                                 start=(kt == 0), stop=(kt == FT - 1))
            nc.vector.tensor_copy(out=mean[:, nt0:nt0 + n], in_=ps_s[:, :n])
            nc.vector.tensor_copy(out=rstd[:, nt0:nt0 + n], in_=ps_q[:, :n])
        # rstd = (E[x^2] + eps - mean^2) ** -0.5
        msq = pools["scratch"].tile([128, ntok], F32, tag="lnmsq", name=_tn("lnmsq"))
        nc.vector.tensor_tensor(out=msq[:], in0=mean[:], in1=mean[:],
                                op=OP.mult)
        nc.vector.scalar_tensor_tensor(
            out=rstd[:], in0=rstd[:], scalar=1e-5, in1=msq[:],
            op0=OP.add, op1=OP.subtract)
        # rstd = exp(-0.5*ln(var)); Ln+Exp share one ACT table so no
        # table reloads against the attention exps.
        if "rsqrt" not in self.ablate:
            nc.scalar.activation(rstd[:], rstd[:], AF.Ln)
            nc.scalar.activation(rstd[:], rstd[:], AF.Exp, scale=-0.5)
        t1 = pools["scratch"].tile([128, nt_sz], F32, tag="lnt1", name=_tn("lnt1"))
        for ft in range(FT):
            for nt0 in range(0, ntok, nt_sz):
                n = min(nt_sz, ntok - nt0)
                nc.vector.tensor_tensor(
                    out=t1[:, :n],
                    in0=x_f32[:, ft * ntok + nt0: ft * ntok + nt0 + n],
                    in1=mean[:, nt0:nt0 + n], op=OP.subtract)
                nc.vector.tensor_tensor(
                    out=t1[:, :n], in0=t1[:, :n],
                    in1=rstd[:, nt0:nt0 + n], op=OP.mult)
                if "lnapply" in self.ablate:
                    nc.scalar.activation(
                        out_bf[:, ft * ntok + nt0: ft * ntok + nt0 + n],
                        t1[:, :n], AF.Copy)
                else:
                    nc.scalar.activation(
                        out_bf[:, ft * ntok + nt0: ft * ntok + nt0 + n],
                        t1[:, :n], AF.Identity, bias=lnb[:, ft:ft + 1],
                        scale=lnw[:, ft:ft + 1])

    # -- attention for one chunk of sequences ------------------------------
    def attention(self, pools, q_bf, k_bf, v_main, v_tail, maskfac,
                  o_bf, nseq, L, ntok, ntokmax):
        nc, cfg = self.nc, self.cfg
        NH, HD = cfg.NH, cfg.HD
        VW = 2 * HD  # per-head v cols: HD ones (denom replicate) + HD values
        mcs = mchunks(L)
        if "att_m1" in self.ablate:
            mcs = mcs[:1]
        for s in range(nseq):
            for hd in range(NH):
                fp = hd // 2          # column block (head pair)
                po = 64 * (hd % 2)    # 64-row window: [const; head feats]
                qs = q_bf[po:po + 64,
                          fp * ntokmax + s * L: fp * ntokmax + (s + 1) * L]
                o_ps = pools["o_psum"].tile([VW, L], F32, tag="o_ps", name=_tn("o_ps"))
                exps = []
                for mi, (mo, msz) in enumerate(mcs):
                    ks = k_bf[po:po + 64,
                              fp * ntokmax + s * L + mo:
                              fp * ntokmax + s * L + mo + msz]
                    sc_ps = pools["sc_psum"].tile([msz, L], F32,
                                                  tag="sc", name=_tn("sc"))
                    nc.tensor.matmul(sc_ps, ks, qs, start=True, stop=True)
                    ex = pools["exp"].tile([msz, L], BF16, tag=f"exp{mi}", name=_tn(f"exp{mi}"))
                    if "exp" in self.ablate:
                        nc.scalar.activation(ex, sc_ps, AF.Copy)
                    else:
                        nc.scalar.activation(ex, sc_ps, AF.Exp)
                    exps.append((mo, msz, ex))
                for ci, (mo, msz, ex) in enumerate(exps):
                    if mo == 0:
                        va = v_main[0:msz, s * NH * VW + hd * VW:
                                    s * NH * VW + hd * VW + VW]
                    else:
                        va = v_tail[0:msz,
                                    s * NH * VW + hd * VW:
                                    s * NH * VW + hd * VW + VW]
                    nc.tensor.matmul(o_ps, va, ex, start=(ci == 0),
                                     stop=(ci == len(exps) - 1))
                ofp, opo = hd // 4, 32 * (hd % 4)
                if "att_norm" in self.ablate:
                    nc.vector.tensor_copy(
                        out=o_bf[opo:opo + 32,
                                 ofp * ntok + s * L: ofp * ntok + (s + 1) * L],
                        in_=o_ps[HD:2 * HD, :])
                else:
                    rec = pools["scratch"].tile([32, L], F32, tag="recip", name=_tn("recip"))
                    import os as _os
                    if _os.environ.get("KRECIP", "fast") == "fast":
                        # custom-DVE op requires base-partition-0 input;
                        # v_aug puts the denominator rows first for this.
                        nc.vector.reciprocal_approx_fast(rec, o_ps[0:HD, :])
                    else:
                        nc.vector.reciprocal(rec, o_ps[0:HD, :])
                    nc.vector.tensor_tensor(
                        out=o_bf[opo:opo + 32,
                                 ofp * ntok + s * L: ofp * ntok + (s + 1) * L],
                        in0=o_ps[HD:2 * HD, :], in1=rec, op=OP.mult)

    # -- one transformer pass over one b-half ------------------------------
    def transformer_half(self, tc, pools, x_src_fn, maskfac, L, nseq_b,
                         evict_y_fn):
        """x_src_fn(c0, ntok) -> bf16 [128, ntok] input token tile (Win rhs).
        evict_y_fn(s_global_pair_index, nt0, n, psum) writes final y."""
        nc, cfg = self.nc, self.cfg
        E = cfg.E
        FT = E // 128
        CH = cfg.ch_seqs
        NTs = 3 * L            # token tile = 3 seqs (psum [128,480] f32 fits a bank)
        ntokmax = CH * L
        # Layout per head-pair block g (cols [g*ntokmax, (g+1)*ntokmax)):
        #   rows  0:32  = Ind (k) / Mrow (q) mask-factor constants
        #   rows 32:64  = head 2g features
        #   rows 64:96  = head 2g+1 features
        #   rows 96:128 = same constants again
        # head 2g reads rows [0:64], head 2g+1 rows [64:128] - each a legal
        # 64-row base - so one K=64 matmul computes scores+mask together.
        q_bf = pools["qk"].tile([128, 2 * (cfg.E // 128) * ntokmax], BF16,
                                tag="q_bf", name=_tn("q_bf"))
        k_bf = pools["qk"].tile([128, 2 * (cfg.E // 128) * ntokmax], BF16,
                                tag="k_bf", name=_tn("k_bf"))
        ind_p, mrow_p = maskfac
        for blk in range(2 * (cfg.E // 128)):
            for s in range(CH):
                for r0 in (0, 96):
                    nc.sync.dma_start(
                        k_bf[r0:r0 + 32, blk * ntokmax + s * L:
                             blk * ntokmax + (s + 1) * L],
                        self.params[ind_p])
                    nc.sync.dma_start(
                        q_bf[r0:r0 + 32, blk * ntokmax + s * L:
                             blk * ntokmax + (s + 1) * L],
                        self.params[mrow_p])
        for c0 in range(0, nseq_b, CH):
            ns = min(CH, nseq_b - c0)
            ntok = ns * L
            x_bf = x_src_fn(c0, ntok)
            tok_f = pools["big1"].tile([128, FT * ntok], F32, tag="tok_f", name=_tn("tok_f"))
            tok_bf = pools["big"].tile([128, FT * ntok], BF16, tag="tok_bf", name=_tn("tok_bf"))

            def ev_tok(mt, nt0, n, ps):
                nc.vector.tensor_copy(
                    out=tok_f[:, mt * ntok + nt0: mt * ntok + nt0 + n],
                    in_=ps[:, :n])
                nc.scalar.activation(
                    tok_bf[:, mt * ntok + nt0: mt * ntok + nt0 + n],
                    ps[:, :n], AF.Copy)

            self.gemm(pools["psum"], self.WinT, cfg.C, E,
                      lambda kt, nt0, n: x_bf[:, nt0:nt0 + n],
                      ntok, NTs, ev_tok, tag="gemm", name=_tn("gemm"))

            tn = pools["big"].tile([128, FT * ntok], BF16, tag="tn", name=_tn("tn"))
            if "ln" in self.ablate:
                nc.vector.tensor_copy(out=tn, in_=tok_bf)
            else:
                self.ln_chunk_v2(tc, pools, tok_f, self.lnw1, self.lnb1, tn,
                                 ntok, NTs, also_sq_src=tok_bf)

            def mk_ev(dst):
                # 32-row copies: >32-partition APs must be 64-aligned, and
                # the const-interleaved layout puts head rows at 32:96.
                def ev(mt, nt0, n, ps):
                    for qi in range(4):
                        blk = 2 * mt + qi // 2
                        dr = 32 + 32 * (qi % 2)
                        nc.vector.tensor_copy(
                            out=dst[dr:dr + 32, blk * ntokmax + nt0:
                                    blk * ntokmax + nt0 + n],
                            in_=ps[32 * qi:32 * qi + 32, :n])
                return ev

            rhs_tn = lambda kt, nt0, n: tn[:, kt * ntok + nt0: kt * ntok + nt0 + n]
            self.gemm(pools["psum"], self.WqT, E, E, rhs_tn, ntok, NTs,
                      mk_ev(q_bf), tag="gemm", name=_tn("gemm"))
            self.gemm(pools["psum"], self.WkT, E, E, rhs_tn, ntok, NTs,
                      mk_ev(k_bf), tag="gemm", name=_tn("gemm"))

            # V token-major with ones column, [tok, NH*(HD+1)]
            NH, HD = cfg.NH, cfg.HD
            VW = 2 * HD
            v_main = pools["big"].tile([128, CH * NH * VW], BF16, tag="v_main", name=_tn("v_main"))
            has_tail = L > 128
            v_tail = None
            if has_tail:
                v_tail = pools["big"].tile([32, CH * NH * VW], BF16,
                                           tag="v_tail", name=_tn("v_tail"))
            # ones columns
            nc.vector.memset(
                v_main[:, 0:ns * NH * VW].rearrange(
                    "p (s x) -> p s x", x=VW)[:, :, 0:HD], 1.0)
            if has_tail:
                nc.vector.memset(
                    v_tail[:, 0:ns * NH * VW].rearrange(
                        "p (s x) -> p s x", x=VW)[:, :, 0:HD], 1.0)
            Lm = min(128, L)
            for s in range(ns):
                ps = pools["psum"].tile([128, E], F32, tag="gemm", name=_tn("gemm"))
                for kt in range(FT):
                    nc.tensor.matmul(
                        ps[0:Lm, :],
                        tok_bf[:, kt * ntok + s * L: kt * ntok + s * L + Lm],
                        self.WvT[:, kt * E:(kt + 1) * E],
                        start=(kt == 0), stop=(kt == FT - 1))
                nc.vector.tensor_copy(
                    out=v_main[0:Lm,
                               s * NH * VW: s * NH * VW + NH * VW].rearrange(
                        "p (nh x) -> p nh x", x=VW)[:, :, HD:2 * HD],
                    in_=ps[0:Lm, :].rearrange(
                        "p (nh d) -> p nh d", d=HD))
            if has_tail:
                tl = L - 128
                for s in range(ns):
                    ps = pools["psum"].tile([128, E], F32, tag="gemm", name=_tn("gemm"))
                    for kt in range(FT):
                        nc.tensor.matmul(
                            ps[0:tl, :],
                            tok_bf[:, kt * ntok + s * L + 128:
                                   kt * ntok + s * L + 128 + tl],
                            self.WvT[:, kt * E:(kt + 1) * E],
                            start=(kt == 0), stop=(kt == FT - 1))
                    nc.vector.tensor_copy(
                        out=v_tail[0:tl,
                                   s * NH * VW: s * NH * VW + NH * VW].rearrange(
                            "p (nh x) -> p nh x", x=VW)[:, :, HD:2 * HD],
                        in_=ps[0:tl, :].rearrange(
                            "p (nh d) -> p nh d", d=HD))

            o_bf = pools["big"].tile([128, FT * ntok], BF16, tag="o_bf", name=_tn("o_bf"))
            if "attn" in self.ablate:
                nc.vector.tensor_copy(out=o_bf, in_=tn)
            else:
                self.attention(pools, q_bf, k_bf, v_main, v_tail, maskfac,
                               o_bf, ns, L, ntok, ntokmax)

            # out-proj + residual
            outp_f = pools["big1"].tile([128, FT * ntok], F32, tag="outp_f", name=_tn("outp_f"))
            outp_bf = pools["big"].tile([128, FT * ntok], BF16, tag="outp_bf", name=_tn("outp_bf"))

            def ev_outp(mt, nt0, n, ps):
                nc.vector.tensor_tensor(
                    out=outp_f[:, mt * ntok + nt0: mt * ntok + nt0 + n],
                    in0=ps[:, :n],
                    in1=tok_f[:, mt * ntok + nt0: mt * ntok + nt0 + n],
                    op=OP.add)
                nc.scalar.activation(
                    outp_bf[:, mt * ntok + nt0: mt * ntok + nt0 + n],
                    outp_f[:, mt * ntok + nt0: mt * ntok + nt0 + n], AF.Copy)

            rhs_o = lambda kt, nt0, n: o_bf[:, kt * ntok + nt0: kt * ntok + nt0 + n]
            self.gemm(pools["psum"], self.WoT, E, E, rhs_o, ntok, NTs,
                      ev_outp, tag="gemm", name=_tn("gemm"))

            tn2 = pools["big"].tile([128, FT * ntok], BF16, tag="tn2", name=_tn("tn2"))
            if "ln" in self.ablate:
                nc.vector.tensor_copy(out=tn2, in_=outp_bf)
            else:
                self.ln_chunk_v2(tc, pools, outp_f, self.lnw2, self.lnb2, tn2,
                                 ntok, NTs, also_sq_src=outp_bf)

            ffh = pools["big"].tile([128, 2 * FT * ntok], BF16, tag="ffh", name=_tn("ffh"))

            def ev_ffh(mt, nt0, n, ps):
                nc.vector.tensor_relu(
                    out=ffh[:, mt * ntok + nt0: mt * ntok + nt0 + n],
                    in_=ps[:, :n])

            rhs_tn2 = lambda kt, nt0, n: tn2[:, kt * ntok + nt0: kt * ntok + nt0 + n]
            self.gemm(pools["psum"], self.Wff1T, E, 2 * E, rhs_tn2, ntok, NTs,
                      ev_ffh, tag="gemm", name=_tn("gemm"))

            res2_f = pools["big1"].tile([128, FT * ntok], F32, tag="res2_f", name=_tn("res2_f"))
            res2_bf = pools["big"].tile([128, FT * ntok], BF16, tag="res2_bf", name=_tn("res2_bf"))

            def ev_ffo(mt, nt0, n, ps):
                nc.vector.tensor_tensor(
                    out=res2_f[:, mt * ntok + nt0: mt * ntok + nt0 + n],
                    in0=ps[:, :n],
                    in1=outp_f[:, mt * ntok + nt0: mt * ntok + nt0 + n],
                    op=OP.add)
                nc.scalar.activation(
                    res2_bf[:, mt * ntok + nt0: mt * ntok + nt0 + n],
                    res2_f[:, mt * ntok + nt0: mt * ntok + nt0 + n], AF.Copy)

            rhs_ffh = lambda kt, nt0, n: ffh[:, kt * ntok + nt0: kt * ntok + nt0 + n]
            self.gemm(pools["psum"], self.Wff2T, 2 * E, E, rhs_ffh, ntok, NTs,
                      ev_ffo, tag="gemm", name=_tn("gemm"))

            rhs_r2 = lambda kt, nt0, n: res2_bf[:, kt * ntok + nt0: kt * ntok + nt0 + n]
            self.gemm(pools["psum"], self.WoutT, E, cfg.C, rhs_r2, ntok, NTs,
                      lambda mt, nt0, n, ps: evict_y_fn(c0, nt0, n, ps),
                      tag="gemm", name=_tn("gemm"))

    # -- conv 3x3 over (h,w) for one b-half --------------------------------
    def conv_half(self, pools, cin, evict_fn):
        """cin: sbuf [128, A*6*WPAD*A(v)] bf16 window tile (w-padded).
        out pixels (u, hl, w, v); evict_fn(u, hp, n, psum) with pixel tiles
        [128, 2*W*A(v)] (hl pairs)."""
        nc, cfg = self.nc, self.cfg
        A, W = cfg.A, cfg.W
        WP = W + 2
        # cin strides: v:1, w:A, hwin:WP*A, u:6*WP*A
        su, sh, sw = 6 * WP * A, WP * A, A
        cv = cin.rearrange("p (u h w v) -> p u h w v", u=A, h=6, w=WP)
        for u in range(A):
            for hp in range(0, cfg.HL, 2):
                nh = min(2, cfg.HL - hp)
                ps = pools["psum"].tile([128, 2 * W * A], F32, tag="gemm", name=_tn("gemm"))
                first = True
                for dy in range(3):
                    for dx in range(3):
                        rhs = cv[:, u, hp + dy:hp + dy + nh, dx:dx + W, :]
                        nc.tensor.matmul(
                            ps[:, :nh * W * A],
                            self.taps[:, (dy * 3 + dx) * 128:(dy * 3 + dx + 1) * 128],
                            rhs,
                            start=first, stop=(dy == 2 and dx == 2))
                        first = False
                evict_fn(u, hp, nh, ps)

    # -- full graph --------------------------------------------------------
    def build(self):
        cfg = self.cfg
        nc = self.nc
        A, C, E, B, H, W = cfg.A, cfg.C, cfg.E, cfg.B, cfg.H, cfg.W
        L1, L2 = cfg.L1, cfg.L2
        FT = E // 128
        WL, HL, HP = cfg.WL, cfg.HL, cfg.HPAD
        NC = cfg.ncores
        WPAD = W + 2
        chunk1_cols = A * HP * WL * A          # y_perm cols (u hpad wl v)
        chunk2_cols = A * HL * W * A           # y2 cols (u hl w v)

        # ---- dram params
        P = {}
        def par(name, shape, dt):
            P[name] = nc.dram_tensor(name, shape, dt, kind="ExternalInput").ap()
        par("xtok1", [C, B * cfg.SEQ1 * L1], BF16)
        par("res1", [C, B * chunk2_cols], F32)
        par("WinT", [C, E], BF16)
        for n in ("WqT", "WkT", "WvT", "WoT"):
            par(n, [E, E], BF16)
        par("Wff1T", [E, 2 * E], BF16)
        par("Wff2T", [2 * E, E], BF16)
        par("WoutT", [E, C], BF16)
        par("tapT", [3, 3, C, C], BF16)
        par("ind1", [32, L1], BF16)
        par("mrow1", [32, L1], BF16)
        par("ind2", [32, L2], BF16)
        par("mrow2", [32, L2], BF16)
        for n in ("lnw1", "lnb1", "lnw2", "lnb2"):
            par(n, [128, FT], F32)
        self.params = P
        out_ext = nc.dram_tensor("out", [C, B * chunk2_cols], F32,
                                 kind="ExternalOutput")

        with tile.TileContext(nc) as tc:
            import contextlib
            with contextlib.ExitStack() as ctx:
                pools = {}
                pools["const"] = ctx.enter_context(
                    tc.tile_pool(name="const", bufs=1))
                pools["big"] = ctx.enter_context(
                    tc.tile_pool(name="big", bufs=2))
                pools["big1"] = ctx.enter_context(
                    tc.tile_pool(name="big1", bufs=1))
                pools["qk"] = ctx.enter_context(
                    tc.tile_pool(name="qk", bufs=1))
                pools["scratch"] = ctx.enter_context(
                    tc.tile_pool(name="scratch", bufs=1))
                pools["io"] = ctx.enter_context(
                    tc.tile_pool(name="io", bufs=1))
                pools["psum"] = ctx.enter_context(
                    tc.tile_pool(name="psum", bufs=2, space="PSUM"))
                pools["stat_psum"] = ctx.enter_context(
                    tc.tile_pool(name="stat_psum", bufs=2, space="PSUM"))
                pools["sc_psum"] = ctx.enter_context(
                    tc.tile_pool(name="sc_psum", bufs=2, space="PSUM"))
                pools["o_psum"] = ctx.enter_context(
                    tc.tile_pool(name="o_psum", bufs=2, space="PSUM"))
                pools["exp"] = ctx.enter_context(
                    tc.tile_pool(name="exp", bufs=2))
                pools["dram"] = ctx.enter_context(
                    tc.tile_pool(name="dram", bufs=1, space="DRAM"))
                self._build_body(tc, pools, out_ext)
        nc.compile()
        return nc

    def _build_body(self, tc, pools, out_ext):
        nc, cfg = self.nc, self.cfg
        A, C, E, B, H, W = cfg.A, cfg.C, cfg.E, cfg.B, cfg.H, cfg.W
        L1, L2 = cfg.L1, cfg.L2
        FT = E // 128
        WL, HL, HP = cfg.WL, cfg.HL, cfg.HPAD
        NC = cfg.ncores
        WPAD = W + 2
        ch1 = A * HP * WL * A
        ch2 = A * HL * W * A
        dram = pools["dram"]

        self.load_weights(tc, pools["const"])
        import os as _os
        _simpid = _os.environ.get("KSIMPID")
        pid = int(_simpid) if _simpid else nc.partition_id()

        x1_chunk = [dram.tile([C, ch1], BF16, tag=f"x1c{b}", name=_tn(f"x1c{b}")) for b in range(B)]
        x1_full = [dram.tile([NC * C, ch1], BF16, addr_space="Shared",
                             tag=f"x1f{b}", name=_tn(f"x1f{b}")) for b in range(B)]
        bnd_cols = 2 * A * W * A
        y2_chunk = [dram.tile([C, ch2], BF16, tag=f"y2c{b}", name=_tn(f"y2c{b}")) for b in range(B)]
        bnd_snd = [dram.tile([C, bnd_cols], BF16, tag=f"bs{b}", name=_tn(f"bs{b}")) for b in range(B)]
        bnd_all = [dram.tile([NC * C, bnd_cols], BF16, tag=f"ba{b}", name=_tn(f"ba{b}")) for b in range(B)]
        sc2_dram = [dram.tile([C, ch2], F32, tag=f"s2d{b}", name=_tn(f"s2d{b}")) for b in range(B)]

        # ---------------- pass 1 (per b-half)
        sc2_f = []
        y2_sbs = []
        for b in range(B):
            y_perm = pools["io"].tile([C, ch1], BF16, tag="y_perm", name=_tn("y_perm"))
            # zero hpad rows 0 and HP-1:  cols (u, {0,HP-1}, wl, v)
            yv = y_perm.rearrange("p (u h w v) -> p u h w v", u=A, h=HP, w=WL)
            nc.vector.memset(yv[:, :, 0:1, :, :], 0.0)
            nc.vector.memset(yv[:, :, HP - 1:HP, :, :], 0.0)

            def x_src(c0, ntok, b=b):
                t = pools["big"].tile([128, ntok], BF16, tag="x_in", name=_tn("x_in"))
                nc.sync.dma_start(
                    t, self.params["xtok1"][:, b * cfg.SEQ1 * L1 + c0 * L1:
                                            b * cfg.SEQ1 * L1 + c0 * L1 + ntok])
                return t

            def ev_y(c0, nt0, n, ps, y_perm=y_perm):
                # psum [128, n] tokens of seqs starting at s0=c0+nt0//L1
                # seq (v, wl): v = s//WL, wl = s%WL ; token (u, h)
                # y_perm col = u*(HP*WL*A) + (h+1)*(WL*A) + wl*A + v
                s0 = c0 + nt0 // L1
                npair = n // L1
                for i in range(0, npair):
                    s = s0 + i
                    v, wl = s // WL, s % WL
                    dst = y_perm.rearrange(
                        "p (u h w vv) -> p u h w vv", u=A, h=HP, w=WL)
                    nc.vector.tensor_copy(
                        out=dst[:, :, 1:H + 1, wl, v],
                        in_=ps[:, i * L1:(i + 1) * L1].rearrange(
                            "p (u h) -> p u h", h=H))

            self.transformer_half(tc, pools, x_src, ("ind1", "mrow1"), L1,
                                  cfg.SEQ1, ev_y)
            nc.sync.dma_start(x1_chunk[b][:, :], y_perm)
            nc.gpsimd.collective_compute(
                "AllGather", mybir.AluOpType.bypass,
                ins=[x1_chunk[b].opt()], outs=[x1_full[b].opt()],
                replica_groups=[list(range(NC))])

        # ---------------- conv1 + residual -> sc2 ; then pass 2
        for b in range(B):
            cin = pools["io"].tile([C, A * 6 * WPAD * A], BF16, tag="cin", name=_tn("cin"))
            cinv = cin.rearrange("p (u h w v) -> p u h w v", u=A, h=6, w=WPAD)
            nc.vector.memset(cinv[:, :, :, 0:1, :], 0.0)
            nc.vector.memset(cinv[:, :, :, WPAD - 1:WPAD, :], 0.0)
            xf = x1_full[b].rearrange("(wc c) (u h w v) -> wc c u h w v",
                                      c=C, u=A, h=HP, w=WL)
            for wc in range(NC):
                nc.sync.dma_start(
                    cinv[:, :, :, 1 + wc * WL:1 + (wc + 1) * WL, :],
                    xf[wc, :, :, ds(pid * HL, 6), :, :])
            scf = pools["io"].tile([C, ch2], BF16, tag=f"sc2f{b}", name=_tn(f"sc2f{b}"))
            sc2_f.append(scf)

            res_cache = {}

            def ev_c1b(u, hp, nh, ps, b=b, scf=scf, res_cache=res_cache):
                col = u * (HL * W * A) + hp * (W * A)
                n = nh * W * A
                if u not in res_cache:
                    rt = pools["big"].tile([C, HL * W * A], F32, tag="res_u", name=_tn("res_u"))
                    ub = u * (HL * W * A)
                    nc.sync.dma_start(
                        rt, self.params["res1"][:, b * ch2 + ub:
                                                b * ch2 + ub + HL * W * A])
                    res_cache[u] = rt
                sct = pools["big"].tile([C, 2 * W * A], F32, tag="out_t", name=_tn("sc_t"))
                nc.vector.tensor_tensor(
                    out=sct[:, :n], in0=ps[:, :n],
                    in1=res_cache[u][:, hp * W * A: hp * W * A + n], op=OP.add)
                nc.scalar.activation(scf[:, col:col + n], sct[:, :n], AF.Copy)
                nc.sync.dma_start(sc2_dram[b][:, col:col + n], sct[:, :n])

            self.conv_half(pools, cin, ev_c1b)

            # ---- pass 2 on sc2
            y2_sb = pools["io"].tile([C, ch2], BF16, tag=f"y2sb{b}", name=_tn(f"y2sb{b}"))
            y2_sbs.append(y2_sb)

            def x_src2(c0, ntok, scf=scf):
                return scf[:, c0 * L2: c0 * L2 + ntok]

            def ev_y2(c0, nt0, n, ps, y2_sb=y2_sb):
                nc.vector.tensor_copy(
                    out=y2_sb[:, c0 * L2 + nt0: c0 * L2 + nt0 + n],
                    in_=ps[:, :n])

            self.transformer_half(tc, pools, x_src2, ("ind2", "mrow2"), L2,
                                  cfg.SEQ2, ev_y2)
            nc.sync.dma_start(y2_chunk[b][:, :], y2_sb)
            ycv = y2_chunk[b].rearrange("p (u hl wv) -> p u hl wv",
                                        u=A, hl=HL)
            nc.sync.dma_start(
                bnd_snd[b][:, 0:A * W * A].rearrange(
                    "p (u wv) -> p u wv", u=A),
                ycv[:, :, 0, :])
            nc.sync.dma_start(
                bnd_snd[b][:, A * W * A:2 * A * W * A].rearrange(
                    "p (u wv) -> p u wv", u=A),
                ycv[:, :, HL - 1, :])
            nc.gpsimd.collective_compute(
                "AllGather", mybir.AluOpType.bypass,
                ins=[bnd_snd[b].opt()],
                outs=[bnd_all[b].opt()],
                replica_groups=[list(range(NC))])

        # ---------------- conv2 + sc2 -> out
        for b in range(B):
            cin = pools["io"].tile([C, A * 6 * WPAD * A], BF16, tag="cin", name=_tn("cin"))
            cinv = cin.rearrange("p (u h w v) -> p u h w v", u=A, h=6, w=WPAD)
            nc.vector.memset(cinv[:, :, :, 0:1, :], 0.0)
            nc.vector.memset(cinv[:, :, :, WPAD - 1:WPAD, :], 0.0)
            ysv = y2_chunk[b].rearrange("p (u hl w v) -> p u hl w v",
                                        u=A, hl=HL, w=W)
            for u in range(A):
                nc.sync.dma_start(cinv[:, u, 1:1 + HL, 1:1 + W, :], ysv[:, u])
            blkA = (pid + NC - 1) % NC
            blkC = (pid + 1) % NC
            bav_t = bnd_all[b][ds(blkA * C, C), :].rearrange(
                "c (e u w v) -> c e u w v", e=2, u=A, w=W)
            nc.sync.dma_start(cinv[:, :, 0, 1:1 + W, :], bav_t[:, 1])
            bav_b = bnd_all[b][ds(blkC * C, C), :].rearrange(
                "c (e u w v) -> c e u w v", e=2, u=A, w=W)
            nc.sync.dma_start(cinv[:, :, 5, 1:1 + W, :], bav_b[:, 0])
            if isinstance(pid, int):
                if pid < 1:
                    nc.vector.memset(cinv[:, :, 0:1, :, :], 0.0)
                if pid > NC - 2:
                    nc.vector.memset(cinv[:, :, 5:6, :, :], 0.0)
            else:
                with tc.If(pid < 1):
                    nc.vector.memset(cinv[:, :, 0:1, :, :], 0.0)
                with tc.If(pid > NC - 2):
                    nc.vector.memset(cinv[:, :, 5:6, :, :], 0.0)
            res2_cache = {}

            def ev_c2(u, hp, nh, ps, b=b, res2_cache=res2_cache):
                col = u * (HL * W * A) + hp * (W * A)
                n = nh * W * A
                if u not in res2_cache:
                    rt = pools["big"].tile([C, HL * W * A], F32, tag="res_u", name=_tn("res2_u"))
                    ub = u * (HL * W * A)
                    nc.sync.dma_start(
                        rt, sc2_dram[b][:, ub: ub + HL * W * A])
                    res2_cache[u] = rt
                ot = pools["big"].tile([128, 2 * W * A], F32, tag="out_t", name=_tn("out_t"))
                nc.vector.tensor_tensor(
                    out=ot[:, :n], in0=ps[:, :n],
                    in1=res2_cache[u][:, hp * W * A: hp * W * A + n], op=OP.add)
                nc.sync.dma_start(
                    out_ext.ap()[:, b * ch2 + col: b * ch2 + col + n],
                    ot[:, :n])

            self.conv_half(pools, cin, ev_c2)


# ---------------------------------------------------------------- entry point

_CACHE = {}


def kernel(**inputs):
    import numpy as np
    from concourse.bass_utils import run_bass_kernel_spmd
    cfg = Cfg()
    if "nc" not in _CACHE:
        ker = Ker(cfg)
        _CACHE["nc"] = ker.build()
    nc = _CACHE["nc"]
    in_maps = host_prep(cfg, inputs)
    res = run_bass_kernel_spmd(nc, in_maps, core_ids=list(range(cfg.ncores)),
                               trace=False)
    outs = [res.results[i]["out"] for i in range(cfg.ncores)]
    return host_unshard(cfg, outs).astype(np.float32)



# revision 33
# speedup vs baseline: 1.2263x; 1.2263x over previous
"""Bass kernel for nn_AltFilter: dual-pass windowed transformer + conv.

Sharding: pass-1 data-parallel over w (8 chunks of W/8), pass-2 over h.
Between passes: AllGather of pass-1 output (bf16), conv read from gathered.

Layouts (per core, per b-half):
  xtok1   [C, (v, wl, u, h)]        pass-1 token input (host prepped, bf16)
  y_perm  [C, (u, hpad34, wl, v)]   pass-1 output staged for AG (bf16)
  x1_full [8*C, (u, hpad34, wl, v)] AG result, wc-major blocks
  conv1in [C, (u, 6, Wpad, v)]      conv window (bf16) ; w = 4*wc+wl
  sc2     [C, (u, hl, w, v)]        conv1+res (f32/bf16) == pass-2 tokens
  y2      [C, (u, hl, w, v)]        pass-2 out chunk (bf16) -> AG (10-block padded)
  out     [C, (b, u, hl, w, v)]     final (f32)
"""

import numpy as np
from dataclasses import dataclass
from concourse import bass, bacc, tile, mybir
from concourse.bass import ds

BF16 = mybir.dt.bfloat16
F32 = mybir.dt.float32
AF = mybir.ActivationFunctionType
OP = mybir.AluOpType


@dataclass
class Cfg:
    A: int = 5
    C: int = 128
    E: int = 256
    NH: int = 8
    B: int = 2
    H: int = 32
    W: int = 32
    ncores: int = 8
    ch_seqs: int = 5      # sequences per processing chunk
    win: int = 5          # attention half-window (KW//2)

    @property
    def HD(self):
        return self.E // self.NH

    @property
    def WL(self):
        return self.W // self.ncores

    @property
    def HL(self):
        return self.H // self.ncores

    @property
    def L1(self):
        return self.A * self.H       # pass-1 tokens per seq (u, h)

    @property
    def L2(self):
        return self.W * self.A       # pass-2 tokens per seq (w, v)

    @property
    def SEQ1(self):
        return self.A * self.WL      # per-b seqs pass 1 (v, wl)

    @property
    def SEQ2(self):
        return self.A * self.HL      # per-b seqs pass 2 (u, hl)

    @property
    def HPAD(self):
        return self.H + 2



_tname_ctr = [0]


def _tn(tag):
    _tname_ctr[0] += 1
    return f"{tag}_{_tname_ctr[0]}"

def mchunks(L):
    out = []
    o = 0
    while o < L:
        sz = min(128, L - o)
        out.append((o, sz))
        o += sz
    return out


# ---------------------------------------------------------------- host prep

def host_prep(cfg, inputs):
    """Build per-core in_maps from full inputs. Returns list of dicts."""
    import ml_dtypes
    bf = ml_dtypes.bfloat16
    A, C, E, B, H, W = cfg.A, cfg.C, cfg.E, cfg.B, cfg.H, cfg.W
    NC = cfg.ncores
    WL, HL = cfg.WL, cfg.HL

    buf = np.asarray(inputs["buffer"], np.float32)
    buf6 = buf.reshape(B, C, A, A, H, W)                    # b c u v h w

    # pass-1 tokens: [c, b, v, w, u, h] -> core k takes w slice
    xt = np.ascontiguousarray(buf6.transpose(1, 0, 3, 5, 2, 4))  # c b v w u h
    # conv1 residual: [c, b, u, h, w, v] -> core k takes h slice
    rs = np.ascontiguousarray(buf6.transpose(1, 0, 2, 4, 5, 3))  # c b u h w v

    ip = np.asarray(inputs["in_proj"], np.float32)
    sc = 1.0 / np.sqrt(cfg.HD)
    WqT = (ip[0:E].T * sc).astype(bf)
    WkT = ip[E:2 * E].T.astype(bf)
    WvT = ip[2 * E:3 * E].T.astype(bf)
    WinT = np.asarray(inputs["Win"], np.float32).T.astype(bf)       # (C, E)
    WoT = np.asarray(inputs["attn_out_w"], np.float32).T.astype(bf)  # (E, E)
    Wff1T = np.asarray(inputs["ff1"], np.float32).T.astype(bf)       # (E, 2E)
    Wff2T = np.asarray(inputs["ff2"], np.float32).T.astype(bf)       # (2E, E)
    WoutT = np.asarray(inputs["Wout"], np.float32).T.astype(bf)      # (E, C)
    cw = np.asarray(inputs["conv_w"], np.float32)[:, :, 0]           # (O,I,3,3)
    tapT = np.ascontiguousarray(cw.transpose(2, 3, 1, 0)).astype(bf)  # ky kx I O

    def band_mask(L, blk, n_outer, outer_major):
        # tokens: outer_major=True -> l = outer*blk_count... build via indices
        # pass1: l = u*H + h, band over h ; pass2: l = w*A + v, band over w
        l = np.arange(L)
        if outer_major:
            pos = l % blk          # h = l % H  (u-major, h inner)
        else:
            pos = l // n_outer     # w = l // A (w-major, v inner)
        d = np.abs(pos[:, None] - pos[None, :])
        m = np.where(d <= cfg.win, 0.0, -30000.0).astype(np.float32)
        return m.astype(bf)

    mask1 = band_mask(cfg.L1, cfg.H, cfg.A, True)
    mask2 = band_mask(cfg.L2, cfg.W, cfg.A, False)

    # attention-psum mask-init tiles: scA [128, 3L] = 3 head replicas of
    # mask rows 0:128; scB [128, 2L] = head-3 main + tail (rows 0:32);
    # scC [128, 3L] = 3 tail replicas at rows 0:32 (rest zero).
    def mk_masks(mask):
        m_main = np.asarray(mask[0:128, :], np.float32)   # [128, 160]
        m_tail = np.asarray(mask[128:160, :], np.float32)  # [32, 160]
        m3 = np.concatenate([m_main] * 3, axis=1)          # [128, 480]
        tail_pad = np.concatenate(
            [m_tail, np.full((96, m_tail.shape[1]), -30000.0, np.float32)], 0)
        mB = np.concatenate([m_main, tail_pad], axis=1)    # [128, 320]
        tail_z = np.concatenate(
            [m_tail, np.zeros((96, m_tail.shape[1]), np.float32)], 0)
        mC = np.concatenate([tail_z] * 3, axis=1)          # [128, 480]
        return (np.ascontiguousarray(m3).astype(bf),
                np.ascontiguousarray(mB).astype(bf),
                np.ascontiguousarray(mC).astype(bf))

    mask3_1, maskB_1, maskC_1 = mk_masks(mask1)
    mask3_2, maskB_2, maskC_2 = mk_masks(mask2)
    ident = np.eye(128, dtype=np.float32).astype(bf)

    lnw1 = np.asarray(inputs["ln_w"], np.float32).reshape(E // 128, 128).T.copy()
    lnb1 = np.asarray(inputs["ln_b"], np.float32).reshape(E // 128, 128).T.copy()
    lnw2 = np.asarray(inputs["ffn_ln_w"], np.float32).reshape(E // 128, 128).T.copy()
    lnb2 = np.asarray(inputs["ffn_ln_b"], np.float32).reshape(E // 128, 128).T.copy()

    shared = dict(WinT=WinT, WqT=WqT, WkT=WkT, WvT=WvT, WoT=WoT,
                  Wff1T=Wff1T, Wff2T=Wff2T, WoutT=WoutT, tapT=tapT,
                  mask3_1=mask3_1, maskB_1=maskB_1, maskC_1=maskC_1,
                  mask3_2=mask3_2, maskB_2=maskB_2, maskC_2=maskC_2,
                  ident=ident,
                  lnw1=lnw1, lnb1=lnb1, lnw2=lnw2, lnb2=lnb2)

    maps = []
    for k in range(NC):
        xk = xt[:, :, :, k * WL:(k + 1) * WL]   # c b v wl u h
        xk = np.ascontiguousarray(xk).reshape(C, -1).astype(bf)
        rk = rs[:, :, :, k * HL:(k + 1) * HL]   # c b u hl w v
        rk = np.ascontiguousarray(rk).reshape(C, -1).astype(np.float32)
        m = dict(shared)
        m["xtok1"] = xk
        m["res1"] = rk
        maps.append(m)
    return maps


def host_unshard(cfg, outs):
    """outs: list of per-core 'out' arrays [C, B*A*HL*W*A] -> full output."""
    A, C, B, H, W = cfg.A, cfg.C, cfg.B, cfg.H, cfg.W
    o = np.stack(outs)  # j c b u hl w v
    o = o.reshape(cfg.ncores, C, B, A, cfg.HL, W, A)
    o = o.transpose(2, 1, 3, 6, 0, 4, 5)  # b c u v j hl w
    return np.ascontiguousarray(o).reshape(B, C, A * A, H, W)


# ---------------------------------------------------------------- builder

class Ker:
    """Holds nc + pools + weight tiles while building."""

    _tables_pinned = False

    @classmethod
    def _pin_act_tables(cls):
        import os as _os
        if cls._tables_pinned or _os.environ.get("KTAB", "pin") != "pin":
            return
        cls._tables_pinned = True
        import concourse.bacc as _baccmod
        from concourse.hw_specs import get_activation_tables as _gat

        def pinned(arch):
            tabs = _gat(arch)
            keep = "natural_log_exp_and_others"
            mine = {AF.Exp, AF.Ln, AF.Copy, AF.Identity}
            out = {}
            for name, s in tabs.items():
                out[name] = s if name == keep else (s - mine)
            return out

        _baccmod.get_activation_tables = pinned

    def __init__(self, cfg, stage=4, ablate=()):
        self._pin_act_tables()
        self.cfg = cfg
        self.stage = stage  # 1=pass1, 2=+conv1, 3=+pass2, 4=full
        self.ablate = set(ablate)
        self.nc = bacc.Bacc("TRN2", target_bir_lowering=False, debug=False,
                            num_devices=cfg.ncores)

    # -- weights to sbuf ---------------------------------------------------
    def load_weights(self, tc, pool):
        nc, cfg = self.nc, self.cfg
        E = cfg.E

        def wtile(name, K, M):
            p = self.params[name]
            t = pool.tile([128, (K // 128) * M], BF16, tag=name)
            for kt in range(K // 128):
                nc.sync.dma_start(t[:, kt * M:(kt + 1) * M],
                                  p[kt * 128:(kt + 1) * 128, :])
            return t

        self.WinT = wtile("WinT", cfg.C, E)
        self.WqT = wtile("WqT", E, E)
        self.WkT = wtile("WkT", E, E)
        self.WvT = wtile("WvT", E, E)
        self.WoT = wtile("WoT", E, E)
        self.Wff1T = wtile("Wff1T", E, 2 * E)
        self.Wff2T = wtile("Wff2T", 2 * E, E)
        self.WoutT = wtile("WoutT", E, cfg.C)

        tap = self.params["tapT"]  # (3,3,I,O)
        self.taps = pool.tile([128, 9 * 128], BF16, tag="taps", name=_tn("taps"))
        for ky in range(3):
            for kx in range(3):
                i = ky * 3 + kx
                nc.sync.dma_start(self.taps[:, i * 128:(i + 1) * 128],
                                  tap[ky, kx])


        def lnt(name):
            t = pool.tile([128, E // 128], F32, tag=name)
            nc.sync.dma_start(t, self.params[name])
            return t

        self.lnw1, self.lnb1 = lnt("lnw1"), lnt("lnb1")
        self.lnw2, self.lnb2 = lnt("lnw2"), lnt("lnb2")

        self.ones = pool.tile([128, 128], BF16, tag="ones", name=_tn("ones"))
        nc.vector.memset(self.ones[:], 1.0 / E)

        def bftile(name, shape):
            t = pool.tile(shape, BF16, tag=name)
            nc.sync.dma_start(t, self.params[name])
            return t

        self.mask3_1 = bftile("mask3_1", [128, 480])
        self.maskB_1 = bftile("maskB_1", [128, 320])
        self.maskC_1 = bftile("maskC_1", [128, 480])
        self.mask3_2 = bftile("mask3_2", [128, 480])
        self.maskB_2 = bftile("maskB_2", [128, 320])
        self.maskC_2 = bftile("maskC_2", [128, 480])
        self.ident = bftile("ident", [128, 128])
        self.ones_att = pool.tile([128, 32], BF16, tag="ones_att",
                                  name=_tn("ones_att"))
        nc.vector.memset(self.ones_att[:], 1.0)


    # -- generic GEMM over one token chunk --------------------------------
    def gemm(self, psum_pool, wt, K, M, rhs_fn, ntok, nt_sz, evict_fn,
             tag="gemm", name=_tn("gemm")):
        """out[mt] = sum_kt  wt[kt,mt].T @ rhs(kt, nt) ; evict per (mt, nt)."""
        nc = self.nc
        KT, MT = K // 128, M // 128
        for mt in range(MT):
            for nt0 in range(0, ntok, nt_sz):
                n = min(nt_sz, ntok - nt0)
                ps = psum_pool.tile([128, nt_sz], F32, tag=tag)
                for kt in range(KT):
                    nc.tensor.matmul(
                        ps[:, :n],
                        wt[:, kt * M + mt * 128: kt * M + mt * 128 + 128],
                        rhs_fn(kt, nt0, n),
                        start=(kt == 0), stop=(kt == KT - 1))
                evict_fn(mt, nt0, n, ps)

    # -- layernorm over one chunk -----------------------------------------
    def ln_chunk_v2(self, tc, pools, x_f32, lnw, lnb, out_bf, ntok, nt_sz,
                    also_sq_src=None):
        """Feature-major LN. Stats are computed via all-ones matmuls whose
        M=128 stationary replicates sum across all partitions, so no
        partition-broadcast is ever needed."""
        nc, cfg = self.nc, self.cfg
        FT = cfg.E // 128
        x_bf = also_sq_src
        sq = pools["scratch"].tile([128, FT * ntok], BF16, tag="lnsq", name=_tn("lnsq"))
        for ft in range(FT):
            nc.vector.tensor_tensor(
                out=sq[:, ft * ntok:(ft + 1) * ntok],
                in0=x_bf[:, ft * ntok:(ft + 1) * ntok],
                in1=x_bf[:, ft * ntok:(ft + 1) * ntok], op=OP.mult)
        mean = pools["scratch"].tile([128, ntok], F32, tag="lnmean", name=_tn("lnmean"))
        rstd = pools["scratch"].tile([128, ntok], F32, tag="lnrstd", name=_tn("lnrstd"))
        for nt0 in range(0, ntok, nt_sz):
            n = min(nt_sz, ntok - nt0)
            ps_s = pools["stat_psum"].tile([128, nt_sz], F32, tag="lnstat", name=_tn("lnstat"))
            ps_q = pools["stat_psum"].tile([128, nt_sz], F32, tag="lnstat", name=_tn("lnstat"))
            for kt in range(FT):
                nc.tensor.matmul(ps_s[:, :n], self.ones,
                                 x_bf[:, kt * ntok + nt0: kt * ntok + nt0 + n],
                                 start=(kt == 0), stop=(kt == FT - 1))
            for kt in range(FT):
                nc.tensor.matmul(ps_q[:, :n], self.ones,
                                 sq[:, kt * ntok + nt0: kt * ntok + nt0 + n],
                                 start=(kt == 0), stop=(kt == FT - 1))
            nc.vector.tensor_copy(out=mean[:, nt0:nt0 + n], in_=ps_s[:, :n])
            nc.vector.tensor_copy(out=rstd[:, nt0:nt0 + n], in_=ps_q[:, :n])
        # rstd = (E[x^2] + eps - mean^2) ** -0.5
        msq = pools["scratch"].tile([128, ntok], F32, tag="lnmsq", name=_tn("lnmsq"))
        nc.vector.tensor_tensor(out=msq[:], in0=mean[:], in1=mean[:],
                                op=OP.mult)
        nc.vector.scalar_tensor_tensor(
            out=rstd[:], in0=rstd[:], scalar=1e-5, in1=msq[:],
            op0=OP.add, op1=OP.subtract)
        # rstd = exp(-0.5*ln(var)); Ln+Exp share one ACT table so no
        # table reloads against the attention exps.
        if "rsqrt" not in self.ablate:
            nc.scalar.activation(rstd[:], rstd[:], AF.Ln)
            nc.scalar.activation(rstd[:], rstd[:], AF.Exp, scale=-0.5)
        t1 = pools["scratch"].tile([128, nt_sz], F32, tag="lnt1", name=_tn("lnt1"))
        for ft in range(FT):
            for nt0 in range(0, ntok, nt_sz):
                n = min(nt_sz, ntok - nt0)
                nc.vector.tensor_tensor(
                    out=t1[:, :n],
                    in0=x_f32[:, ft * ntok + nt0: ft * ntok + nt0 + n],
                    in1=mean[:, nt0:nt0 + n], op=OP.subtract)
                nc.vector.tensor_tensor(
                    out=t1[:, :n], in0=t1[:, :n],
                    in1=rstd[:, nt0:nt0 + n], op=OP.mult)
                if "lnapply" in self.ablate:
                    nc.scalar.activation(
                        out_bf[:, ft * ntok + nt0: ft * ntok + nt0 + n],
                        t1[:, :n], AF.Copy)
                else:
                    nc.scalar.activation(
                        out_bf[:, ft * ntok + nt0: ft * ntok + nt0 + n],
                        t1[:, :n], AF.Identity, bias=lnb[:, ft:ft + 1],
                        scale=lnw[:, ft:ft + 1])

    # -- attention for one chunk of sequences ------------------------------
    def attention(self, pools, q_s, k_s, v_m, v_t, masks, o_bf, nseq, L,
                  ntok, ntokmax):
        """Per (seq, head-quad): scores for 4 heads into two psum banks
        (scA = heads 0-2 main [128, 3L]; scB = head-3 main [128, L] + the
        4 stacked [32, L] m-tails), mask added via one identity matmul from
        a precomputed SBUF tile, one Exp per bank, then col-tiled AV into an
        O|D bank: rows 32c = head c, cols 0:L = o, L:2L = denominator.
        One reciprocal + one multiply normalize all 4 heads."""
        nc, cfg = self.nc, self.cfg
        E = cfg.E
        mask3_t, maskB_t, maskC_t = masks
        for s in range(nseq):
            for qd in range(2):
                scA = pools["sc"].tile([128, 3 * L], F32, tag="scA", name=_tn("scA"))
                scB = pools["sc"].tile([128, 2 * L], F32, tag="scB", name=_tn("scB"))
                scC = pools["sc"].tile([32, 3 * L], F32, tag="scC", name=_tn("scC"))
                nc.tensor.matmul(scA, self.ident, mask3_t, start=True,
                                 stop=False, skip_group_check=True)
                nc.tensor.matmul(scB, self.ident, maskB_t, start=True,
                                 stop=False, skip_group_check=True)
                nc.tensor.matmul(scC, self.ident[:, 0:32], maskC_t,
                                 start=True, stop=False,
                                 skip_group_check=True)
                base = qd * ntokmax + s * L
                for c in range(4):
                    ks = k_s[4 * qd + c]
                    qs = q_s[4 * qd + c][0:32, s * L: (s + 1) * L]
                    out_main = (scA[:, L * c:L * (c + 1)] if c < 3
                                else scB[:, 0:L])
                    nc.tensor.matmul(out_main,
                                     ks[0:32, s * L: s * L + 128], qs,
                                     start=False, stop=True,
                                     skip_group_check=True)
                    tdst = (scC[0:32, L * c:L * (c + 1)] if c < 3
                            else scB[0:32, L:2 * L])
                    nc.tensor.matmul(tdst,
                                     ks[0:32, s * L + 128: (s + 1) * L], qs,
                                     start=False, stop=True,
                                     skip_group_check=True)
                exA = pools["exp"].tile([128, 3 * L], BF16, tag="exA", name=_tn("exA"))
                nc.scalar.activation(exA, scA, AF.Exp)
                exB = pools["exp"].tile([128, 2 * L], BF16, tag="exB", name=_tn("exB"))
                nc.scalar.activation(exB, scB, AF.Exp)
                exC = pools["exp"].tile([32, 3 * L], BF16, tag="exC", name=_tn("exC"))
                nc.scalar.activation(exC, scC, AF.Exp)
                od = pools["od"].tile([128, 2 * L], F32, tag="od", name=_tn("od"))
                for c in range(4):
                    r0 = 32 * c
                    hg = 4 * qd + c
                    exm = exA[:, L * c:L * (c + 1)] if c < 3 else exB[:, 0:L]
                    ext = (exC[0:32, L * c:L * (c + 1)] if c < 3
                           else exB[0:32, L:2 * L])
                    vm = v_m[:, s * E + 32 * hg: s * E + 32 * hg + 32]
                    vt = v_t[0:32, s * E + 32 * hg: s * E + 32 * hg + 32]
                    nc.tensor.matmul(od[r0:r0 + 32, 0:L], vm, exm,
                                     start=True, stop=False,
                                     tile_position=(0, r0))
                    nc.tensor.matmul(od[r0:r0 + 32, 0:L], vt, ext,
                                     start=False, stop=True,
                                     tile_position=(0, r0))
                    nc.tensor.matmul(od[r0:r0 + 32, L:2 * L],
                                     self.ones_att[0:128, 0:32], exm,
                                     start=True, stop=False,
                                     tile_position=(0, r0))
                    nc.tensor.matmul(od[r0:r0 + 32, L:2 * L],
                                     self.ones_att[0:32, 0:32], ext,
                                     start=False, stop=True,
                                     tile_position=(0, r0))
                rec = pools["scratch"].tile([128, L], F32, tag="rec", name=_tn("rec"))
                nc.vector.reciprocal_approx_fast(rec, od[:, L:2 * L])
                nc.vector.tensor_tensor(
                    out=o_bf[:, qd * ntok + s * L: qd * ntok + (s + 1) * L],
                    in0=od[:, 0:L], in1=rec, op=OP.mult)

    # -- one transformer pass over one b-half ------------------------------
    def transformer_half(self, tc, pools, x_src_fn, masks, L, nseq_b,
                         evict_y_fn):
        """x_src_fn(c0, ntok) -> bf16 [128, ntok] input token tile (Win rhs).
        evict_y_fn(s_global_pair_index, nt0, n, psum) writes final y."""
        nc, cfg = self.nc, self.cfg
        E = cfg.E
        FT = E // 128
        CH = cfg.ch_seqs
        NTs = 3 * L            # token tile = 3 seqs (psum [128,480] f32 fits a bank)
        ntokmax = CH * L
        # q_s/k_s: one 32-row tile per head (base partition 0), cols = tokens.
        # Avoids row-tiled matmuls (broken on HW); K=32 score MMs all run at
        # array rows 0:32.
        q_s = [pools["qk"].tile([32, ntokmax], BF16, tag=f"q_s{h}",
                                name=_tn(f"q_s{h}")) for h in range(cfg.NH)]
        k_s = [pools["qk"].tile([32, ntokmax], BF16, tag=f"k_s{h}",
                                name=_tn(f"k_s{h}")) for h in range(cfg.NH)]
        for c0 in range(0, nseq_b, CH):
            ns = min(CH, nseq_b - c0)
            ntok = ns * L
            x_bf = x_src_fn(c0, ntok)
            tok_f = pools["big1"].tile([128, FT * ntok], F32, tag="tok_f", name=_tn("tok_f"))
            tok_bf = pools["big"].tile([128, FT * ntok], BF16, tag="tok_bf", name=_tn("tok_bf"))

            def ev_tok(mt, nt0, n, ps):
                nc.vector.tensor_copy(
                    out=tok_f[:, mt * ntok + nt0: mt * ntok + nt0 + n],
                    in_=ps[:, :n])
                nc.scalar.activation(
                    tok_bf[:, mt * ntok + nt0: mt * ntok + nt0 + n],
                    ps[:, :n], AF.Copy)

            self.gemm(pools["psum"], self.WinT, cfg.C, E,
                      lambda kt, nt0, n: x_bf[:, nt0:nt0 + n],
                      ntok, NTs, ev_tok, tag="gemm", name=_tn("gemm"))

            tn = pools["big"].tile([128, FT * ntok], BF16, tag="tn", name=_tn("tn"))
            if "ln" in self.ablate:
                nc.vector.tensor_copy(out=tn, in_=tok_bf)
            else:
                self.ln_chunk_v2(tc, pools, tok_f, self.lnw1, self.lnb1, tn,
                                 ntok, NTs, also_sq_src=tok_bf)

            def mk_ev(dst, use_act):
                def ev(mt, nt0, n, ps):
                    for c in range(4):
                        d = dst[4 * mt + c][0:32, nt0: nt0 + n]
                        if use_act:
                            nc.scalar.activation(
                                d, ps[32 * c:32 * c + 32, :n], AF.Copy)
                        else:
                            nc.vector.tensor_copy(
                                out=d, in_=ps[32 * c:32 * c + 32, :n])
                return ev

            rhs_tn = lambda kt, nt0, n: tn[:, kt * ntok + nt0: kt * ntok + nt0 + n]
            self.gemm(pools["psum"], self.WqT, E, E, rhs_tn, ntok, NTs,
                      mk_ev(q_s, True), tag="gemm", name=_tn("gemm"))
            self.gemm(pools["psum"], self.WkT, E, E, rhs_tn, ntok, NTs,
                      mk_ev(k_s, False), tag="gemm", name=_tn("gemm"))

            # V token-major [tok, E]
            v_m = pools["big"].tile([128, CH * E], BF16, tag="v_m", name=_tn("v_m"))
            has_tail = L > 128
            v_t = None
            if has_tail:
                v_t = pools["big"].tile([32, CH * E], BF16,
                                        tag="v_t", name=_tn("v_t"))
            Lm = min(128, L)
            for s in range(ns):
                ps = pools["psum"].tile([128, E], F32, tag="gemm", name=_tn("gemm"))
                for kt in range(FT):
                    nc.tensor.matmul(
                        ps[0:Lm, :],
                        tok_bf[:, kt * ntok + s * L: kt * ntok + s * L + Lm],
                        self.WvT[:, kt * E:(kt + 1) * E],
                        start=(kt == 0), stop=(kt == FT - 1))
                nc.vector.tensor_copy(
                    out=v_m[0:Lm, s * E:(s + 1) * E], in_=ps[0:Lm, :])
            if has_tail:
                tl = L - 128
                for s in range(ns):
                    ps = pools["psum"].tile([128, E], F32, tag="gemm", name=_tn("gemm"))
                    for kt in range(FT):
                        nc.tensor.matmul(
                            ps[0:tl, :],
                            tok_bf[:, kt * ntok + s * L + 128:
                                   kt * ntok + s * L + 128 + tl],
                            self.WvT[:, kt * E:(kt + 1) * E],
                            start=(kt == 0), stop=(kt == FT - 1))
                    nc.scalar.activation(
                        v_t[0:tl, s * E:(s + 1) * E], ps[0:tl, :], AF.Copy)

            o_bf = pools["big"].tile([128, FT * ntok], BF16, tag="o_bf", name=_tn("o_bf"))
            if "attn" in self.ablate:
                nc.vector.tensor_copy(out=o_bf, in_=tn)
            else:
                self.attention(pools, q_s, k_s, v_m, v_t, masks,
                               o_bf, ns, L, ntok, ntokmax)

            # out-proj + residual
            outp_f = pools["big1"].tile([128, FT * ntok], F32, tag="outp_f", name=_tn("outp_f"))
            outp_bf = pools["big"].tile([128, FT * ntok], BF16, tag="outp_bf", name=_tn("outp_bf"))

            def ev_outp(mt, nt0, n, ps):
                nc.vector.tensor_tensor(
                    out=outp_f[:, mt * ntok + nt0: mt * ntok + nt0 + n],
                    in0=ps[:, :n],
                    in1=tok_f[:, mt * ntok + nt0: mt * ntok + nt0 + n],
                    op=OP.add)
                nc.scalar.activation(
                    outp_bf[:, mt * ntok + nt0: mt * ntok + nt0 + n],
                    outp_f[:, mt * ntok + nt0: mt * ntok + nt0 + n], AF.Copy)

            rhs_o = lambda kt, nt0, n: o_bf[:, kt * ntok + nt0: kt * ntok + nt0 + n]
            self.gemm(pools["psum"], self.WoT, E, E, rhs_o, ntok, NTs,
                      ev_outp, tag="gemm", name=_tn("gemm"))

            tn2 = pools["big"].tile([128, FT * ntok], BF16, tag="tn2", name=_tn("tn2"))
            if "ln" in self.ablate:
                nc.vector.tensor_copy(out=tn2, in_=outp_bf)
            else:
                self.ln_chunk_v2(tc, pools, outp_f, self.lnw2, self.lnb2, tn2,
                                 ntok, NTs, also_sq_src=outp_bf)

            ffh = pools["big"].tile([128, 2 * FT * ntok], BF16, tag="ffh", name=_tn("ffh"))

            def ev_ffh(mt, nt0, n, ps):
                nc.vector.tensor_relu(
                    out=ffh[:, mt * ntok + nt0: mt * ntok + nt0 + n],
                    in_=ps[:, :n])

            rhs_tn2 = lambda kt, nt0, n: tn2[:, kt * ntok + nt0: kt * ntok + nt0 + n]
            self.gemm(pools["psum"], self.Wff1T, E, 2 * E, rhs_tn2, ntok, NTs,
                      ev_ffh, tag="gemm", name=_tn("gemm"))

            res2_f = pools["big1"].tile([128, FT * ntok], F32, tag="res2_f", name=_tn("res2_f"))
            res2_bf = pools["big"].tile([128, FT * ntok], BF16, tag="res2_bf", name=_tn("res2_bf"))

            def ev_ffo(mt, nt0, n, ps):
                nc.vector.tensor_tensor(
                    out=res2_f[:, mt * ntok + nt0: mt * ntok + nt0 + n],
                    in0=ps[:, :n],
                    in1=outp_f[:, mt * ntok + nt0: mt * ntok + nt0 + n],
                    op=OP.add)
                nc.scalar.activation(
                    res2_bf[:, mt * ntok + nt0: mt * ntok + nt0 + n],
                    res2_f[:, mt * ntok + nt0: mt * ntok + nt0 + n], AF.Copy)

            rhs_ffh = lambda kt, nt0, n: ffh[:, kt * ntok + nt0: kt * ntok + nt0 + n]
            self.gemm(pools["psum"], self.Wff2T, 2 * E, E, rhs_ffh, ntok, NTs,
                      ev_ffo, tag="gemm", name=_tn("gemm"))

            rhs_r2 = lambda kt, nt0, n: res2_bf[:, kt * ntok + nt0: kt * ntok + nt0 + n]
            self.gemm(pools["psum"], self.WoutT, E, cfg.C, rhs_r2, ntok, NTs,
                      lambda mt, nt0, n, ps: evict_y_fn(c0, nt0, n, ps),
                      tag="gemm", name=_tn("gemm"))

    # -- conv 3x3 over (h,w) for one b-half --------------------------------
    def conv_half(self, pools, cin, evict_fn, rows=None):
        """cin: sbuf [128, A*6*WPAD*A(v)] bf16 window tile (w-padded).
        out pixels (u, hl, w, v); evict_fn(u, hp, n, psum) with pixel tiles
        [128, 2*W*A(v)].  rows: list of (hp, nh) output-row groups; default
        covers all HL rows in pairs."""
        nc, cfg = self.nc, self.cfg
        A, W = cfg.A, cfg.W
        WP = W + 2
        if rows is None:
            rows = [(hp, min(2, cfg.HL - hp)) for hp in range(0, cfg.HL, 2)]
        cv = cin.rearrange("p (u h w v) -> p u h w v", u=A, h=6, w=WP)
        for u in range(A):
            for hp, nh in rows:
                ps = pools["psum"].tile([128, 2 * W * A], F32, tag="gemm", name=_tn("gemm"))
                first = True
                for dy in range(3):
                    for dx in range(3):
                        rhs = cv[:, u, hp + dy:hp + dy + nh, dx:dx + W, :]
                        nc.tensor.matmul(
                            ps[:, :nh * W * A],
                            self.taps[:, (dy * 3 + dx) * 128:(dy * 3 + dx + 1) * 128],
                            rhs,
                            start=first, stop=(dy == 2 and dx == 2))
                        first = False
                evict_fn(u, hp, nh, ps)

    # -- full graph --------------------------------------------------------
    def build(self):
        cfg = self.cfg
        nc = self.nc
        A, C, E, B, H, W = cfg.A, cfg.C, cfg.E, cfg.B, cfg.H, cfg.W
        L1, L2 = cfg.L1, cfg.L2
        FT = E // 128
        WL, HL, HP = cfg.WL, cfg.HL, cfg.HPAD
        NC = cfg.ncores
        WPAD = W + 2
        chunk1_cols = A * HP * WL * A          # y_perm cols (u hpad wl v)
        chunk2_cols = A * HL * W * A           # y2 cols (u hl w v)

        # ---- dram params
        P = {}
        def par(name, shape, dt):
            P[name] = nc.dram_tensor(name, shape, dt, kind="ExternalInput").ap()
        par("xtok1", [C, B * cfg.SEQ1 * L1], BF16)
        par("res1", [C, B * chunk2_cols], F32)
        par("WinT", [C, E], BF16)
        for n in ("WqT", "WkT", "WvT", "WoT"):
            par(n, [E, E], BF16)
        par("Wff1T", [E, 2 * E], BF16)
        par("Wff2T", [2 * E, E], BF16)
        par("WoutT", [E, C], BF16)
        par("tapT", [3, 3, C, C], BF16)
        par("mask3_1", [128, 480], BF16)
        par("maskB_1", [128, 320], BF16)
        par("maskC_1", [128, 480], BF16)
        par("mask3_2", [128, 480], BF16)
        par("maskB_2", [128, 320], BF16)
        par("maskC_2", [128, 480], BF16)
        par("ident", [128, 128], BF16)
        for n in ("lnw1", "lnb1", "lnw2", "lnb2"):
            par(n, [128, FT], F32)
        self.params = P
        out_ext = nc.dram_tensor("out", [C, B * chunk2_cols], F32,
                                 kind="ExternalOutput")

        with tile.TileContext(nc) as tc:
            import contextlib
            with contextlib.ExitStack() as ctx:
                pools = {}
                pools["const"] = ctx.enter_context(
                    tc.tile_pool(name="const", bufs=1))
                pools["big"] = ctx.enter_context(
                    tc.tile_pool(name="big", bufs=2))
                pools["big1"] = ctx.enter_context(
                    tc.tile_pool(name="big1", bufs=1))
                pools["qk"] = ctx.enter_context(
                    tc.tile_pool(name="qk", bufs=1))
                pools["scratch"] = ctx.enter_context(
                    tc.tile_pool(name="scratch", bufs=1))
                pools["io"] = ctx.enter_context(
                    tc.tile_pool(name="io", bufs=1))
                pools["psum"] = ctx.enter_context(
                    tc.tile_pool(name="psum", bufs=2, space="PSUM"))
                pools["stat_psum"] = ctx.enter_context(
                    tc.tile_pool(name="stat_psum", bufs=2, space="PSUM"))
                pools["sc"] = ctx.enter_context(
                    tc.tile_pool(name="sc", bufs=1, space="PSUM"))
                pools["od"] = ctx.enter_context(
                    tc.tile_pool(name="od", bufs=1, space="PSUM"))
                pools["exp"] = ctx.enter_context(
                    tc.tile_pool(name="exp", bufs=2))
                pools["dram"] = ctx.enter_context(
                    tc.tile_pool(name="dram", bufs=1, space="DRAM"))
                self._build_body(tc, pools, out_ext)
        nc.compile()
        return nc

    def _build_body(self, tc, pools, out_ext):
        nc, cfg = self.nc, self.cfg
        A, C, E, B, H, W = cfg.A, cfg.C, cfg.E, cfg.B, cfg.H, cfg.W
        L1, L2 = cfg.L1, cfg.L2
        FT = E // 128
        WL, HL, HP = cfg.WL, cfg.HL, cfg.HPAD
        NC = cfg.ncores
        WPAD = W + 2
        ch1 = A * HP * WL * A
        ch2 = A * HL * W * A
        dram = pools["dram"]

        self.load_weights(tc, pools["const"])
        import os as _os
        _simpid = _os.environ.get("KSIMPID")
        pid = int(_simpid) if _simpid else nc.partition_id()

        chS = A * 6 * WL * A   # per-dest all-to-all block (h-window + halo)
        x1_send = [dram.tile([NC * C, chS], BF16, tag=f"x1s{b}", name=_tn(f"x1s{b}")) for b in range(B)]
        x1_recv = [dram.tile([NC * C, chS], BF16, addr_space="Shared",
                             tag=f"x1r{b}", name=_tn(f"x1r{b}")) for b in range(B)]
        bnd_cols = 2 * A * W * A
        y2_chunk = [dram.tile([C, ch2], BF16, tag=f"y2c{b}", name=_tn(f"y2c{b}")) for b in range(B)]
        bnd_snd = [dram.tile([C, bnd_cols], BF16, tag=f"bs{b}", name=_tn(f"bs{b}")) for b in range(B)]
        bnd_all = [dram.tile([NC * C, bnd_cols], BF16, tag=f"ba{b}", name=_tn(f"ba{b}")) for b in range(B)]
        sc2_dram = [dram.tile([C, ch2], F32, tag=f"s2d{b}", name=_tn(f"s2d{b}")) for b in range(B)]

        # ---------------- pass 1 (per b-half)
        sc2_f = []
        y2_sbs = []
        for b in range(B):
            y_perm = pools["io"].tile([C, ch1], BF16, tag="y_perm", name=_tn("y_perm"))
            # zero hpad rows 0 and HP-1:  cols (u, {0,HP-1}, wl, v)
            yv = y_perm.rearrange("p (u h w v) -> p u h w v", u=A, h=HP, w=WL)
            nc.vector.memset(yv[:, :, 0:1, :, :], 0.0)
            nc.vector.memset(yv[:, :, HP - 1:HP, :, :], 0.0)

            def x_src(c0, ntok, b=b):
                t = pools["big"].tile([128, ntok], BF16, tag="x_in", name=_tn("x_in"))
                nc.sync.dma_start(
                    t, self.params["xtok1"][:, b * cfg.SEQ1 * L1 + c0 * L1:
                                            b * cfg.SEQ1 * L1 + c0 * L1 + ntok])
                return t

            def ev_y(c0, nt0, n, ps, y_perm=y_perm):
                # psum [128, n] tokens of seqs starting at s0=c0+nt0//L1
                # seq (v, wl): v = s//WL, wl = s%WL ; token (u, h)
                # y_perm col = u*(HP*WL*A) + (h+1)*(WL*A) + wl*A + v
                s0 = c0 + nt0 // L1
                npair = n // L1
                for i in range(0, npair):
                    s = s0 + i
                    v, wl = s // WL, s % WL
                    dst = y_perm.rearrange(
                        "p (u h w vv) -> p u h w vv", u=A, h=HP, w=WL)
                    nc.vector.tensor_copy(
                        out=dst[:, :, 1:H + 1, wl, v],
                        in_=ps[:, i * L1:(i + 1) * L1].rearrange(
                            "p (u h) -> p u h", h=H))

            self.transformer_half(tc, pools, x_src,
                                  (self.mask3_1, self.maskB_1, self.maskC_1),
                                  L1, cfg.SEQ1, ev_y)
            yv_s = y_perm.rearrange("p (u h w v) -> p u h w v",
                                    u=A, h=HP, w=WL)
            for j in range(NC):
                nc.sync.dma_start(
                    x1_send[b][j * C:(j + 1) * C, :].rearrange(
                        "c (u h w v) -> c u h w v", u=A, h=6, w=WL),
                    yv_s[:, :, j * HL: j * HL + 6, :, :])
            nc.gpsimd.collective_compute(
                "AllToAll", mybir.AluOpType.bypass,
                ins=[x1_send[b].opt()], outs=[x1_recv[b].opt()],
                replica_groups=[list(range(NC))])

        # ---------------- conv1 + residual -> sc2 ; then pass 2
        for b in range(B):
            cin = pools["io"].tile([C, A * 6 * WPAD * A], BF16, tag="cin", name=_tn("cin"))
            cinv = cin.rearrange("p (u h w v) -> p u h w v", u=A, h=6, w=WPAD)
            nc.vector.memset(cinv[:, :, :, 0:1, :], 0.0)
            nc.vector.memset(cinv[:, :, :, WPAD - 1:WPAD, :], 0.0)
            xr = x1_recv[b].rearrange("(wc c) (u h w v) -> wc c u h w v",
                                      c=C, u=A, h=6, w=WL)
            for wc in range(NC):
                nc.sync.dma_start(
                    cinv[:, :, :, 1 + wc * WL:1 + (wc + 1) * WL, :],
                    xr[wc])
            scf = pools["io"].tile([C, ch2], BF16, tag=f"sc2f{b}", name=_tn(f"sc2f{b}"))
            sc2_f.append(scf)

            res_cache = {}

            def ev_c1b(u, hp, nh, ps, b=b, scf=scf, res_cache=res_cache):
                col = u * (HL * W * A) + hp * (W * A)
                n = nh * W * A
                if u not in res_cache:
                    rt = pools["big"].tile([C, HL * W * A], F32, tag="res_u", name=_tn("res_u"))
                    ub = u * (HL * W * A)
                    nc.sync.dma_start(
                        rt, self.params["res1"][:, b * ch2 + ub:
                                                b * ch2 + ub + HL * W * A])
                    res_cache[u] = rt
                sct = pools["big"].tile([C, 2 * W * A], F32, tag="out_t", name=_tn("sc_t"))
                nc.vector.tensor_tensor(
                    out=sct[:, :n], in0=ps[:, :n],
                    in1=res_cache[u][:, hp * W * A: hp * W * A + n], op=OP.add)
                nc.scalar.activation(scf[:, col:col + n], sct[:, :n], AF.Copy)
                nc.sync.dma_start(sc2_dram[b][:, col:col + n], sct[:, :n])

            self.conv_half(pools, cin, ev_c1b)

            # ---- pass 2 on sc2
            y2_sb = pools["io"].tile([C, ch2], BF16, tag=f"y2sb{b}", name=_tn(f"y2sb{b}"))
            y2_sbs.append(y2_sb)

            def x_src2(c0, ntok, scf=scf):
                return scf[:, c0 * L2: c0 * L2 + ntok]

            def ev_y2(c0, nt0, n, ps, y2_sb=y2_sb):
                nc.vector.tensor_copy(
                    out=y2_sb[:, c0 * L2 + nt0: c0 * L2 + nt0 + n],
                    in_=ps[:, :n])

            self.transformer_half(tc, pools, x_src2,
                                  (self.mask3_2, self.maskB_2, self.maskC_2),
                                  L2, cfg.SEQ2, ev_y2)
            nc.sync.dma_start(y2_chunk[b][:, :], y2_sb)
            ycv = y2_chunk[b].rearrange("p (u hl wv) -> p u hl wv",
                                        u=A, hl=HL)
            nc.sync.dma_start(
                bnd_snd[b][:, 0:A * W * A].rearrange(
                    "p (u wv) -> p u wv", u=A),
                ycv[:, :, 0, :])
            nc.sync.dma_start(
                bnd_snd[b][:, A * W * A:2 * A * W * A].rearrange(
                    "p (u wv) -> p u wv", u=A),
                ycv[:, :, HL - 1, :])
            nc.gpsimd.collective_compute(
                "AllGather", mybir.AluOpType.bypass,
                ins=[bnd_snd[b].opt()],
                outs=[bnd_all[b].opt()],
                replica_groups=[list(range(NC))])

        # ---------------- conv2 + sc2 -> out
        for b in range(B):
            cin = pools["io"].tile([C, A * 6 * WPAD * A], BF16, tag="cin", name=_tn("cin"))
            cinv = cin.rearrange("p (u h w v) -> p u h w v", u=A, h=6, w=WPAD)
            nc.vector.memset(cinv[:, :, :, 0:1, :], 0.0)
            nc.vector.memset(cinv[:, :, :, WPAD - 1:WPAD, :], 0.0)
            ysv = y2_chunk[b].rearrange("p (u hl w v) -> p u hl w v",
                                        u=A, hl=HL, w=W)
            for u in range(A):
                nc.sync.dma_start(cinv[:, u, 1:1 + HL, 1:1 + W, :], ysv[:, u])
            blkA = (pid + NC - 1) % NC
            blkC = (pid + 1) % NC
            bav_t = bnd_all[b][ds(blkA * C, C), :].rearrange(
                "c (e u w v) -> c e u w v", e=2, u=A, w=W)
            nc.sync.dma_start(cinv[:, :, 0, 1:1 + W, :], bav_t[:, 1])
            bav_b = bnd_all[b][ds(blkC * C, C), :].rearrange(
                "c (e u w v) -> c e u w v", e=2, u=A, w=W)
            nc.sync.dma_start(cinv[:, :, 5, 1:1 + W, :], bav_b[:, 0])
            if isinstance(pid, int):
                if pid < 1:
                    nc.vector.memset(cinv[:, :, 0:1, :, :], 0.0)
                if pid > NC - 2:
                    nc.vector.memset(cinv[:, :, 5:6, :, :], 0.0)
            else:
                with tc.If(pid < 1):
                    nc.vector.memset(cinv[:, :, 0:1, :, :], 0.0)
                with tc.If(pid > NC - 2):
                    nc.vector.memset(cinv[:, :, 5:6, :, :], 0.0)
            res2_cache = {}

            def ev_c2(u, hp, nh, ps, b=b, res2_cache=res2_cache):
                col = u * (HL * W * A) + hp * (W * A)
                n = nh * W * A
                if u not in res2_cache:
                    rt = pools["big"].tile([C, HL * W * A], F32, tag="res_u", name=_tn("res2_u"))
                    ub = u * (HL * W * A)
                    nc.sync.dma_start(
                        rt, sc2_dram[b][:, ub: ub + HL * W * A])
                    res2_cache[u] = rt
                ot = pools["big"].tile([128, 2 * W * A], F32, tag="out_t", name=_tn("out_t"))
                nc.vector.tensor_tensor(
                    out=ot[:, :n], in0=ps[:, :n],
                    in1=res2_cache[u][:, hp * W * A: hp * W * A + n], op=OP.add)
                nc.sync.dma_start(
                    out_ext.ap()[:, b * ch2 + col: b * ch2 + col + n],
                    ot[:, :n])

            self.conv_half(pools, cin, ev_c2)


# ---------------------------------------------------------------- entry point

_CACHE = {}


def kernel(**inputs):
    import numpy as np
    from concourse.bass_utils import run_bass_kernel_spmd
    import os as _os
    cfg = Cfg()
    if "nc" not in _CACHE:
        abl = tuple(x for x in _os.environ.get("KABL", "").split(",") if x)
        ker = Ker(cfg, ablate=abl)
        _CACHE["nc"] = ker.build()
    nc = _CACHE["nc"]
    in_maps = host_prep(cfg, inputs)
    res = run_bass_kernel_spmd(nc, in_maps, core_ids=list(range(cfg.ncores)),
                               trace=False)
    outs = [res.results[i]["out"] for i in range(cfg.ncores)]
    return host_unshard(cfg, outs).astype(np.float32)



# revision 47
# speedup vs baseline: 1.2790x; 1.0430x over previous
"""Bass kernel for nn_AltFilter: dual-pass windowed transformer + conv.

Sharding: pass-1 data-parallel over w (8 chunks of W/8), pass-2 over h.
Between passes: AllGather of pass-1 output (bf16), conv read from gathered.

Layouts (per core, per b-half):
  xtok1   [C, (v, wl, u, h)]        pass-1 token input (host prepped, bf16)
  y_perm  [C, (u, hpad34, wl, v)]   pass-1 output staged for AG (bf16)
  x1_full [8*C, (u, hpad34, wl, v)] AG result, wc-major blocks
  conv1in [C, (u, 6, Wpad, v)]      conv window (bf16) ; w = 4*wc+wl
  sc2     [C, (u, hl, w, v)]        conv1+res (f32/bf16) == pass-2 tokens
  y2      [C, (u, hl, w, v)]        pass-2 out chunk (bf16) -> AG (10-block padded)
  out     [C, (b, u, hl, w, v)]     final (f32)
"""

import numpy as np
from dataclasses import dataclass
from concourse import bass, bacc, tile, mybir
from concourse.bass import ds

BF16 = mybir.dt.bfloat16
F32 = mybir.dt.float32
AF = mybir.ActivationFunctionType
OP = mybir.AluOpType


@dataclass
class Cfg:
    A: int = 5
    C: int = 128
    E: int = 256
    NH: int = 8
    B: int = 2
    H: int = 32
    W: int = 32
    ncores: int = 8
    ch_seqs: int = 5      # sequences per processing chunk
    win: int = 5          # attention half-window (KW//2)

    @property
    def HD(self):
        return self.E // self.NH

    @property
    def WL(self):
        return self.W // self.ncores

    @property
    def HL(self):
        return self.H // self.ncores

    @property
    def L1(self):
        return self.A * self.H       # pass-1 tokens per seq (u, h)

    @property
    def L2(self):
        return self.W * self.A       # pass-2 tokens per seq (w, v)

    @property
    def SEQ1(self):
        return self.A * self.WL      # per-b seqs pass 1 (v, wl)

    @property
    def SEQ2(self):
        return self.A * self.HL      # per-b seqs pass 2 (u, hl)

    @property
    def HPAD(self):
        return self.H + 2



_tname_ctr = [0]


def _tn(tag):
    _tname_ctr[0] += 1
    return f"{tag}_{_tname_ctr[0]}"

def mchunks(L):
    out = []
    o = 0
    while o < L:
        sz = min(128, L - o)
        out.append((o, sz))
        o += sz
    return out


# ---------------------------------------------------------------- host prep

def host_prep(cfg, inputs):
    """Build per-core in_maps from full inputs. Returns list of dicts."""
    import ml_dtypes
    bf = ml_dtypes.bfloat16
    A, C, E, B, H, W = cfg.A, cfg.C, cfg.E, cfg.B, cfg.H, cfg.W
    NC = cfg.ncores
    WL, HL = cfg.WL, cfg.HL

    buf = np.asarray(inputs["buffer"], np.float32)
    buf6 = buf.reshape(B, C, A, A, H, W)                    # b c u v h w

    # pass-1 tokens: [c, b, v, w, u, h] -> core k takes w slice
    xt = np.ascontiguousarray(buf6.transpose(1, 0, 3, 5, 2, 4))  # c b v w u h
    # conv1 residual: [c, b, u, h, w, v] -> core k takes h slice
    rs = np.ascontiguousarray(buf6.transpose(1, 0, 2, 4, 5, 3))  # c b u h w v

    ip = np.asarray(inputs["in_proj"], np.float32)
    sc = 1.0 / np.sqrt(cfg.HD)
    WqT = (ip[0:E].T * sc).astype(bf)
    WkT = ip[E:2 * E].T.astype(bf)
    WvT = ip[2 * E:3 * E].T.astype(bf)
    WinT = np.asarray(inputs["Win"], np.float32).T.astype(bf)       # (C, E)
    WoT = np.asarray(inputs["attn_out_w"], np.float32).T.astype(bf)  # (E, E)
    Wff1T = np.asarray(inputs["ff1"], np.float32).T.astype(bf)       # (E, 2E)
    Wff2T = np.asarray(inputs["ff2"], np.float32).T.astype(bf)       # (2E, E)
    WoutT = np.asarray(inputs["Wout"], np.float32).T.astype(bf)      # (E, C)
    cw = np.asarray(inputs["conv_w"], np.float32)[:, :, 0]           # (O,I,3,3)
    tapT = np.ascontiguousarray(cw.transpose(2, 3, 1, 0)).astype(bf)  # ky kx I O

    def band_mask(L, blk, n_outer, outer_major):
        # tokens: outer_major=True -> l = outer*blk_count... build via indices
        # pass1: l = u*H + h, band over h ; pass2: l = w*A + v, band over w
        l = np.arange(L)
        if outer_major:
            pos = l % blk          # h = l % H  (u-major, h inner)
        else:
            pos = l // n_outer     # w = l // A (w-major, v inner)
        d = np.abs(pos[:, None] - pos[None, :])
        m = np.where(d <= cfg.win, 0.0, -30000.0).astype(np.float32)
        return m.astype(bf)

    mask1 = band_mask(cfg.L1, cfg.H, cfg.A, True)
    mask2 = band_mask(cfg.L2, cfg.W, cfg.A, False)

    # attention-psum mask-init tiles: scA [128, 3L] = 3 head replicas of
    # mask rows 0:128; scB [128, 2L] = head-3 main + tail (rows 0:32);
    # scC [128, 3L] = 3 tail replicas at rows 0:32 (rest zero).
    def mk_masks(mask):
        m_main = np.asarray(mask[0:128, :], np.float32)   # [128, 160]
        m_tail = np.asarray(mask[128:160, :], np.float32)  # [32, 160]
        m3 = np.concatenate([m_main] * 3, axis=1)          # [128, 480]
        tail_pad = np.concatenate(
            [m_tail, np.full((96, m_tail.shape[1]), -30000.0, np.float32)], 0)
        mB = np.concatenate([m_main, tail_pad], axis=1)    # [128, 320]
        tail_z = np.concatenate(
            [m_tail, np.zeros((96, m_tail.shape[1]), np.float32)], 0)
        mC = np.concatenate([tail_z] * 3, axis=1)          # [128, 480]
        return (np.ascontiguousarray(m3).astype(bf),
                np.ascontiguousarray(mB).astype(bf),
                np.ascontiguousarray(mC).astype(bf))

    mask3_1, maskB_1, maskC_1 = mk_masks(mask1)
    mask3_2, maskB_2, maskC_2 = mk_masks(mask2)
    ident = np.eye(128, dtype=np.float32).astype(bf)

    lnw1 = np.asarray(inputs["ln_w"], np.float32).reshape(E // 128, 128).T.copy()
    lnb1 = np.asarray(inputs["ln_b"], np.float32).reshape(E // 128, 128).T.copy()
    lnw2 = np.asarray(inputs["ffn_ln_w"], np.float32).reshape(E // 128, 128).T.copy()
    lnb2 = np.asarray(inputs["ffn_ln_b"], np.float32).reshape(E // 128, 128).T.copy()

    shared = dict(WinT=WinT, WqT=WqT, WkT=WkT, WvT=WvT, WoT=WoT,
                  Wff1T=Wff1T, Wff2T=Wff2T, WoutT=WoutT, tapT=tapT,
                  mask3_1=mask3_1, maskB_1=maskB_1, maskC_1=maskC_1,
                  mask3_2=mask3_2, maskB_2=maskB_2, maskC_2=maskC_2,
                  ident=ident,
                  lnw1=lnw1, lnb1=lnb1, lnw2=lnw2, lnb2=lnb2)

    maps = []
    for k in range(NC):
        xk = xt[:, :, :, k * WL:(k + 1) * WL]   # c b v wl u h
        xk = np.ascontiguousarray(xk).reshape(C, -1).astype(bf)
        rk = rs[:, :, :, k * HL:(k + 1) * HL]   # c b u hl w v
        rk = np.ascontiguousarray(rk).reshape(C, -1).astype(np.float32)
        m = dict(shared)
        m["xtok1"] = xk
        m["res1"] = rk
        maps.append(m)
    return maps


def host_unshard(cfg, outs):
    """outs: list of per-core 'out' arrays [C, B*A*HL*W*A] -> full output."""
    A, C, B, H, W = cfg.A, cfg.C, cfg.B, cfg.H, cfg.W
    o = np.stack(outs)  # j c b u hl w v
    o = o.reshape(cfg.ncores, C, B, A, cfg.HL, W, A)
    o = o.transpose(2, 1, 3, 6, 0, 4, 5)  # b c u v j hl w
    return np.ascontiguousarray(o).reshape(B, C, A * A, H, W)


# ---------------------------------------------------------------- builder

class Ker:
    """Holds nc + pools + weight tiles while building."""

    _tables_pinned = False

    @classmethod
    def _pin_act_tables(cls):
        import os as _os
        if cls._tables_pinned or _os.environ.get("KTAB", "pin") != "pin":
            return
        cls._tables_pinned = True
        import concourse.bacc as _baccmod
        from concourse.hw_specs import get_activation_tables as _gat

        def pinned(arch):
            tabs = _gat(arch)
            keep = "natural_log_exp_and_others"
            mine = {AF.Exp, AF.Ln, AF.Copy, AF.Identity}
            out = {}
            for name, s in tabs.items():
                out[name] = s if name == keep else (s - mine)
            return out

        _baccmod.get_activation_tables = pinned

    def __init__(self, cfg, stage=4, ablate=()):
        self._pin_act_tables()
        self.cfg = cfg
        self.stage = stage  # 1=pass1, 2=+conv1, 3=+pass2, 4=full
        self.ablate = set(ablate)
        self.nc = bacc.Bacc("TRN2", target_bir_lowering=False, debug=False,
                            num_devices=cfg.ncores)

    # -- weights to sbuf ---------------------------------------------------
    def load_weights(self, tc, pool):
        nc, cfg = self.nc, self.cfg
        E = cfg.E

        def wtile(name, K, M):
            p = self.params[name]
            t = pool.tile([128, (K // 128) * M], BF16, tag=name)
            for kt in range(K // 128):
                nc.sync.dma_start(t[:, kt * M:(kt + 1) * M],
                                  p[kt * 128:(kt + 1) * 128, :])
            return t

        self.WinT = wtile("WinT", cfg.C, E)
        self.WqT = wtile("WqT", E, E)
        self.WkT = wtile("WkT", E, E)
        self.WvT = wtile("WvT", E, E)
        self.WoT = wtile("WoT", E, E)
        self.Wff1T = wtile("Wff1T", E, 2 * E)
        self.Wff2T = wtile("Wff2T", 2 * E, E)
        self.WoutT = wtile("WoutT", E, cfg.C)

        tap = self.params["tapT"]  # (3,3,I,O)
        self.taps = pool.tile([128, 9 * 128], BF16, tag="taps", name=_tn("taps"))
        for ky in range(3):
            for kx in range(3):
                i = ky * 3 + kx
                nc.sync.dma_start(self.taps[:, i * 128:(i + 1) * 128],
                                  tap[ky, kx])


        def lnt(name):
            t = pool.tile([128, E // 128], F32, tag=name)
            nc.sync.dma_start(t, self.params[name])
            return t

        self.lnw1, self.lnb1 = lnt("lnw1"), lnt("lnb1")
        self.lnw2, self.lnb2 = lnt("lnw2"), lnt("lnb2")

        self.ones = pool.tile([128, 128], BF16, tag="ones", name=_tn("ones"))
        nc.vector.memset(self.ones[:], 1.0 / E)

        def bftile(name, shape):
            t = pool.tile(shape, BF16, tag=name)
            nc.sync.dma_start(t, self.params[name])
            return t

        self.mask3_1 = bftile("mask3_1", [128, 480])
        self.maskB_1 = bftile("maskB_1", [128, 320])
        self.maskC_1 = bftile("maskC_1", [128, 480])
        self.mask3_2 = bftile("mask3_2", [128, 480])
        self.maskB_2 = bftile("maskB_2", [128, 320])
        self.maskC_2 = bftile("maskC_2", [128, 480])
        self.ident = bftile("ident", [128, 128])
        self.ones_att = pool.tile([128, 32], BF16, tag="ones_att",
                                  name=_tn("ones_att"))
        nc.vector.memset(self.ones_att[:], 1.0)


    # -- generic GEMM over one token chunk --------------------------------
    def gemm(self, psum_pool, wt, K, M, rhs_fn, ntok, nt_sz, evict_fn,
             tag="gemm", name=_tn("gemm")):
        """out[mt] = sum_kt  wt[kt,mt].T @ rhs(kt, nt) ; evict per (mt, nt)."""
        nc = self.nc
        KT, MT = K // 128, M // 128
        for mt in range(MT):
            for nt0 in range(0, ntok, nt_sz):
                n = min(nt_sz, ntok - nt0)
                ps = psum_pool.tile([128, nt_sz], F32, tag=tag)
                for kt in range(KT):
                    nc.tensor.matmul(
                        ps[:, :n],
                        wt[:, kt * M + mt * 128: kt * M + mt * 128 + 128],
                        rhs_fn(kt, nt0, n),
                        start=(kt == 0), stop=(kt == KT - 1))
                evict_fn(mt, nt0, n, ps)

    # -- layernorm over one chunk -----------------------------------------
    def ln_chunk_v2(self, tc, pools, x_f32, lnw, lnb, out_bf, ntok, nt_sz,
                    also_sq_src=None):
        """Feature-major LN. Stats are computed via all-ones matmuls whose
        M=128 stationary replicates sum across all partitions, so no
        partition-broadcast is ever needed."""
        nc, cfg = self.nc, self.cfg
        FT = cfg.E // 128
        x_bf = also_sq_src
        sq = pools["scratch"].tile([128, FT * ntok], BF16, tag="lnsq", name=_tn("lnsq"))
        for ft in range(FT):
            nc.vector.tensor_tensor(
                out=sq[:, ft * ntok:(ft + 1) * ntok],
                in0=x_bf[:, ft * ntok:(ft + 1) * ntok],
                in1=x_bf[:, ft * ntok:(ft + 1) * ntok], op=OP.mult)
        mean = pools["scratch"].tile([128, ntok], F32, tag="lnmean", name=_tn("lnmean"))
        rstd = pools["scratch"].tile([128, ntok], F32, tag="lnrstd", name=_tn("lnrstd"))
        for nt0 in range(0, ntok, nt_sz):
            n = min(nt_sz, ntok - nt0)
            ps_s = pools["stat_psum"].tile([128, nt_sz], F32, tag="lnstat", name=_tn("lnstat"))
            ps_q = pools["stat_psum"].tile([128, nt_sz], F32, tag="lnstat", name=_tn("lnstat"))
            for kt in range(FT):
                nc.tensor.matmul(ps_s[:, :n], self.ones,
                                 x_bf[:, kt * ntok + nt0: kt * ntok + nt0 + n],
                                 start=(kt == 0), stop=(kt == FT - 1))
            for kt in range(FT):
                nc.tensor.matmul(ps_q[:, :n], self.ones,
                                 sq[:, kt * ntok + nt0: kt * ntok + nt0 + n],
                                 start=(kt == 0), stop=(kt == FT - 1))
            nc.vector.tensor_copy(out=mean[:, nt0:nt0 + n], in_=ps_s[:, :n])
            nc.vector.tensor_copy(out=rstd[:, nt0:nt0 + n], in_=ps_q[:, :n])
        # rstd = (E[x^2] + eps - mean^2) ** -0.5
        msq = pools["scratch"].tile([128, ntok], F32, tag="lnmsq", name=_tn("lnmsq"))
        nc.vector.tensor_tensor(out=msq[:], in0=mean[:], in1=mean[:],
                                op=OP.mult)
        nc.vector.scalar_tensor_tensor(
            out=rstd[:], in0=rstd[:], scalar=1e-5, in1=msq[:],
            op0=OP.add, op1=OP.subtract)
        # rstd = exp(-0.5*ln(var)); Ln+Exp share one ACT table so no
        # table reloads against the attention exps.
        if "rsqrt" not in self.ablate:
            nc.scalar.activation(rstd[:], rstd[:], AF.Ln)
            nc.scalar.activation(rstd[:], rstd[:], AF.Exp, scale=-0.5)
        t1 = pools["scratch"].tile([128, nt_sz], F32, tag="lnt1", name=_tn("lnt1"))
        for ft in range(FT):
            for nt0 in range(0, ntok, nt_sz):
                n = min(nt_sz, ntok - nt0)
                nc.vector.tensor_tensor(
                    out=t1[:, :n],
                    in0=x_f32[:, ft * ntok + nt0: ft * ntok + nt0 + n],
                    in1=mean[:, nt0:nt0 + n], op=OP.subtract)
                nc.vector.tensor_tensor(
                    out=t1[:, :n], in0=t1[:, :n],
                    in1=rstd[:, nt0:nt0 + n], op=OP.mult)
                if "lnapply" in self.ablate:
                    nc.scalar.activation(
                        out_bf[:, ft * ntok + nt0: ft * ntok + nt0 + n],
                        t1[:, :n], AF.Copy)
                else:
                    nc.scalar.activation(
                        out_bf[:, ft * ntok + nt0: ft * ntok + nt0 + n],
                        t1[:, :n], AF.Identity, bias=lnb[:, ft:ft + 1],
                        scale=lnw[:, ft:ft + 1])

    # -- attention for one chunk of sequences ------------------------------
    def attention(self, pools, q_s, k_s, v_m, v_t, masks, o_bf, nseq, L,
                  ntok, ntokmax):
        """Per (seq, head-quad): scores for 4 heads into two psum banks
        (scA = heads 0-2 main [128, 3L]; scB = head-3 main [128, L] + the
        4 stacked [32, L] m-tails), mask added via one identity matmul from
        a precomputed SBUF tile, one Exp per bank, then col-tiled AV into an
        O|D bank: rows 32c = head c, cols 0:L = o, L:2L = denominator.
        One reciprocal + one multiply normalize all 4 heads."""
        nc, cfg = self.nc, self.cfg
        E = cfg.E
        mask3_t, maskB_t, maskC_t = masks
        for s in range(nseq):
            for qd in range(2):
                scA = pools["sc"].tile([128, 3 * L], F32, tag="scA", name=_tn("scA"))
                scB = pools["sc"].tile([128, 2 * L], F32, tag="scB", name=_tn("scB"))
                scC = pools["sc"].tile([32, 3 * L], F32, tag="scC", name=_tn("scC"))
                nc.tensor.matmul(scA, self.ident, mask3_t, start=True,
                                 stop=False, skip_group_check=True)
                nc.tensor.matmul(scB, self.ident, maskB_t, start=True,
                                 stop=False, skip_group_check=True)
                nc.tensor.matmul(scC, self.ident[:, 0:32], maskC_t,
                                 start=True, stop=False,
                                 skip_group_check=True)
                base = qd * ntokmax + s * L
                for c in range(4):
                    ks = k_s[4 * qd + c]
                    qs = q_s[4 * qd + c][0:32, s * L: (s + 1) * L]
                    out_main = (scA[:, L * c:L * (c + 1)] if c < 3
                                else scB[:, 0:L])
                    nc.tensor.matmul(out_main,
                                     ks[0:32, s * L: s * L + 128], qs,
                                     start=False, stop=True,
                                     skip_group_check=True)
                    tdst = (scC[0:32, L * c:L * (c + 1)] if c < 3
                            else scB[0:32, L:2 * L])
                    nc.tensor.matmul(tdst,
                                     ks[0:32, s * L + 128: (s + 1) * L], qs,
                                     start=False, stop=True,
                                     skip_group_check=True)
                exA = pools["exp"].tile([128, 3 * L], BF16, tag="exA", name=_tn("exA"))
                nc.scalar.activation(exA, scA, AF.Exp)
                exB = pools["exp"].tile([128, 2 * L], BF16, tag="exB", name=_tn("exB"))
                nc.scalar.activation(exB, scB, AF.Exp)
                exC = pools["exp"].tile([32, 3 * L], BF16, tag="exC", name=_tn("exC"))
                nc.scalar.activation(exC, scC, AF.Exp)
                od = pools["od"].tile([128, 2 * L], F32, tag="od", name=_tn("od"))
                for c in range(4):
                    r0 = 32 * c
                    hg = 4 * qd + c
                    exm = exA[:, L * c:L * (c + 1)] if c < 3 else exB[:, 0:L]
                    ext = (exC[0:32, L * c:L * (c + 1)] if c < 3
                           else exB[0:32, L:2 * L])
                    vm = v_m[:, s * E + 32 * hg: s * E + 32 * hg + 32]
                    vt = v_t[0:32, s * E + 32 * hg: s * E + 32 * hg + 32]
                    nc.tensor.matmul(od[r0:r0 + 32, 0:L], vm, exm,
                                     start=True, stop=False,
                                     tile_position=(0, r0))
                    nc.tensor.matmul(od[r0:r0 + 32, 0:L], vt, ext,
                                     start=False, stop=True,
                                     tile_position=(0, r0))
                    nc.tensor.matmul(od[r0:r0 + 32, L:2 * L],
                                     self.ones_att[0:128, 0:32], exm,
                                     start=True, stop=False,
                                     tile_position=(0, r0))
                    nc.tensor.matmul(od[r0:r0 + 32, L:2 * L],
                                     self.ones_att[0:32, 0:32], ext,
                                     start=False, stop=True,
                                     tile_position=(0, r0))
                rec = pools["scratch"].tile([128, L], F32, tag="rec", name=_tn("rec"))
                nc.vector.reciprocal_approx_fast(rec, od[:, L:2 * L])
                nc.vector.tensor_tensor(
                    out=o_bf[:, qd * ntok + s * L: qd * ntok + (s + 1) * L],
                    in0=od[:, 0:L], in1=rec, op=OP.mult)

    # -- one transformer pass over one b-half ------------------------------
    def transformer_half(self, tc, pools, x_src_fn, masks, L, nseq_b,
                         evict_y_fn):
        """x_src_fn(c0, ntok) -> bf16 [128, ntok] input token tile (Win rhs).
        evict_y_fn(s_global_pair_index, nt0, n, psum) writes final y."""
        nc, cfg = self.nc, self.cfg
        E = cfg.E
        FT = E // 128
        CH = cfg.ch_seqs
        NTs = 3 * L            # token tile = 3 seqs (psum [128,480] f32 fits a bank)
        ntokmax = CH * L
        # q_s/k_s: one 32-row tile per head (base partition 0), cols = tokens.
        # Avoids row-tiled matmuls (broken on HW); K=32 score MMs all run at
        # array rows 0:32.
        q_s = [pools["qk"].tile([32, ntokmax], BF16, tag=f"q_s{h}",
                                name=_tn(f"q_s{h}")) for h in range(cfg.NH)]
        k_s = [pools["qk"].tile([32, ntokmax], BF16, tag=f"k_s{h}",
                                name=_tn(f"k_s{h}")) for h in range(cfg.NH)]
        for c0 in range(0, nseq_b, CH):
            ns = min(CH, nseq_b - c0)
            ntok = ns * L
            x_bf = x_src_fn(c0, ntok)
            tok_f = pools["big1"].tile([128, FT * ntok], F32, tag="tok_f", name=_tn("tok_f"))
            tok_bf = pools["big"].tile([128, FT * ntok], BF16, tag="tok_bf", name=_tn("tok_bf"))

            def ev_tok(mt, nt0, n, ps):
                nc.vector.tensor_copy(
                    out=tok_f[:, mt * ntok + nt0: mt * ntok + nt0 + n],
                    in_=ps[:, :n])
                nc.scalar.activation(
                    tok_bf[:, mt * ntok + nt0: mt * ntok + nt0 + n],
                    ps[:, :n], AF.Copy)

            self.gemm(pools["psum"], self.WinT, cfg.C, E,
                      lambda kt, nt0, n: x_bf[:, nt0:nt0 + n],
                      ntok, NTs, ev_tok, tag="gemm", name=_tn("gemm"))

            tn = pools["big"].tile([128, FT * ntok], BF16, tag="tn", name=_tn("tn"))
            if "ln" in self.ablate:
                nc.vector.tensor_copy(out=tn, in_=tok_bf)
            else:
                self.ln_chunk_v2(tc, pools, tok_f, self.lnw1, self.lnb1, tn,
                                 ntok, NTs, also_sq_src=tok_bf)

            def mk_ev(dst, use_act):
                def ev(mt, nt0, n, ps):
                    for c in range(4):
                        d = dst[4 * mt + c][0:32, nt0: nt0 + n]
                        if use_act:
                            nc.scalar.activation(
                                d, ps[32 * c:32 * c + 32, :n], AF.Copy)
                        else:
                            nc.vector.tensor_copy(
                                out=d, in_=ps[32 * c:32 * c + 32, :n])
                return ev

            rhs_tn = lambda kt, nt0, n: tn[:, kt * ntok + nt0: kt * ntok + nt0 + n]
            self.gemm(pools["psum"], self.WqT, E, E, rhs_tn, ntok, NTs,
                      mk_ev(q_s, True), tag="gemm", name=_tn("gemm"))
            self.gemm(pools["psum"], self.WkT, E, E, rhs_tn, ntok, NTs,
                      mk_ev(k_s, False), tag="gemm", name=_tn("gemm"))

            # V token-major [tok, E]
            v_m = pools["big"].tile([128, CH * E], BF16, tag="v_m", name=_tn("v_m"))
            has_tail = L > 128
            v_t = None
            if has_tail:
                v_t = pools["big"].tile([32, CH * E], BF16,
                                        tag="v_t", name=_tn("v_t"))
            Lm = min(128, L)
            for s in range(ns):
                ps = pools["psum"].tile([128, E], F32, tag="gemm", name=_tn("gemm"))
                for kt in range(FT):
                    nc.tensor.matmul(
                        ps[0:Lm, :],
                        tok_bf[:, kt * ntok + s * L: kt * ntok + s * L + Lm],
                        self.WvT[:, kt * E:(kt + 1) * E],
                        start=(kt == 0), stop=(kt == FT - 1))
                nc.vector.tensor_copy(
                    out=v_m[0:Lm, s * E:(s + 1) * E], in_=ps[0:Lm, :])
            if has_tail:
                tl = L - 128
                for s in range(ns):
                    ps = pools["psum"].tile([128, E], F32, tag="gemm", name=_tn("gemm"))
                    for kt in range(FT):
                        nc.tensor.matmul(
                            ps[0:tl, :],
                            tok_bf[:, kt * ntok + s * L + 128:
                                   kt * ntok + s * L + 128 + tl],
                            self.WvT[:, kt * E:(kt + 1) * E],
                            start=(kt == 0), stop=(kt == FT - 1))
                    nc.scalar.activation(
                        v_t[0:tl, s * E:(s + 1) * E], ps[0:tl, :], AF.Copy)

            o_bf = pools["big"].tile([128, FT * ntok], BF16, tag="o_bf", name=_tn("o_bf"))
            if "attn" in self.ablate:
                nc.vector.tensor_copy(out=o_bf, in_=tn)
            else:
                self.attention(pools, q_s, k_s, v_m, v_t, masks,
                               o_bf, ns, L, ntok, ntokmax)

            # out-proj + residual
            outp_f = pools["big1"].tile([128, FT * ntok], F32, tag="outp_f", name=_tn("outp_f"))
            outp_bf = pools["big"].tile([128, FT * ntok], BF16, tag="outp_bf", name=_tn("outp_bf"))

            def ev_outp(mt, nt0, n, ps):
                nc.vector.tensor_tensor(
                    out=outp_f[:, mt * ntok + nt0: mt * ntok + nt0 + n],
                    in0=ps[:, :n],
                    in1=tok_f[:, mt * ntok + nt0: mt * ntok + nt0 + n],
                    op=OP.add)
                nc.scalar.activation(
                    outp_bf[:, mt * ntok + nt0: mt * ntok + nt0 + n],
                    outp_f[:, mt * ntok + nt0: mt * ntok + nt0 + n], AF.Copy)

            rhs_o = lambda kt, nt0, n: o_bf[:, kt * ntok + nt0: kt * ntok + nt0 + n]
            self.gemm(pools["psum"], self.WoT, E, E, rhs_o, ntok, NTs,
                      ev_outp, tag="gemm", name=_tn("gemm"))

            tn2 = pools["big"].tile([128, FT * ntok], BF16, tag="tn2", name=_tn("tn2"))
            if "ln" in self.ablate:
                nc.vector.tensor_copy(out=tn2, in_=outp_bf)
            else:
                self.ln_chunk_v2(tc, pools, outp_f, self.lnw2, self.lnb2, tn2,
                                 ntok, NTs, also_sq_src=outp_bf)

            ffh = pools["big"].tile([128, 2 * FT * ntok], BF16, tag="ffh", name=_tn("ffh"))

            def ev_ffh(mt, nt0, n, ps):
                nc.vector.tensor_relu(
                    out=ffh[:, mt * ntok + nt0: mt * ntok + nt0 + n],
                    in_=ps[:, :n])

            rhs_tn2 = lambda kt, nt0, n: tn2[:, kt * ntok + nt0: kt * ntok + nt0 + n]
            self.gemm(pools["psum"], self.Wff1T, E, 2 * E, rhs_tn2, ntok, NTs,
                      ev_ffh, tag="gemm", name=_tn("gemm"))

            res2_f = pools["big1"].tile([128, FT * ntok], F32, tag="res2_f", name=_tn("res2_f"))
            res2_bf = pools["big"].tile([128, FT * ntok], BF16, tag="res2_bf", name=_tn("res2_bf"))

            def ev_ffo(mt, nt0, n, ps):
                nc.vector.tensor_tensor(
                    out=res2_f[:, mt * ntok + nt0: mt * ntok + nt0 + n],
                    in0=ps[:, :n],
                    in1=outp_f[:, mt * ntok + nt0: mt * ntok + nt0 + n],
                    op=OP.add)
                nc.scalar.activation(
                    res2_bf[:, mt * ntok + nt0: mt * ntok + nt0 + n],
                    res2_f[:, mt * ntok + nt0: mt * ntok + nt0 + n], AF.Copy)

            rhs_ffh = lambda kt, nt0, n: ffh[:, kt * ntok + nt0: kt * ntok + nt0 + n]
            self.gemm(pools["psum"], self.Wff2T, 2 * E, E, rhs_ffh, ntok, NTs,
                      ev_ffo, tag="gemm", name=_tn("gemm"))

            rhs_r2 = lambda kt, nt0, n: res2_bf[:, kt * ntok + nt0: kt * ntok + nt0 + n]
            self.gemm(pools["psum"], self.WoutT, E, cfg.C, rhs_r2, ntok, NTs,
                      lambda mt, nt0, n, ps: evict_y_fn(c0, nt0, n, ps),
                      tag="gemm", name=_tn("gemm"))

    # -- conv 3x3 over (h,w) for one b-half --------------------------------
    def conv_half(self, pools, cin, evict_fn, rows=None):
        """cin: sbuf [128, A*6*WPAD*A(v)] bf16 window tile (w-padded).
        out pixels (u, hl, w, v); evict_fn(u, hp, n, psum) with pixel tiles
        [128, 2*W*A(v)].  rows: list of (hp, nh) output-row groups; default
        covers all HL rows in pairs."""
        nc, cfg = self.nc, self.cfg
        A, W = cfg.A, cfg.W
        WP = W + 2
        if rows is None:
            rows = [(hp, min(2, cfg.HL - hp)) for hp in range(0, cfg.HL, 2)]
        cv = cin.rearrange("p (u h w v) -> p u h w v", u=A, h=6, w=WP)
        for u in range(A):
            for hp, nh in rows:
                ps = pools["psum"].tile([128, 2 * W * A], F32, tag="gemm", name=_tn("gemm"))
                first = True
                for dy in range(3):
                    for dx in range(3):
                        rhs = cv[:, u, hp + dy:hp + dy + nh, dx:dx + W, :]
                        nc.tensor.matmul(
                            ps[:, :nh * W * A],
                            self.taps[:, (dy * 3 + dx) * 128:(dy * 3 + dx + 1) * 128],
                            rhs,
                            start=first, stop=(dy == 2 and dx == 2))
                        first = False
                evict_fn(u, hp, nh, ps)

    # -- full graph --------------------------------------------------------
    def build(self):
        cfg = self.cfg
        nc = self.nc
        A, C, E, B, H, W = cfg.A, cfg.C, cfg.E, cfg.B, cfg.H, cfg.W
        L1, L2 = cfg.L1, cfg.L2
        FT = E // 128
        WL, HL, HP = cfg.WL, cfg.HL, cfg.HPAD
        NC = cfg.ncores
        WPAD = W + 2
        chunk1_cols = A * HP * WL * A          # y_perm cols (u hpad wl v)
        chunk2_cols = A * HL * W * A           # y2 cols (u hl w v)

        # ---- dram params
        P = {}
        def par(name, shape, dt):
            P[name] = nc.dram_tensor(name, shape, dt, kind="ExternalInput").ap()
        par("xtok1", [C, B * cfg.SEQ1 * L1], BF16)
        par("res1", [C, B * chunk2_cols], F32)
        par("WinT", [C, E], BF16)
        for n in ("WqT", "WkT", "WvT", "WoT"):
            par(n, [E, E], BF16)
        par("Wff1T", [E, 2 * E], BF16)
        par("Wff2T", [2 * E, E], BF16)
        par("WoutT", [E, C], BF16)
        par("tapT", [3, 3, C, C], BF16)
        par("mask3_1", [128, 480], BF16)
        par("maskB_1", [128, 320], BF16)
        par("maskC_1", [128, 480], BF16)
        par("mask3_2", [128, 480], BF16)
        par("maskB_2", [128, 320], BF16)
        par("maskC_2", [128, 480], BF16)
        par("ident", [128, 128], BF16)
        for n in ("lnw1", "lnb1", "lnw2", "lnb2"):
            par(n, [128, FT], F32)
        self.params = P
        out_ext = nc.dram_tensor("out", [C, B * chunk2_cols], F32,
                                 kind="ExternalOutput")

        with tile.TileContext(nc) as tc:
            import contextlib
            with contextlib.ExitStack() as ctx:
                pools = {}
                pools["const"] = ctx.enter_context(
                    tc.tile_pool(name="const", bufs=1))
                pools["big"] = ctx.enter_context(
                    tc.tile_pool(name="big", bufs=2))
                pools["big1"] = ctx.enter_context(
                    tc.tile_pool(name="big1", bufs=1))
                pools["qk"] = ctx.enter_context(
                    tc.tile_pool(name="qk", bufs=1))
                pools["scratch"] = ctx.enter_context(
                    tc.tile_pool(name="scratch", bufs=1))
                pools["io"] = ctx.enter_context(
                    tc.tile_pool(name="io", bufs=1))
                pools["psum"] = ctx.enter_context(
                    tc.tile_pool(name="psum", bufs=2, space="PSUM"))
                pools["stat_psum"] = ctx.enter_context(
                    tc.tile_pool(name="stat_psum", bufs=2, space="PSUM"))
                pools["sc"] = ctx.enter_context(
                    tc.tile_pool(name="sc", bufs=1, space="PSUM"))
                pools["od"] = ctx.enter_context(
                    tc.tile_pool(name="od", bufs=1, space="PSUM"))
                pools["exp"] = ctx.enter_context(
                    tc.tile_pool(name="exp", bufs=2))
                pools["dram"] = ctx.enter_context(
                    tc.tile_pool(name="dram", bufs=1, space="DRAM"))
                self._build_body(tc, pools, out_ext)
        nc.compile()
        return nc

    def _build_body(self, tc, pools, out_ext):
        nc, cfg = self.nc, self.cfg
        A, C, E, B, H, W = cfg.A, cfg.C, cfg.E, cfg.B, cfg.H, cfg.W
        L1, L2 = cfg.L1, cfg.L2
        FT = E // 128
        WL, HL, HP = cfg.WL, cfg.HL, cfg.HPAD
        NC = cfg.ncores
        WPAD = W + 2
        ch1 = A * HP * WL * A
        ch2 = A * HL * W * A
        dram = pools["dram"]

        self.load_weights(tc, pools["const"])
        import os as _os
        _simpid = _os.environ.get("KSIMPID")
        pid = int(_simpid) if _simpid else nc.partition_id()

        x1_chunk = [dram.tile([C, ch1], BF16, tag=f"x1c{b}", name=_tn(f"x1c{b}")) for b in range(B)]
        x1_full = [dram.tile([NC * C, ch1], BF16, addr_space="Shared",
                             tag=f"x1f{b}", name=_tn(f"x1f{b}")) for b in range(B)]
        bnd_cols = 2 * A * W * A
        bnd_snd = [dram.tile([C, bnd_cols], BF16, tag=f"bs{b}", name=_tn(f"bs{b}")) for b in range(B)]
        bnd_all = [dram.tile([NC * C, bnd_cols], BF16, tag=f"ba{b}", name=_tn(f"ba{b}")) for b in range(B)]
        sc2_dram = [dram.tile([C, ch2], F32, tag=f"s2d{b}", name=_tn(f"s2d{b}")) for b in range(B)]

        # ---------------- pass 1 (per b-half)
        sc2_f = []
        y2_sbs = []
        for b in range(B):
            y_perm = pools["io"].tile([C, ch1], BF16, tag="y_perm", name=_tn("y_perm"))
            # zero hpad rows 0 and HP-1:  cols (u, {0,HP-1}, wl, v)
            yv = y_perm.rearrange("p (u h w v) -> p u h w v", u=A, h=HP, w=WL)
            nc.vector.memset(yv[:, :, 0:1, :, :], 0.0)
            nc.vector.memset(yv[:, :, HP - 1:HP, :, :], 0.0)

            def x_src(c0, ntok, b=b):
                t = pools["big"].tile([128, ntok], BF16, tag="x_in", name=_tn("x_in"))
                nc.sync.dma_start(
                    t, self.params["xtok1"][:, b * cfg.SEQ1 * L1 + c0 * L1:
                                            b * cfg.SEQ1 * L1 + c0 * L1 + ntok])
                return t

            def ev_y(c0, nt0, n, ps, y_perm=y_perm):
                # psum [128, n] tokens of seqs starting at s0=c0+nt0//L1
                # seq (v, wl): v = s//WL, wl = s%WL ; token (u, h)
                # y_perm col = u*(HP*WL*A) + (h+1)*(WL*A) + wl*A + v
                s0 = c0 + nt0 // L1
                npair = n // L1
                for i in range(0, npair):
                    s = s0 + i
                    v, wl = s // WL, s % WL
                    dst = y_perm.rearrange(
                        "p (u h w vv) -> p u h w vv", u=A, h=HP, w=WL)
                    nc.vector.tensor_copy(
                        out=dst[:, :, 1:H + 1, wl, v],
                        in_=ps[:, i * L1:(i + 1) * L1].rearrange(
                            "p (u h) -> p u h", h=H))

            self.transformer_half(tc, pools, x_src,
                                  (self.mask3_1, self.maskB_1, self.maskC_1),
                                  L1, cfg.SEQ1, ev_y)
            nc.sync.dma_start(x1_chunk[b][:, :], y_perm)
            nc.gpsimd.collective_compute(
                "AllGather", mybir.AluOpType.bypass,
                ins=[x1_chunk[b].opt()], outs=[x1_full[b].opt()],
                replica_groups=[list(range(NC))])

        # ---------------- conv1 + residual -> sc2 ; then pass 2
        for b in range(B):
            cin = pools["io"].tile([C, A * 6 * WPAD * A], BF16, tag="cin", name=_tn("cin"))
            cinv = cin.rearrange("p (u h w v) -> p u h w v", u=A, h=6, w=WPAD)
            nc.vector.memset(cinv[:, :, :, 0:1, :], 0.0)
            nc.vector.memset(cinv[:, :, :, WPAD - 1:WPAD, :], 0.0)
            xf = x1_full[b].rearrange("(wc c) (u h w v) -> wc c u h w v",
                                      c=C, u=A, h=HP, w=WL)
            for wc in range(NC):
                nc.sync.dma_start(
                    cinv[:, :, :, 1 + wc * WL:1 + (wc + 1) * WL, :],
                    xf[wc, :, :, ds(pid * HL, 6), :, :])
            scf = pools["io"].tile([C, ch2], BF16, tag=f"sc2f{b}", name=_tn(f"sc2f{b}"))
            sc2_f.append(scf)

            res_cache = {}

            def ev_c1b(u, hp, nh, ps, b=b, scf=scf, res_cache=res_cache):
                col = u * (HL * W * A) + hp * (W * A)
                n = nh * W * A
                if u not in res_cache:
                    rt = pools["big"].tile([C, HL * W * A], F32, tag="res_u", name=_tn("res_u"))
                    ub = u * (HL * W * A)
                    nc.sync.dma_start(
                        rt, self.params["res1"][:, b * ch2 + ub:
                                                b * ch2 + ub + HL * W * A])
                    res_cache[u] = rt
                sct = pools["big"].tile([C, 2 * W * A], F32, tag="out_t", name=_tn("sc_t"))
                nc.vector.tensor_tensor(
                    out=sct[:, :n], in0=ps[:, :n],
                    in1=res_cache[u][:, hp * W * A: hp * W * A + n], op=OP.add)
                nc.scalar.activation(scf[:, col:col + n], sct[:, :n], AF.Copy)
                nc.sync.dma_start(sc2_dram[b][:, col:col + n], sct[:, :n])

            self.conv_half(pools, cin, ev_c1b)

            # ---- pass 2 on sc2 (boundary hl rows first so the halo
            # exchange overlaps interior compute)
            y2_sb = pools["io"].tile([C, ch2], BF16, tag=f"y2sb{b}", name=_tn(f"y2sb{b}"))
            y2_sbs.append(y2_sb)
            perm2 = ([u * HL for u in range(A)]
                     + [u * HL + HL - 1 for u in range(A)]
                     + [u * HL + hl for hl in range(1, HL - 1)
                        for u in range(A)])

            def x_src2(c0, ntok, scf=scf):
                t = pools["big"].tile([128, ntok], BF16, tag="x2g", name=_tn("x2g"))
                for i in range(ntok // L2):
                    st = perm2[c0 + i]
                    nc.vector.tensor_copy(
                        out=t[:, i * L2:(i + 1) * L2],
                        in_=scf[:, st * L2:(st + 1) * L2])
                return t

            def ev_y2(c0, nt0, n, ps, y2_sb=y2_sb):
                for i in range(n // L2):
                    st = perm2[c0 + nt0 // L2 + i]
                    nc.vector.tensor_copy(
                        out=y2_sb[:, st * L2:(st + 1) * L2],
                        in_=ps[:, i * L2:(i + 1) * L2])

            self.transformer_half(tc, pools, x_src2,
                                  (self.mask3_2, self.maskB_2, self.maskC_2),
                                  L2, cfg.SEQ2, ev_y2)
            ysb_v = y2_sb.rearrange("p (u hl wv) -> p u hl wv", u=A, hl=HL)
            nc.sync.dma_start(
                bnd_snd[b][:, 0:A * W * A].rearrange(
                    "p (u wv) -> p u wv", u=A),
                ysb_v[:, :, 0, :])
            nc.sync.dma_start(
                bnd_snd[b][:, A * W * A:2 * A * W * A].rearrange(
                    "p (u wv) -> p u wv", u=A),
                ysb_v[:, :, HL - 1, :])
            nc.gpsimd.collective_compute(
                "AllGather", mybir.AluOpType.bypass,
                ins=[bnd_snd[b].opt()],
                outs=[bnd_all[b].opt()],
                replica_groups=[list(range(NC))])

        # ---------------- conv2 + sc2 -> out
        for b in range(B):
            cin = pools["io"].tile([C, A * 6 * WPAD * A], BF16, tag="cin", name=_tn("cin"))
            cinv = cin.rearrange("p (u h w v) -> p u h w v", u=A, h=6, w=WPAD)
            nc.vector.memset(cinv[:, :, :, 0:1, :], 0.0)
            nc.vector.memset(cinv[:, :, :, WPAD - 1:WPAD, :], 0.0)
            ysv = y2_sbs[b].rearrange("p (u hl w v) -> p u hl w v",
                                      u=A, hl=HL, w=W)
            for u in range(A):
                nc.sync.dma_start(cinv[:, u, 1:1 + HL, 1:1 + W, :], ysv[:, u])
            # interior output rows need only local y2 — run before halo
            def ev_c2(u, hp, nh, ps, b=b):
                col = u * (HL * W * A) + hp * (W * A)
                n = nh * W * A
                rt = pools["big"].tile([C, 2 * W * A], F32, tag="res_u", name=_tn("res2_u"))
                nc.sync.dma_start(rt[:, :n], sc2_dram[b][:, col: col + n])
                ot = pools["big"].tile([128, 2 * W * A], F32, tag="out_t", name=_tn("out_t"))
                nc.vector.tensor_tensor(
                    out=ot[:, :n], in0=ps[:, :n], in1=rt[:, :n], op=OP.add)
                nc.sync.dma_start(
                    out_ext.ap()[:, b * ch2 + col: b * ch2 + col + n],
                    ot[:, :n])

            self.conv_half(pools, cin, ev_c2, rows=[(1, 2)])
            blkA = (pid + NC - 1) % NC
            blkC = (pid + 1) % NC
            bav_t = bnd_all[b][ds(blkA * C, C), :].rearrange(
                "c (e u w v) -> c e u w v", e=2, u=A, w=W)
            nc.sync.dma_start(cinv[:, :, 0, 1:1 + W, :], bav_t[:, 1])
            bav_b = bnd_all[b][ds(blkC * C, C), :].rearrange(
                "c (e u w v) -> c e u w v", e=2, u=A, w=W)
            nc.sync.dma_start(cinv[:, :, 5, 1:1 + W, :], bav_b[:, 0])
            if isinstance(pid, int):
                if pid < 1:
                    nc.vector.memset(cinv[:, :, 0:1, :, :], 0.0)
                if pid > NC - 2:
                    nc.vector.memset(cinv[:, :, 5:6, :, :], 0.0)
            else:
                with tc.If(pid < 1):
                    nc.vector.memset(cinv[:, :, 0:1, :, :], 0.0)
                with tc.If(pid > NC - 2):
                    nc.vector.memset(cinv[:, :, 5:6, :, :], 0.0)
            self.conv_half(pools, cin, ev_c2, rows=[(0, 1), (HL - 1, 1)])


# ---------------------------------------------------------------- entry point

_CACHE = {}


def kernel(**inputs):
    import numpy as np
    from concourse.bass_utils import run_bass_kernel_spmd
    import os as _os
    cfg = Cfg()
    if "nc" not in _CACHE:
        abl = tuple(x for x in _os.environ.get("KABL", "").split(",") if x)
        ker = Ker(cfg, ablate=abl)
        _CACHE["nc"] = ker.build()
    nc = _CACHE["nc"]
    in_maps = host_prep(cfg, inputs)
    res = run_bass_kernel_spmd(nc, in_maps, core_ids=list(range(cfg.ncores)),
                               trace=False)
    outs = [res.results[i]["out"] for i in range(cfg.ncores)]
    return host_unshard(cfg, outs).astype(np.float32)



# revision 53
# speedup vs baseline: 1.3145x; 1.0277x over previous
"""Bass kernel for nn_AltFilter: dual-pass windowed transformer + conv.

Sharding: pass-1 data-parallel over w (8 chunks of W/8), pass-2 over h.
Between passes: AllGather of pass-1 output (bf16), conv read from gathered.

Layouts (per core, per b-half):
  xtok1   [C, (v, wl, u, h)]        pass-1 token input (host prepped, bf16)
  y_perm  [C, (u, hpad34, wl, v)]   pass-1 output staged for AG (bf16)
  x1_full [8*C, (u, hpad34, wl, v)] AG result, wc-major blocks
  conv1in [C, (u, 6, Wpad, v)]      conv window (bf16) ; w = 4*wc+wl
  sc2     [C, (u, hl, w, v)]        conv1+res (f32/bf16) == pass-2 tokens
  y2      [C, (u, hl, w, v)]        pass-2 out chunk (bf16) -> AG (10-block padded)
  out     [C, (b, u, hl, w, v)]     final (f32)
"""

import numpy as np
from dataclasses import dataclass
from concourse import bass, bacc, tile, mybir
from concourse.bass import ds

BF16 = mybir.dt.bfloat16
F32 = mybir.dt.float32
AF = mybir.ActivationFunctionType
OP = mybir.AluOpType


@dataclass
class Cfg:
    A: int = 5
    C: int = 128
    E: int = 256
    NH: int = 8
    B: int = 2
    H: int = 32
    W: int = 32
    ncores: int = 8
    ch_seqs: int = 4      # sequences per processing chunk
    win: int = 5          # attention half-window (KW//2)

    @property
    def HD(self):
        return self.E // self.NH

    @property
    def WL(self):
        return self.W // self.ncores

    @property
    def HL(self):
        return self.H // self.ncores

    @property
    def L1(self):
        return self.A * self.H       # pass-1 tokens per seq (u, h)

    @property
    def L2(self):
        return self.W * self.A       # pass-2 tokens per seq (w, v)

    @property
    def SEQ1(self):
        return self.A * self.WL      # per-b seqs pass 1 (v, wl)

    @property
    def SEQ2(self):
        return self.A * self.HL      # per-b seqs pass 2 (u, hl)

    @property
    def HPAD(self):
        return self.H + 2



_tname_ctr = [0]


def _tn(tag):
    _tname_ctr[0] += 1
    return f"{tag}_{_tname_ctr[0]}"

def mchunks(L):
    out = []
    o = 0
    while o < L:
        sz = min(128, L - o)
        out.append((o, sz))
        o += sz
    return out


# ---------------------------------------------------------------- host prep

def host_prep(cfg, inputs):
    """Build per-core in_maps from full inputs. Returns list of dicts."""
    import ml_dtypes
    bf = ml_dtypes.bfloat16
    A, C, E, B, H, W = cfg.A, cfg.C, cfg.E, cfg.B, cfg.H, cfg.W
    NC = cfg.ncores
    WL, HL = cfg.WL, cfg.HL

    buf = np.asarray(inputs["buffer"], np.float32)
    buf6 = buf.reshape(B, C, A, A, H, W)                    # b c u v h w

    # pass-1 tokens: [c, b, v, w, u, h] -> core k takes w slice
    xt = np.ascontiguousarray(buf6.transpose(1, 0, 3, 5, 2, 4))  # c b v w u h
    # conv1 residual: [c, b, u, h, w, v] -> core k takes h slice
    rs = np.ascontiguousarray(buf6.transpose(1, 0, 2, 4, 5, 3))  # c b u h w v

    ip = np.asarray(inputs["in_proj"], np.float32)
    sc = 1.0 / np.sqrt(cfg.HD)
    WqT = (ip[0:E].T * sc).astype(bf)
    WkT = ip[E:2 * E].T.astype(bf)
    WvT = ip[2 * E:3 * E].T.astype(bf)
    WinT = np.asarray(inputs["Win"], np.float32).T.astype(bf)       # (C, E)
    WoT = np.asarray(inputs["attn_out_w"], np.float32).T.astype(bf)  # (E, E)
    Wff1T = np.asarray(inputs["ff1"], np.float32).T.astype(bf)       # (E, 2E)
    Wff2T = np.asarray(inputs["ff2"], np.float32).T.astype(bf)       # (2E, E)
    WoutT = np.asarray(inputs["Wout"], np.float32).T.astype(bf)      # (E, C)
    cw = np.asarray(inputs["conv_w"], np.float32)[:, :, 0]           # (O,I,3,3)
    tapT = np.ascontiguousarray(cw.transpose(2, 3, 1, 0)).astype(bf)  # ky kx I O

    def band_mask(L, blk, n_outer, outer_major):
        # tokens: outer_major=True -> l = outer*blk_count... build via indices
        # pass1: l = u*H + h, band over h ; pass2: l = w*A + v, band over w
        l = np.arange(L)
        if outer_major:
            pos = l % blk          # h = l % H  (u-major, h inner)
        else:
            pos = l // n_outer     # w = l // A (w-major, v inner)
        d = np.abs(pos[:, None] - pos[None, :])
        m = np.where(d <= cfg.win, 0.0, -30000.0).astype(np.float32)
        return m.astype(bf)

    mask1 = band_mask(cfg.L1, cfg.H, cfg.A, True)
    mask2 = band_mask(cfg.L2, cfg.W, cfg.A, False)

    # attention-psum mask-init tiles: scA [128, 3L] = 3 head replicas of
    # mask rows 0:128; scB [128, 2L] = head-3 main + tail (rows 0:32);
    # scC [128, 3L] = 3 tail replicas at rows 0:32 (rest zero).
    def mk_masks(mask):
        m_main = np.asarray(mask[0:128, :], np.float32)   # [128, 160]
        m_tail = np.asarray(mask[128:160, :], np.float32)  # [32, 160]
        m3 = np.concatenate([m_main] * 3, axis=1)          # [128, 480]
        tail_pad = np.concatenate(
            [m_tail, np.full((96, m_tail.shape[1]), -30000.0, np.float32)], 0)
        mB = np.concatenate([m_main, tail_pad], axis=1)    # [128, 320]
        tail_z = np.concatenate(
            [m_tail, np.zeros((96, m_tail.shape[1]), np.float32)], 0)
        mC = np.concatenate([tail_z] * 3, axis=1)          # [128, 480]
        return (np.ascontiguousarray(m3).astype(bf),
                np.ascontiguousarray(mB).astype(bf),
                np.ascontiguousarray(mC).astype(bf))

    mask3_1, maskB_1, maskC_1 = mk_masks(mask1)
    mask3_2, maskB_2, maskC_2 = mk_masks(mask2)
    ident = np.eye(128, dtype=np.float32).astype(bf)

    lnw1 = np.asarray(inputs["ln_w"], np.float32).reshape(E // 128, 128).T.copy()
    lnb1 = np.asarray(inputs["ln_b"], np.float32).reshape(E // 128, 128).T.copy()
    lnw2 = np.asarray(inputs["ffn_ln_w"], np.float32).reshape(E // 128, 128).T.copy()
    lnb2 = np.asarray(inputs["ffn_ln_b"], np.float32).reshape(E // 128, 128).T.copy()

    shared = dict(WinT=WinT, WqT=WqT, WkT=WkT, WvT=WvT, WoT=WoT,
                  Wff1T=Wff1T, Wff2T=Wff2T, WoutT=WoutT, tapT=tapT,
                  mask3_1=mask3_1, maskB_1=maskB_1, maskC_1=maskC_1,
                  mask3_2=mask3_2, maskB_2=maskB_2, maskC_2=maskC_2,
                  ident=ident,
                  lnw1=lnw1, lnb1=lnb1, lnw2=lnw2, lnb2=lnb2)

    maps = []
    for k in range(NC):
        xk = xt[:, :, :, k * WL:(k + 1) * WL]   # c b v wl u h
        xk = np.ascontiguousarray(xk).reshape(C, -1).astype(bf)
        rk = rs[:, :, :, k * HL:(k + 1) * HL]   # c b u hl w v
        rk = np.ascontiguousarray(rk).reshape(C, -1).astype(np.float32)
        m = dict(shared)
        m["xtok1"] = xk
        m["res1"] = rk
        maps.append(m)
    return maps


def host_unshard(cfg, outs):
    """outs: list of per-core 'out' arrays [C, B*A*HL*W*A] -> full output."""
    A, C, B, H, W = cfg.A, cfg.C, cfg.B, cfg.H, cfg.W
    o = np.stack(outs)  # j c b u hl w v
    o = o.reshape(cfg.ncores, C, B, A, cfg.HL, W, A)
    o = o.transpose(2, 1, 3, 6, 0, 4, 5)  # b c u v j hl w
    return np.ascontiguousarray(o).reshape(B, C, A * A, H, W)


# ---------------------------------------------------------------- builder

class Ker:
    """Holds nc + pools + weight tiles while building."""

    _tables_pinned = False

    @classmethod
    def _pin_act_tables(cls):
        import os as _os
        if cls._tables_pinned or _os.environ.get("KTAB", "pin") != "pin":
            return
        cls._tables_pinned = True
        import concourse.bacc as _baccmod
        from concourse.hw_specs import get_activation_tables as _gat

        def pinned(arch):
            tabs = _gat(arch)
            keep = "natural_log_exp_and_others"
            mine = {AF.Exp, AF.Ln, AF.Copy, AF.Identity}
            out = {}
            for name, s in tabs.items():
                out[name] = s if name == keep else (s - mine)
            return out

        _baccmod.get_activation_tables = pinned

    def __init__(self, cfg, stage=4, ablate=()):
        self._pin_act_tables()
        self.cfg = cfg
        self.stage = stage  # 1=pass1, 2=+conv1, 3=+pass2, 4=full
        self.ablate = set(ablate)
        self.nc = bacc.Bacc("TRN2", target_bir_lowering=False, debug=False,
                            num_devices=cfg.ncores)

    # -- weights to sbuf ---------------------------------------------------
    def load_weights(self, tc, pool):
        nc, cfg = self.nc, self.cfg
        E = cfg.E

        def wtile(name, K, M):
            p = self.params[name]
            t = pool.tile([128, (K // 128) * M], BF16, tag=name)
            for kt in range(K // 128):
                nc.sync.dma_start(t[:, kt * M:(kt + 1) * M],
                                  p[kt * 128:(kt + 1) * 128, :])
            return t

        self.WinT = wtile("WinT", cfg.C, E)
        self.WqT = wtile("WqT", E, E)
        self.WkT = wtile("WkT", E, E)
        self.WvT = wtile("WvT", E, E)
        self.WoT = wtile("WoT", E, E)
        self.Wff1T = wtile("Wff1T", E, 2 * E)
        self.Wff2T = wtile("Wff2T", 2 * E, E)
        self.WoutT = wtile("WoutT", E, cfg.C)

        tap = self.params["tapT"]  # (3,3,I,O)
        self.taps = pool.tile([128, 9 * 128], BF16, tag="taps", name=_tn("taps"))
        for ky in range(3):
            for kx in range(3):
                i = ky * 3 + kx
                nc.sync.dma_start(self.taps[:, i * 128:(i + 1) * 128],
                                  tap[ky, kx])


        def lnt(name):
            t = pool.tile([128, E // 128], F32, tag=name)
            nc.sync.dma_start(t, self.params[name])
            return t

        self.lnw1, self.lnb1 = lnt("lnw1"), lnt("lnb1")
        self.lnw2, self.lnb2 = lnt("lnw2"), lnt("lnb2")

        self.ones = pool.tile([128, 128], BF16, tag="ones", name=_tn("ones"))
        nc.vector.memset(self.ones[:], 1.0 / E)

        def bftile(name, shape):
            t = pool.tile(shape, BF16, tag=name)
            nc.sync.dma_start(t, self.params[name])
            return t

        self.mask3_1 = bftile("mask3_1", [128, 480])
        self.maskB_1 = bftile("maskB_1", [128, 320])
        self.maskC_1 = bftile("maskC_1", [128, 480])
        self.mask3_2 = bftile("mask3_2", [128, 480])
        self.maskB_2 = bftile("maskB_2", [128, 320])
        self.maskC_2 = bftile("maskC_2", [128, 480])
        self.ident = bftile("ident", [128, 128])
        self.ones_att = pool.tile([128, 32], BF16, tag="ones_att",
                                  name=_tn("ones_att"))
        nc.vector.memset(self.ones_att[:], 1.0)


    # -- generic GEMM over one token chunk --------------------------------
    def gemm(self, psum_pool, wt, K, M, rhs_fn, ntok, nt_sz, evict_fn,
             tag="gemm", name=_tn("gemm")):
        """out[mt] = sum_kt  wt[kt,mt].T @ rhs(kt, nt) ; evict per (mt, nt)."""
        nc = self.nc
        KT, MT = K // 128, M // 128
        for mt in range(MT):
            for nt0 in range(0, ntok, nt_sz):
                n = min(nt_sz, ntok - nt0)
                ps = psum_pool.tile([128, nt_sz], F32, tag=tag)
                for kt in range(KT):
                    nc.tensor.matmul(
                        ps[:, :n],
                        wt[:, kt * M + mt * 128: kt * M + mt * 128 + 128],
                        rhs_fn(kt, nt0, n),
                        start=(kt == 0), stop=(kt == KT - 1))
                evict_fn(mt, nt0, n, ps)

    # -- layernorm over one chunk -----------------------------------------
    def ln_chunk_v2(self, tc, pools, x_f32, lnw, lnb, out_bf, ntok, nt_sz,
                    also_sq_src=None):
        """Feature-major LN. Stats are computed via all-ones matmuls whose
        M=128 stationary replicates sum across all partitions, so no
        partition-broadcast is ever needed."""
        nc, cfg = self.nc, self.cfg
        FT = cfg.E // 128
        x_bf = also_sq_src
        sq = pools["scratch"].tile([128, FT * ntok], BF16, tag="lnsq", name=_tn("lnsq"))
        for ft in range(FT):
            nc.vector.tensor_tensor(
                out=sq[:, ft * ntok:(ft + 1) * ntok],
                in0=x_bf[:, ft * ntok:(ft + 1) * ntok],
                in1=x_bf[:, ft * ntok:(ft + 1) * ntok], op=OP.mult)
        mean = pools["scratch"].tile([128, ntok], F32, tag="lnmean", name=_tn("lnmean"))
        rstd = pools["scratch"].tile([128, ntok], F32, tag="lnrstd", name=_tn("lnrstd"))
        for nt0 in range(0, ntok, nt_sz):
            n = min(nt_sz, ntok - nt0)
            ps_s = pools["psum"].tile([128, nt_sz], F32, tag="gemm", name=_tn("lnstat"))
            ps_q = pools["psum"].tile([128, nt_sz], F32, tag="gemm", name=_tn("lnstat"))
            for kt in range(FT):
                nc.tensor.matmul(ps_s[:, :n], self.ones,
                                 x_bf[:, kt * ntok + nt0: kt * ntok + nt0 + n],
                                 start=(kt == 0), stop=(kt == FT - 1))
            for kt in range(FT):
                nc.tensor.matmul(ps_q[:, :n], self.ones,
                                 sq[:, kt * ntok + nt0: kt * ntok + nt0 + n],
                                 start=(kt == 0), stop=(kt == FT - 1))
            nc.vector.tensor_copy(out=mean[:, nt0:nt0 + n], in_=ps_s[:, :n])
            nc.vector.tensor_copy(out=rstd[:, nt0:nt0 + n], in_=ps_q[:, :n])
        # rstd = (E[x^2] + eps - mean^2) ** -0.5
        msq = pools["scratch"].tile([128, ntok], F32, tag="lnmsq", name=_tn("lnmsq"))
        nc.vector.tensor_tensor(out=msq[:], in0=mean[:], in1=mean[:],
                                op=OP.mult)
        nc.vector.scalar_tensor_tensor(
            out=rstd[:], in0=rstd[:], scalar=1e-5, in1=msq[:],
            op0=OP.add, op1=OP.subtract)
        # rstd = exp(-0.5*ln(var)); Ln+Exp share one ACT table so no
        # table reloads against the attention exps.
        if "rsqrt" not in self.ablate:
            nc.scalar.activation(rstd[:], rstd[:], AF.Ln)
            nc.scalar.activation(rstd[:], rstd[:], AF.Exp, scale=-0.5)
        t1 = pools["scratch"].tile([128, nt_sz], F32, tag="lnt1", name=_tn("lnt1"))
        for ft in range(FT):
            for nt0 in range(0, ntok, nt_sz):
                n = min(nt_sz, ntok - nt0)
                nc.vector.tensor_tensor(
                    out=t1[:, :n],
                    in0=x_f32[:, ft * ntok + nt0: ft * ntok + nt0 + n],
                    in1=mean[:, nt0:nt0 + n], op=OP.subtract)
                nc.vector.tensor_tensor(
                    out=t1[:, :n], in0=t1[:, :n],
                    in1=rstd[:, nt0:nt0 + n], op=OP.mult)
                if "lnapply" in self.ablate:
                    nc.scalar.activation(
                        out_bf[:, ft * ntok + nt0: ft * ntok + nt0 + n],
                        t1[:, :n], AF.Copy)
                else:
                    nc.scalar.activation(
                        out_bf[:, ft * ntok + nt0: ft * ntok + nt0 + n],
                        t1[:, :n], AF.Identity, bias=lnb[:, ft:ft + 1],
                        scale=lnw[:, ft:ft + 1])

    # -- attention for one chunk of sequences ------------------------------
    def attention(self, pools, q_s, k_s, v_m, v_t, masks, o_bf, nseq, L,
                  ntok, ntokmax):
        """Per (seq, head-quad): scores for 4 heads into two psum banks
        (scA = heads 0-2 main [128, 3L]; scB = head-3 main [128, L] + the
        4 stacked [32, L] m-tails), mask added via one identity matmul from
        a precomputed SBUF tile, one Exp per bank, then col-tiled AV into an
        O|D bank: rows 32c = head c, cols 0:L = o, L:2L = denominator.
        One reciprocal + one multiply normalize all 4 heads."""
        nc, cfg = self.nc, self.cfg
        E = cfg.E
        mask3_t, maskB_t, maskC_t = masks
        for s in range(nseq):
            for qd in range(2):
                scA = pools["sc"].tile([128, 3 * L], F32, tag="scA", name=_tn("scA"))
                scB = pools["sc"].tile([128, 2 * L], F32, tag="scB", name=_tn("scB"))
                scC = pools["sc"].tile([32, 3 * L], F32, tag="scC", name=_tn("scC"))
                nc.tensor.matmul(scA, self.ident, mask3_t, start=True,
                                 stop=False, skip_group_check=True)
                nc.tensor.matmul(scB, self.ident, maskB_t, start=True,
                                 stop=False, skip_group_check=True)
                nc.tensor.matmul(scC, self.ident[:, 0:32], maskC_t,
                                 start=True, stop=False,
                                 skip_group_check=True)
                base = qd * ntokmax + s * L
                for c in range(4):
                    ks = k_s[4 * qd + c]
                    qs = q_s[4 * qd + c][0:32, s * L: (s + 1) * L]
                    out_main = (scA[:, L * c:L * (c + 1)] if c < 3
                                else scB[:, 0:L])
                    nc.tensor.matmul(out_main,
                                     ks[0:32, s * L: s * L + 128], qs,
                                     start=False, stop=True,
                                     skip_group_check=True)
                    tdst = (scC[0:32, L * c:L * (c + 1)] if c < 3
                            else scB[0:32, L:2 * L])
                    nc.tensor.matmul(tdst,
                                     ks[0:32, s * L + 128: (s + 1) * L], qs,
                                     start=False, stop=True,
                                     skip_group_check=True)
                exA = pools["exp"].tile([128, 3 * L], BF16, tag="exA", name=_tn("exA"))
                nc.scalar.activation(exA, scA, AF.Exp)
                exB = pools["exp"].tile([128, 2 * L], BF16, tag="exB", name=_tn("exB"))
                nc.scalar.activation(exB, scB, AF.Exp)
                exC = pools["exp"].tile([32, 3 * L], BF16, tag="exC", name=_tn("exC"))
                nc.scalar.activation(exC, scC, AF.Exp)
                # AV + denominators reuse scB's bank: its scores are dead
                # once exB is taken, so no extra PSUM bank is needed.
                for c in range(4):
                    r0 = 32 * c
                    hg = 4 * qd + c
                    exm = exA[:, L * c:L * (c + 1)] if c < 3 else exB[:, 0:L]
                    ext = (exC[0:32, L * c:L * (c + 1)] if c < 3
                           else exB[0:32, L:2 * L])
                    vm = v_m[:, s * E + 32 * hg: s * E + 32 * hg + 32]
                    vt = v_t[0:32, s * E + 32 * hg: s * E + 32 * hg + 32]
                    nc.tensor.matmul(scB[r0:r0 + 32, 0:L], vm, exm,
                                     start=True, stop=False,
                                     tile_position=(0, r0))
                    nc.tensor.matmul(scB[r0:r0 + 32, 0:L], vt, ext,
                                     start=False, stop=True,
                                     tile_position=(0, r0))
                    nc.tensor.matmul(scB[r0:r0 + 32, L:2 * L],
                                     self.ones_att[0:128, 0:32], exm,
                                     start=True, stop=False,
                                     tile_position=(0, r0))
                    nc.tensor.matmul(scB[r0:r0 + 32, L:2 * L],
                                     self.ones_att[0:32, 0:32], ext,
                                     start=False, stop=True,
                                     tile_position=(0, r0))
                rec = pools["scratch"].tile([128, L], F32, tag="rec", name=_tn("rec"))
                nc.vector.reciprocal_approx_fast(rec, scB[:, L:2 * L])
                nc.vector.tensor_tensor(
                    out=o_bf[:, qd * ntok + s * L: qd * ntok + (s + 1) * L],
                    in0=scB[:, 0:L], in1=rec, op=OP.mult)

    # -- one transformer pass over one b-half ------------------------------
    def transformer_half(self, tc, pools, x_src_fn, masks, L, nseq_b,
                         evict_y_fn):
        """x_src_fn(c0, ntok) -> bf16 [128, ntok] input token tile (Win rhs).
        evict_y_fn(s_global_pair_index, nt0, n, psum) writes final y."""
        nc, cfg = self.nc, self.cfg
        E = cfg.E
        FT = E // 128
        CH = cfg.ch_seqs
        NTs = 3 * L            # token tile = 3 seqs (psum [128,480] f32 fits a bank)
        ntokmax = CH * L
        for c0 in range(0, nseq_b, CH):
            # q_s/k_s: one 32-row tile per head (base partition 0), cols =
            # tokens. Avoids row-tiled matmuls (broken on HW); K=32 score
            # MMs all run at array rows 0:32. Allocated per chunk from a
            # double-buffered pool so chunk k+1's Q/K evicts don't wait on
            # chunk k's score matmuls.
            q_s = [pools["qk"].tile([32, ntokmax], BF16, tag=f"q_s{h}",
                                    name=_tn(f"q_s{h}")) for h in range(cfg.NH)]
            k_s = [pools["qk"].tile([32, ntokmax], BF16, tag=f"k_s{h}",
                                    name=_tn(f"k_s{h}")) for h in range(cfg.NH)]
            ns = min(CH, nseq_b - c0)
            ntok = ns * L
            x_bf = x_src_fn(c0, ntok)
            tok_f = pools["big1"].tile([128, FT * ntok], F32, tag="tok_f", name=_tn("tok_f"))
            tok_bf = pools["big"].tile([128, FT * ntok], BF16, tag="tok_bf", name=_tn("tok_bf"))

            def ev_tok(mt, nt0, n, ps):
                nc.vector.tensor_copy(
                    out=tok_f[:, mt * ntok + nt0: mt * ntok + nt0 + n],
                    in_=ps[:, :n])
                nc.scalar.activation(
                    tok_bf[:, mt * ntok + nt0: mt * ntok + nt0 + n],
                    ps[:, :n], AF.Copy)

            self.gemm(pools["psum"], self.WinT, cfg.C, E,
                      lambda kt, nt0, n: x_bf[:, nt0:nt0 + n],
                      ntok, NTs, ev_tok, tag="gemm", name=_tn("gemm"))

            tn = pools["big"].tile([128, FT * ntok], BF16, tag="tn", name=_tn("tn"))
            if "ln" in self.ablate:
                nc.vector.tensor_copy(out=tn, in_=tok_bf)
            else:
                self.ln_chunk_v2(tc, pools, tok_f, self.lnw1, self.lnb1, tn,
                                 ntok, NTs, also_sq_src=tok_bf)

            def mk_ev(dst, use_act):
                def ev(mt, nt0, n, ps):
                    for c in range(4):
                        d = dst[4 * mt + c][0:32, nt0: nt0 + n]
                        if use_act:
                            nc.scalar.activation(
                                d, ps[32 * c:32 * c + 32, :n], AF.Copy)
                        else:
                            nc.vector.tensor_copy(
                                out=d, in_=ps[32 * c:32 * c + 32, :n])
                return ev

            rhs_tn = lambda kt, nt0, n: tn[:, kt * ntok + nt0: kt * ntok + nt0 + n]
            self.gemm(pools["psum"], self.WqT, E, E, rhs_tn, ntok, NTs,
                      mk_ev(q_s, True), tag="gemm", name=_tn("gemm"))
            self.gemm(pools["psum"], self.WkT, E, E, rhs_tn, ntok, NTs,
                      mk_ev(k_s, False), tag="gemm", name=_tn("gemm"))

            # V token-major [tok, E]
            v_m = pools["big"].tile([128, CH * E], BF16, tag="v_m", name=_tn("v_m"))
            has_tail = L > 128
            v_t = None
            if has_tail:
                v_t = pools["big"].tile([32, CH * E], BF16,
                                        tag="v_t", name=_tn("v_t"))
            Lm = min(128, L)
            for s in range(ns):
                ps = pools["psum"].tile([128, E], F32, tag="gemm", name=_tn("gemm"))
                for kt in range(FT):
                    nc.tensor.matmul(
                        ps[0:Lm, :],
                        tok_bf[:, kt * ntok + s * L: kt * ntok + s * L + Lm],
                        self.WvT[:, kt * E:(kt + 1) * E],
                        start=(kt == 0), stop=(kt == FT - 1))
                nc.vector.tensor_copy(
                    out=v_m[0:Lm, s * E:(s + 1) * E], in_=ps[0:Lm, :])
            if has_tail:
                tl = L - 128
                for s in range(ns):
                    ps = pools["psum"].tile([128, E], F32, tag="gemm", name=_tn("gemm"))
                    for kt in range(FT):
                        nc.tensor.matmul(
                            ps[0:tl, :],
                            tok_bf[:, kt * ntok + s * L + 128:
                                   kt * ntok + s * L + 128 + tl],
                            self.WvT[:, kt * E:(kt + 1) * E],
                            start=(kt == 0), stop=(kt == FT - 1))
                    nc.scalar.activation(
                        v_t[0:tl, s * E:(s + 1) * E], ps[0:tl, :], AF.Copy)

            o_bf = pools["big"].tile([128, FT * ntok], BF16, tag="o_bf", name=_tn("o_bf"))
            if "attn" in self.ablate:
                nc.vector.tensor_copy(out=o_bf, in_=tn)
            else:
                self.attention(pools, q_s, k_s, v_m, v_t, masks,
                               o_bf, ns, L, ntok, ntokmax)

            # out-proj + residual
            outp_f = pools["big1"].tile([128, FT * ntok], F32, tag="outp_f", name=_tn("outp_f"))
            outp_bf = pools["big"].tile([128, FT * ntok], BF16, tag="outp_bf", name=_tn("outp_bf"))

            def ev_outp(mt, nt0, n, ps):
                nc.vector.tensor_tensor(
                    out=outp_f[:, mt * ntok + nt0: mt * ntok + nt0 + n],
                    in0=ps[:, :n],
                    in1=tok_f[:, mt * ntok + nt0: mt * ntok + nt0 + n],
                    op=OP.add)
                nc.scalar.activation(
                    outp_bf[:, mt * ntok + nt0: mt * ntok + nt0 + n],
                    outp_f[:, mt * ntok + nt0: mt * ntok + nt0 + n], AF.Copy)

            rhs_o = lambda kt, nt0, n: o_bf[:, kt * ntok + nt0: kt * ntok + nt0 + n]
            self.gemm(pools["psum"], self.WoT, E, E, rhs_o, ntok, NTs,
                      ev_outp, tag="gemm", name=_tn("gemm"))

            tn2 = pools["big"].tile([128, FT * ntok], BF16, tag="tn2", name=_tn("tn2"))
            if "ln" in self.ablate:
                nc.vector.tensor_copy(out=tn2, in_=outp_bf)
            else:
                self.ln_chunk_v2(tc, pools, outp_f, self.lnw2, self.lnb2, tn2,
                                 ntok, NTs, also_sq_src=outp_bf)

            ffh = pools["big"].tile([128, 2 * FT * ntok], BF16, tag="ffh", name=_tn("ffh"))

            def ev_ffh(mt, nt0, n, ps):
                nc.vector.tensor_relu(
                    out=ffh[:, mt * ntok + nt0: mt * ntok + nt0 + n],
                    in_=ps[:, :n])

            rhs_tn2 = lambda kt, nt0, n: tn2[:, kt * ntok + nt0: kt * ntok + nt0 + n]
            self.gemm(pools["psum"], self.Wff1T, E, 2 * E, rhs_tn2, ntok, NTs,
                      ev_ffh, tag="gemm", name=_tn("gemm"))

            res2_f = pools["big1"].tile([128, FT * ntok], F32, tag="res2_f", name=_tn("res2_f"))
            res2_bf = pools["big"].tile([128, FT * ntok], BF16, tag="res2_bf", name=_tn("res2_bf"))

            def ev_ffo(mt, nt0, n, ps):
                nc.vector.tensor_tensor(
                    out=res2_f[:, mt * ntok + nt0: mt * ntok + nt0 + n],
                    in0=ps[:, :n],
                    in1=outp_f[:, mt * ntok + nt0: mt * ntok + nt0 + n],
                    op=OP.add)
                nc.scalar.activation(
                    res2_bf[:, mt * ntok + nt0: mt * ntok + nt0 + n],
                    res2_f[:, mt * ntok + nt0: mt * ntok + nt0 + n], AF.Copy)

            rhs_ffh = lambda kt, nt0, n: ffh[:, kt * ntok + nt0: kt * ntok + nt0 + n]
            self.gemm(pools["psum"], self.Wff2T, 2 * E, E, rhs_ffh, ntok, NTs,
                      ev_ffo, tag="gemm", name=_tn("gemm"))

            rhs_r2 = lambda kt, nt0, n: res2_bf[:, kt * ntok + nt0: kt * ntok + nt0 + n]
            self.gemm(pools["psum"], self.WoutT, E, cfg.C, rhs_r2, ntok, NTs,
                      lambda mt, nt0, n, ps: evict_y_fn(c0, nt0, n, ps),
                      tag="gemm", name=_tn("gemm"))

    # -- conv 3x3 over (h,w) for one b-half --------------------------------
    def conv_half(self, pools, cin, evict_fn, rows=None):
        """cin: sbuf [128, A*6*WPAD*A(v)] bf16 window tile (w-padded).
        out pixels (u, hl, w, v); evict_fn(u, hp, n, psum) with pixel tiles
        [128, 2*W*A(v)].  rows: list of (hp, nh) output-row groups; default
        covers all HL rows in pairs."""
        nc, cfg = self.nc, self.cfg
        A, W = cfg.A, cfg.W
        WP = W + 2
        if rows is None:
            rows = [(hp, min(2, cfg.HL - hp)) for hp in range(0, cfg.HL, 2)]
        cv = cin.rearrange("p (u h w v) -> p u h w v", u=A, h=6, w=WP)
        for u in range(A):
            for hp, nh in rows:
                ps = pools["psum"].tile([128, 2 * W * A], F32, tag="gemm", name=_tn("gemm"))
                first = True
                for dy in range(3):
                    for dx in range(3):
                        rhs = cv[:, u, hp + dy:hp + dy + nh, dx:dx + W, :]
                        nc.tensor.matmul(
                            ps[:, :nh * W * A],
                            self.taps[:, (dy * 3 + dx) * 128:(dy * 3 + dx + 1) * 128],
                            rhs,
                            start=first, stop=(dy == 2 and dx == 2))
                        first = False
                evict_fn(u, hp, nh, ps)

    # -- full graph --------------------------------------------------------
    def build(self):
        cfg = self.cfg
        nc = self.nc
        A, C, E, B, H, W = cfg.A, cfg.C, cfg.E, cfg.B, cfg.H, cfg.W
        L1, L2 = cfg.L1, cfg.L2
        FT = E // 128
        WL, HL, HP = cfg.WL, cfg.HL, cfg.HPAD
        NC = cfg.ncores
        WPAD = W + 2
        chunk1_cols = A * HP * WL * A          # y_perm cols (u hpad wl v)
        chunk2_cols = A * HL * W * A           # y2 cols (u hl w v)

        # ---- dram params
        P = {}
        def par(name, shape, dt):
            P[name] = nc.dram_tensor(name, shape, dt, kind="ExternalInput").ap()
        par("xtok1", [C, B * cfg.SEQ1 * L1], BF16)
        par("res1", [C, B * chunk2_cols], F32)
        par("WinT", [C, E], BF16)
        for n in ("WqT", "WkT", "WvT", "WoT"):
            par(n, [E, E], BF16)
        par("Wff1T", [E, 2 * E], BF16)
        par("Wff2T", [2 * E, E], BF16)
        par("WoutT", [E, C], BF16)
        par("tapT", [3, 3, C, C], BF16)
        par("mask3_1", [128, 480], BF16)
        par("maskB_1", [128, 320], BF16)
        par("maskC_1", [128, 480], BF16)
        par("mask3_2", [128, 480], BF16)
        par("maskB_2", [128, 320], BF16)
        par("maskC_2", [128, 480], BF16)
        par("ident", [128, 128], BF16)
        for n in ("lnw1", "lnb1", "lnw2", "lnb2"):
            par(n, [128, FT], F32)
        self.params = P
        out_ext = nc.dram_tensor("out", [C, B * chunk2_cols], F32,
                                 kind="ExternalOutput")

        with tile.TileContext(nc) as tc:
            import contextlib
            with contextlib.ExitStack() as ctx:
                pools = {}
                pools["const"] = ctx.enter_context(
                    tc.tile_pool(name="const", bufs=1))
                pools["big"] = ctx.enter_context(
                    tc.tile_pool(name="big", bufs=2))
                pools["big1"] = ctx.enter_context(
                    tc.tile_pool(name="big1", bufs=1))
                pools["qk"] = ctx.enter_context(
                    tc.tile_pool(name="qk", bufs=2))
                pools["scratch"] = ctx.enter_context(
                    tc.tile_pool(name="scratch", bufs=1))
                pools["io"] = ctx.enter_context(
                    tc.tile_pool(name="io", bufs=1))
                pools["psum"] = ctx.enter_context(
                    tc.tile_pool(name="psum", bufs=2, space="PSUM"))
                pools["sc"] = ctx.enter_context(
                    tc.tile_pool(name="sc", bufs=2, space="PSUM"))
                pools["exp"] = ctx.enter_context(
                    tc.tile_pool(name="exp", bufs=2))
                pools["dram"] = ctx.enter_context(
                    tc.tile_pool(name="dram", bufs=1, space="DRAM"))
                self._build_body(tc, pools, out_ext)
        nc.compile()
        return nc

    def _build_body(self, tc, pools, out_ext):
        nc, cfg = self.nc, self.cfg
        A, C, E, B, H, W = cfg.A, cfg.C, cfg.E, cfg.B, cfg.H, cfg.W
        L1, L2 = cfg.L1, cfg.L2
        FT = E // 128
        WL, HL, HP = cfg.WL, cfg.HL, cfg.HPAD
        NC = cfg.ncores
        WPAD = W + 2
        ch1 = A * HP * WL * A
        ch2 = A * HL * W * A
        dram = pools["dram"]

        self.load_weights(tc, pools["const"])
        import os as _os
        _simpid = _os.environ.get("KSIMPID")
        pid = int(_simpid) if _simpid else nc.partition_id()

        x1_chunk = [dram.tile([C, ch1], BF16, tag=f"x1c{b}", name=_tn(f"x1c{b}")) for b in range(B)]
        x1_full = [dram.tile([NC * C, ch1], BF16, addr_space="Shared",
                             tag=f"x1f{b}", name=_tn(f"x1f{b}")) for b in range(B)]
        bnd_cols = 2 * A * W * A
        bnd_snd = [dram.tile([C, bnd_cols], BF16, tag=f"bs{b}", name=_tn(f"bs{b}")) for b in range(B)]
        bnd_all = [dram.tile([NC * C, bnd_cols], BF16, tag=f"ba{b}", name=_tn(f"ba{b}")) for b in range(B)]
        sc2_dram = [dram.tile([C, ch2], F32, tag=f"s2d{b}", name=_tn(f"s2d{b}")) for b in range(B)]

        # ---------------- pass 1 (per b-half)
        sc2_f = []
        y2_sbs = []
        for b in range(B):
            y_perm = pools["io"].tile([C, ch1], BF16, tag="y_perm", name=_tn("y_perm"))
            # zero hpad rows 0 and HP-1:  cols (u, {0,HP-1}, wl, v)
            yv = y_perm.rearrange("p (u h w v) -> p u h w v", u=A, h=HP, w=WL)
            nc.vector.memset(yv[:, :, 0:1, :, :], 0.0)
            nc.vector.memset(yv[:, :, HP - 1:HP, :, :], 0.0)

            def x_src(c0, ntok, b=b):
                t = pools["big"].tile([128, ntok], BF16, tag="x_in", name=_tn("x_in"))
                nc.sync.dma_start(
                    t, self.params["xtok1"][:, b * cfg.SEQ1 * L1 + c0 * L1:
                                            b * cfg.SEQ1 * L1 + c0 * L1 + ntok])
                return t

            def ev_y(c0, nt0, n, ps, y_perm=y_perm):
                # psum [128, n] tokens of seqs starting at s0=c0+nt0//L1
                # seq (v, wl): v = s//WL, wl = s%WL ; token (u, h)
                # y_perm col = u*(HP*WL*A) + (h+1)*(WL*A) + wl*A + v
                s0 = c0 + nt0 // L1
                npair = n // L1
                for i in range(0, npair):
                    s = s0 + i
                    v, wl = s // WL, s % WL
                    dst = y_perm.rearrange(
                        "p (u h w vv) -> p u h w vv", u=A, h=HP, w=WL)
                    nc.vector.tensor_copy(
                        out=dst[:, :, 1:H + 1, wl, v],
                        in_=ps[:, i * L1:(i + 1) * L1].rearrange(
                            "p (u h) -> p u h", h=H))

            self.transformer_half(tc, pools, x_src,
                                  (self.mask3_1, self.maskB_1, self.maskC_1),
                                  L1, cfg.SEQ1, ev_y)
            nc.sync.dma_start(x1_chunk[b][:, :], y_perm)
            nc.gpsimd.collective_compute(
                "AllGather", mybir.AluOpType.bypass,
                ins=[x1_chunk[b].opt()], outs=[x1_full[b].opt()],
                replica_groups=[list(range(NC))])

        # ---------------- conv1 + residual -> sc2 ; then pass 2
        for b in range(B):
            cin = pools["io"].tile([C, A * 6 * WPAD * A], BF16, tag="cin", name=_tn("cin"))
            cinv = cin.rearrange("p (u h w v) -> p u h w v", u=A, h=6, w=WPAD)
            nc.vector.memset(cinv[:, :, :, 0:1, :], 0.0)
            nc.vector.memset(cinv[:, :, :, WPAD - 1:WPAD, :], 0.0)
            xf = x1_full[b].rearrange("(wc c) (u h w v) -> wc c u h w v",
                                      c=C, u=A, h=HP, w=WL)
            for wc in range(NC):
                nc.sync.dma_start(
                    cinv[:, :, :, 1 + wc * WL:1 + (wc + 1) * WL, :],
                    xf[wc, :, :, ds(pid * HL, 6), :, :])
            scf = pools["io"].tile([C, ch2], BF16, tag=f"sc2f{b}", name=_tn(f"sc2f{b}"))
            sc2_f.append(scf)

            res_cache = {}

            def ev_c1b(u, hp, nh, ps, b=b, scf=scf, res_cache=res_cache):
                col = u * (HL * W * A) + hp * (W * A)
                n = nh * W * A
                if u not in res_cache:
                    rt = pools["big"].tile([C, HL * W * A], F32, tag="res_u", name=_tn("res_u"))
                    ub = u * (HL * W * A)
                    nc.sync.dma_start(
                        rt, self.params["res1"][:, b * ch2 + ub:
                                                b * ch2 + ub + HL * W * A])
                    res_cache[u] = rt
                sct = pools["big"].tile([C, 2 * W * A], F32, tag="out_t", name=_tn("sc_t"))
                nc.vector.tensor_tensor(
                    out=sct[:, :n], in0=ps[:, :n],
                    in1=res_cache[u][:, hp * W * A: hp * W * A + n], op=OP.add)
                nc.scalar.activation(scf[:, col:col + n], sct[:, :n], AF.Copy)
                nc.sync.dma_start(sc2_dram[b][:, col:col + n], sct[:, :n])

            self.conv_half(pools, cin, ev_c1b)

            # ---- pass 2 on sc2 (boundary hl rows first so the halo
            # exchange overlaps interior compute)
            y2_sb = pools["io"].tile([C, ch2], BF16, tag=f"y2sb{b}", name=_tn(f"y2sb{b}"))
            y2_sbs.append(y2_sb)
            perm2 = ([u * HL for u in range(A)]
                     + [u * HL + HL - 1 for u in range(A)]
                     + [u * HL + hl for hl in range(1, HL - 1)
                        for u in range(A)])

            def x_src2(c0, ntok, scf=scf):
                t = pools["big"].tile([128, ntok], BF16, tag="x2g", name=_tn("x2g"))
                for i in range(ntok // L2):
                    st = perm2[c0 + i]
                    nc.vector.tensor_copy(
                        out=t[:, i * L2:(i + 1) * L2],
                        in_=scf[:, st * L2:(st + 1) * L2])
                return t

            def ev_y2(c0, nt0, n, ps, y2_sb=y2_sb):
                for i in range(n // L2):
                    st = perm2[c0 + nt0 // L2 + i]
                    nc.vector.tensor_copy(
                        out=y2_sb[:, st * L2:(st + 1) * L2],
                        in_=ps[:, i * L2:(i + 1) * L2])

            self.transformer_half(tc, pools, x_src2,
                                  (self.mask3_2, self.maskB_2, self.maskC_2),
                                  L2, cfg.SEQ2, ev_y2)
            ysb_v = y2_sb.rearrange("p (u hl wv) -> p u hl wv", u=A, hl=HL)
            nc.sync.dma_start(
                bnd_snd[b][:, 0:A * W * A].rearrange(
                    "p (u wv) -> p u wv", u=A),
                ysb_v[:, :, 0, :])
            nc.sync.dma_start(
                bnd_snd[b][:, A * W * A:2 * A * W * A].rearrange(
                    "p (u wv) -> p u wv", u=A),
                ysb_v[:, :, HL - 1, :])
            nc.gpsimd.collective_compute(
                "AllGather", mybir.AluOpType.bypass,
                ins=[bnd_snd[b].opt()],
                outs=[bnd_all[b].opt()],
                replica_groups=[list(range(NC))])

        # ---------------- conv2 + sc2 -> out
        for b in range(B):
            cin = pools["io"].tile([C, A * 6 * WPAD * A], BF16, tag="cin", name=_tn("cin"))
            cinv = cin.rearrange("p (u h w v) -> p u h w v", u=A, h=6, w=WPAD)
            nc.vector.memset(cinv[:, :, :, 0:1, :], 0.0)
            nc.vector.memset(cinv[:, :, :, WPAD - 1:WPAD, :], 0.0)
            ysv = y2_sbs[b].rearrange("p (u hl w v) -> p u hl w v",
                                      u=A, hl=HL, w=W)
            for u in range(A):
                nc.sync.dma_start(cinv[:, u, 1:1 + HL, 1:1 + W, :], ysv[:, u])
            # interior output rows need only local y2 — run before halo
            def ev_c2(u, hp, nh, ps, b=b):
                col = u * (HL * W * A) + hp * (W * A)
                n = nh * W * A
                rt = pools["big"].tile([C, 2 * W * A], F32, tag="res_u", name=_tn("res2_u"))
                nc.sync.dma_start(rt[:, :n], sc2_dram[b][:, col: col + n])
                ot = pools["big"].tile([128, 2 * W * A], F32, tag="out_t", name=_tn("out_t"))
                nc.vector.tensor_tensor(
                    out=ot[:, :n], in0=ps[:, :n], in1=rt[:, :n], op=OP.add)
                nc.sync.dma_start(
                    out_ext.ap()[:, b * ch2 + col: b * ch2 + col + n],
                    ot[:, :n])

            self.conv_half(pools, cin, ev_c2, rows=[(1, 2)])
            blkA = (pid + NC - 1) % NC
            blkC = (pid + 1) % NC
            bav_t = bnd_all[b][ds(blkA * C, C), :].rearrange(
                "c (e u w v) -> c e u w v", e=2, u=A, w=W)
            nc.sync.dma_start(cinv[:, :, 0, 1:1 + W, :], bav_t[:, 1])
            bav_b = bnd_all[b][ds(blkC * C, C), :].rearrange(
                "c (e u w v) -> c e u w v", e=2, u=A, w=W)
            nc.sync.dma_start(cinv[:, :, 5, 1:1 + W, :], bav_b[:, 0])
            if isinstance(pid, int):
                if pid < 1:
                    nc.vector.memset(cinv[:, :, 0:1, :, :], 0.0)
                if pid > NC - 2:
                    nc.vector.memset(cinv[:, :, 5:6, :, :], 0.0)
            else:
                with tc.If(pid < 1):
                    nc.vector.memset(cinv[:, :, 0:1, :, :], 0.0)
                with tc.If(pid > NC - 2):
                    nc.vector.memset(cinv[:, :, 5:6, :, :], 0.0)
            self.conv_half(pools, cin, ev_c2, rows=[(0, 1), (HL - 1, 1)])


# ---------------------------------------------------------------- entry point

_CACHE = {}


def kernel(**inputs):
    import numpy as np
    from concourse.bass_utils import run_bass_kernel_spmd
    import os as _os
    cfg = Cfg()
    if "nc" not in _CACHE:
        abl = tuple(x for x in _os.environ.get("KABL", "").split(",") if x)
        ker = Ker(cfg, ablate=abl)
        _CACHE["nc"] = ker.build()
    nc = _CACHE["nc"]
    in_maps = host_prep(cfg, inputs)
    res = run_bass_kernel_spmd(nc, in_maps, core_ids=list(range(cfg.ncores)),
                               trace=False)
    outs = [res.results[i]["out"] for i in range(cfg.ncores)]
    return host_unshard(cfg, outs).astype(np.float32)



# revision 58
# speedup vs baseline: 1.3641x; 1.0377x over previous
"""Bass kernel for nn_AltFilter: dual-pass windowed transformer + conv.

Sharding: pass-1 data-parallel over w (8 chunks of W/8), pass-2 over h.
Between passes: AllGather of pass-1 output (bf16), conv read from gathered.

Layouts (per core, per b-half):
  xtok1   [C, (v, wl, u, h)]        pass-1 token input (host prepped, bf16)
  y_perm  [C, (u, hpad34, wl, v)]   pass-1 output staged for AG (bf16)
  x1_full [8*C, (u, hpad34, wl, v)] AG result, wc-major blocks
  conv1in [C, (u, 6, Wpad, v)]      conv window (bf16) ; w = 4*wc+wl
  sc2     [C, (u, hl, w, v)]        conv1+res (f32/bf16) == pass-2 tokens
  y2      [C, (u, hl, w, v)]        pass-2 out chunk (bf16) -> AG (10-block padded)
  out     [C, (b, u, hl, w, v)]     final (f32)
"""

import numpy as np
from dataclasses import dataclass
from concourse import bass, bacc, tile, mybir
from concourse.bass import ds

BF16 = mybir.dt.bfloat16
F32 = mybir.dt.float32
AF = mybir.ActivationFunctionType
OP = mybir.AluOpType


@dataclass
class Cfg:
    A: int = 5
    C: int = 128
    E: int = 256
    NH: int = 8
    B: int = 2
    H: int = 32
    W: int = 32
    ncores: int = 8
    ch_seqs: int = 5      # sequences per processing chunk
    win: int = 5          # attention half-window (KW//2)

    @property
    def HD(self):
        return self.E // self.NH

    @property
    def WL(self):
        return self.W // self.ncores

    @property
    def HL(self):
        return self.H // self.ncores

    @property
    def L1(self):
        return self.A * self.H       # pass-1 tokens per seq (u, h)

    @property
    def L2(self):
        return self.W * self.A       # pass-2 tokens per seq (w, v)

    @property
    def SEQ1(self):
        return self.A * self.WL      # per-b seqs pass 1 (v, wl)

    @property
    def SEQ2(self):
        return self.A * self.HL      # per-b seqs pass 2 (u, hl)

    @property
    def HPAD(self):
        return self.H + 2



_tname_ctr = [0]


def _tn(tag):
    _tname_ctr[0] += 1
    return f"{tag}_{_tname_ctr[0]}"

def mchunks(L):
    out = []
    o = 0
    while o < L:
        sz = min(128, L - o)
        out.append((o, sz))
        o += sz
    return out


# ---------------------------------------------------------------- host prep

def host_prep(cfg, inputs):
    """Build per-core in_maps from full inputs. Returns list of dicts."""
    import ml_dtypes
    bf = ml_dtypes.bfloat16
    A, C, E, B, H, W = cfg.A, cfg.C, cfg.E, cfg.B, cfg.H, cfg.W
    NC = cfg.ncores
    WL, HL = cfg.WL, cfg.HL

    buf = np.asarray(inputs["buffer"], np.float32)
    buf6 = buf.reshape(B, C, A, A, H, W)                    # b c u v h w

    # pass-1 tokens: [c, b, v, w, u, h] -> core k takes w slice
    xt = np.ascontiguousarray(buf6.transpose(1, 0, 3, 5, 2, 4))  # c b v w u h
    # conv1 residual: [c, b, u, h, w, v] -> core k takes h slice
    rs = np.ascontiguousarray(buf6.transpose(1, 0, 2, 4, 5, 3))  # c b u h w v

    ip = np.asarray(inputs["in_proj"], np.float32)
    sc = 1.0 / np.sqrt(cfg.HD)
    WqT = (ip[0:E].T * sc).astype(bf)
    WkT = ip[E:2 * E].T.astype(bf)
    WvT = ip[2 * E:3 * E].T.astype(bf)
    WinT = np.asarray(inputs["Win"], np.float32).T.astype(bf)       # (C, E)
    WoT = np.asarray(inputs["attn_out_w"], np.float32).T.astype(bf)  # (E, E)
    Wff1T = np.asarray(inputs["ff1"], np.float32).T.astype(bf)       # (E, 2E)
    Wff2T = np.asarray(inputs["ff2"], np.float32).T.astype(bf)       # (2E, E)
    WoutT = np.asarray(inputs["Wout"], np.float32).T.astype(bf)      # (E, C)
    cw = np.asarray(inputs["conv_w"], np.float32)[:, :, 0]           # (O,I,3,3)
    tapT = np.ascontiguousarray(cw.transpose(2, 3, 1, 0)).astype(bf)  # ky kx I O

    def band_mask(L, blk, n_outer, outer_major):
        # tokens: outer_major=True -> l = outer*blk_count... build via indices
        # pass1: l = u*H + h, band over h ; pass2: l = w*A + v, band over w
        l = np.arange(L)
        if outer_major:
            pos = l % blk          # h = l % H  (u-major, h inner)
        else:
            pos = l // n_outer     # w = l // A (w-major, v inner)
        d = np.abs(pos[:, None] - pos[None, :])
        m = np.where(d <= cfg.win, 0.0, -30000.0).astype(np.float32)
        return m.astype(bf)

    mask1 = band_mask(cfg.L1, cfg.H, cfg.A, True)
    mask2 = band_mask(cfg.L2, cfg.W, cfg.A, False)

    # attention-psum mask-init tiles: scA [128, 3L] = 3 head replicas of
    # mask rows 0:128; scB [128, 2L] = head-3 main + tail (rows 0:32);
    # scC [128, 3L] = 3 tail replicas at rows 0:32 (rest zero).
    def mk_masks(mask):
        m_main = np.asarray(mask[0:128, :], np.float32)   # [128, 160]
        m_tail = np.asarray(mask[128:160, :], np.float32)  # [32, 160]
        m3 = np.concatenate([m_main] * 3, axis=1)          # [128, 480]
        tail_pad = np.concatenate(
            [m_tail, np.full((96, m_tail.shape[1]), -30000.0, np.float32)], 0)
        mB = np.concatenate([m_main, tail_pad], axis=1)    # [128, 320]
        tail_z = np.concatenate(
            [m_tail, np.zeros((96, m_tail.shape[1]), np.float32)], 0)
        mC = np.concatenate([tail_z] * 3, axis=1)          # [128, 480]
        return (np.ascontiguousarray(m3).astype(bf),
                np.ascontiguousarray(mB).astype(bf),
                np.ascontiguousarray(mC).astype(bf))

    mask3_1, maskB_1, maskC_1 = mk_masks(mask1)
    mask3_2, maskB_2, maskC_2 = mk_masks(mask2)
    ident = np.eye(128, dtype=np.float32).astype(bf)

    lnw1 = np.asarray(inputs["ln_w"], np.float32).reshape(E // 128, 128).T.copy()
    lnb1 = np.asarray(inputs["ln_b"], np.float32).reshape(E // 128, 128).T.copy()
    lnw2 = np.asarray(inputs["ffn_ln_w"], np.float32).reshape(E // 128, 128).T.copy()
    lnb2 = np.asarray(inputs["ffn_ln_b"], np.float32).reshape(E // 128, 128).T.copy()

    shared = dict(WinT=WinT, WqT=WqT, WkT=WkT, WvT=WvT, WoT=WoT,
                  Wff1T=Wff1T, Wff2T=Wff2T, WoutT=WoutT, tapT=tapT,
                  mask3_1=mask3_1, maskB_1=maskB_1, maskC_1=maskC_1,
                  mask3_2=mask3_2, maskB_2=maskB_2, maskC_2=maskC_2,
                  ident=ident,
                  lnw1=lnw1, lnb1=lnb1, lnw2=lnw2, lnb2=lnb2)

    maps = []
    for k in range(NC):
        xk = xt[:, :, :, k * WL:(k + 1) * WL]   # c b v wl u h
        xk = np.ascontiguousarray(xk).reshape(C, -1).astype(bf)
        rk = rs[:, :, :, k * HL:(k + 1) * HL]   # c b u hl w v
        rk = np.ascontiguousarray(rk).reshape(C, -1).astype(np.float32)
        m = dict(shared)
        m["xtok1"] = xk
        m["res1"] = rk
        maps.append(m)
    return maps


def host_unshard(cfg, outs):
    """outs: list of per-core 'out' arrays [C, B*A*HL*W*A] -> full output."""
    A, C, B, H, W = cfg.A, cfg.C, cfg.B, cfg.H, cfg.W
    o = np.stack(outs)  # j c b u hl w v
    o = o.reshape(cfg.ncores, C, B, A, cfg.HL, W, A)
    o = o.transpose(2, 1, 3, 6, 0, 4, 5)  # b c u v j hl w
    return np.ascontiguousarray(o).reshape(B, C, A * A, H, W)


# ---------------------------------------------------------------- builder

class Ker:
    """Holds nc + pools + weight tiles while building."""

    _tables_pinned = False

    @classmethod
    def _pin_act_tables(cls):
        import os as _os
        if cls._tables_pinned or _os.environ.get("KTAB", "pin") != "pin":
            return
        cls._tables_pinned = True
        import concourse.bacc as _baccmod
        from concourse.hw_specs import get_activation_tables as _gat

        def pinned(arch):
            tabs = _gat(arch)
            keep = "natural_log_exp_and_others"
            mine = {AF.Exp, AF.Ln, AF.Copy, AF.Identity}
            out = {}
            for name, s in tabs.items():
                out[name] = s if name == keep else (s - mine)
            return out

        _baccmod.get_activation_tables = pinned

    def __init__(self, cfg, stage=4, ablate=()):
        self._pin_act_tables()
        self.cfg = cfg
        self.stage = stage  # 1=pass1, 2=+conv1, 3=+pass2, 4=full
        self.ablate = set(ablate)
        self.nc = bacc.Bacc("TRN2", target_bir_lowering=False, debug=False,
                            num_devices=cfg.ncores)

    # -- weights to sbuf ---------------------------------------------------
    def load_weights(self, tc, pool):
        nc, cfg = self.nc, self.cfg
        E = cfg.E

        def wtile(name, K, M):
            p = self.params[name]
            t = pool.tile([128, (K // 128) * M], BF16, tag=name)
            for kt in range(K // 128):
                nc.sync.dma_start(t[:, kt * M:(kt + 1) * M],
                                  p[kt * 128:(kt + 1) * 128, :])
            return t

        self.WinT = wtile("WinT", cfg.C, E)
        self.WqT = wtile("WqT", E, E)
        self.WkT = wtile("WkT", E, E)
        self.WvT = wtile("WvT", E, E)
        self.WoT = wtile("WoT", E, E)
        self.Wff1T = wtile("Wff1T", E, 2 * E)
        self.Wff2T = wtile("Wff2T", 2 * E, E)
        self.WoutT = wtile("WoutT", E, cfg.C)

        tap = self.params["tapT"]  # (3,3,I,O)
        self.taps = pool.tile([128, 9 * 128], BF16, tag="taps", name=_tn("taps"))
        for ky in range(3):
            for kx in range(3):
                i = ky * 3 + kx
                nc.sync.dma_start(self.taps[:, i * 128:(i + 1) * 128],
                                  tap[ky, kx])


        def lnt(name):
            t = pool.tile([128, E // 128], F32, tag=name)
            nc.sync.dma_start(t, self.params[name])
            return t

        self.lnw1, self.lnb1 = lnt("lnw1"), lnt("lnb1")
        self.lnw2, self.lnb2 = lnt("lnw2"), lnt("lnb2")

        self.ones = pool.tile([128, 128], BF16, tag="ones", name=_tn("ones"))
        nc.vector.memset(self.ones[:], 1.0 / E)

        def bftile(name, shape):
            t = pool.tile(shape, BF16, tag=name)
            nc.sync.dma_start(t, self.params[name])
            return t

        self.mask3_1 = bftile("mask3_1", [128, 480])
        self.maskB_1 = bftile("maskB_1", [128, 320])
        self.maskC_1 = bftile("maskC_1", [128, 480])
        self.mask3_2 = bftile("mask3_2", [128, 480])
        self.maskB_2 = bftile("maskB_2", [128, 320])
        self.maskC_2 = bftile("maskC_2", [128, 480])
        self.ident = bftile("ident", [128, 128])
        self.ones_att = pool.tile([128, 32], BF16, tag="ones_att",
                                  name=_tn("ones_att"))
        nc.vector.memset(self.ones_att[:], 1.0)


    # -- generic GEMM over one token chunk --------------------------------
    def gemm(self, psum_pool, wt, K, M, rhs_fn, ntok, nt_sz, evict_fn,
             tag="gemm", name=_tn("gemm")):
        """out[mt] = sum_kt  wt[kt,mt].T @ rhs(kt, nt) ; evict per (mt, nt)."""
        nc = self.nc
        KT, MT = K // 128, M // 128
        for mt in range(MT):
            for nt0 in range(0, ntok, nt_sz):
                n = min(nt_sz, ntok - nt0)
                ps = psum_pool.tile([128, nt_sz], F32, tag=tag)
                for kt in range(KT):
                    nc.tensor.matmul(
                        ps[:, :n],
                        wt[:, kt * M + mt * 128: kt * M + mt * 128 + 128],
                        rhs_fn(kt, nt0, n),
                        start=(kt == 0), stop=(kt == KT - 1))
                evict_fn(mt, nt0, n, ps)

    # -- layernorm over one chunk -----------------------------------------
    def ln_chunk_v2(self, tc, pools, x_f32, lnw, lnb, out_bf, ntok, nt_sz,
                    also_sq_src=None):
        """Feature-major LN. Stats are computed via all-ones matmuls whose
        M=128 stationary replicates sum across all partitions, so no
        partition-broadcast is ever needed."""
        nc, cfg = self.nc, self.cfg
        FT = cfg.E // 128
        x_bf = also_sq_src
        sq = pools["scratch"].tile([128, FT * ntok], BF16, tag="lnsq", name=_tn("lnsq"))
        for ft in range(FT):
            nc.vector.tensor_tensor(
                out=sq[:, ft * ntok:(ft + 1) * ntok],
                in0=x_bf[:, ft * ntok:(ft + 1) * ntok],
                in1=x_bf[:, ft * ntok:(ft + 1) * ntok], op=OP.mult)
        mean = pools["scratch"].tile([128, ntok], BF16, tag="lnmean", name=_tn("lnmean"))
        rstd = pools["scratch"].tile([128, ntok], F32, tag="lnrstd", name=_tn("lnrstd"))
        rstd_bf = pools["scratch"].tile([128, ntok], BF16, tag="lnrstdb", name=_tn("lnrstdb"))
        for nt0 in range(0, ntok, nt_sz):
            n = min(nt_sz, ntok - nt0)
            ps_s = pools["psum"].tile([128, nt_sz], F32, tag="gemm", name=_tn("lnstat"))
            ps_q = pools["psum"].tile([128, nt_sz], F32, tag="gemm", name=_tn("lnstat"))
            for kt in range(FT):
                nc.tensor.matmul(ps_s[:, :n], self.ones,
                                 x_bf[:, kt * ntok + nt0: kt * ntok + nt0 + n],
                                 start=(kt == 0), stop=(kt == FT - 1))
            for kt in range(FT):
                nc.tensor.matmul(ps_q[:, :n], self.ones,
                                 sq[:, kt * ntok + nt0: kt * ntok + nt0 + n],
                                 start=(kt == 0), stop=(kt == FT - 1))
            nc.vector.tensor_copy(out=mean[:, nt0:nt0 + n], in_=ps_s[:, :n])
            nc.vector.tensor_copy(out=rstd[:, nt0:nt0 + n], in_=ps_q[:, :n])
        # rstd = (E[x^2] + eps - mean^2) ** -0.5
        msq = pools["scratch"].tile([128, ntok], BF16, tag="lnmsq", name=_tn("lnmsq"))
        nc.vector.tensor_tensor(out=msq[:], in0=mean[:], in1=mean[:],
                                op=OP.mult)
        nc.vector.scalar_tensor_tensor(
            out=rstd[:], in0=rstd[:], scalar=1e-5, in1=msq[:],
            op0=OP.add, op1=OP.subtract)
        # rstd = exp(-0.5*ln(var)); Ln+Exp share one ACT table so no
        # table reloads against the attention exps. The Exp writes bf16
        # directly so the apply TTs below run in the 2x DVE mode.
        if "rsqrt" not in self.ablate:
            nc.scalar.activation(rstd[:], rstd[:], AF.Ln)
            nc.scalar.activation(rstd_bf[:], rstd[:], AF.Exp, scale=-0.5)
        else:
            nc.vector.tensor_copy(out=rstd_bf[:], in_=rstd[:])
        t1 = pools["scratch"].tile([128, nt_sz], BF16, tag="lnt1", name=_tn("lnt1"))
        for ft in range(FT):
            for nt0 in range(0, ntok, nt_sz):
                n = min(nt_sz, ntok - nt0)
                nc.vector.tensor_tensor(
                    out=t1[:, :n],
                    in0=x_bf[:, ft * ntok + nt0: ft * ntok + nt0 + n],
                    in1=mean[:, nt0:nt0 + n], op=OP.subtract)
                nc.vector.tensor_tensor(
                    out=t1[:, :n], in0=t1[:, :n],
                    in1=rstd_bf[:, nt0:nt0 + n], op=OP.mult)
                if "lnapply" in self.ablate:
                    nc.scalar.activation(
                        out_bf[:, ft * ntok + nt0: ft * ntok + nt0 + n],
                        t1[:, :n], AF.Copy)
                else:
                    nc.scalar.activation(
                        out_bf[:, ft * ntok + nt0: ft * ntok + nt0 + n],
                        t1[:, :n], AF.Identity, bias=lnb[:, ft:ft + 1],
                        scale=lnw[:, ft:ft + 1])

    # -- attention for one chunk of sequences ------------------------------
    def attention(self, pools, q_s, k_s, v_m, v_t, masks, o_bf, nseq, L,
                  ntok, ntokmax):
        """Per (seq, head-quad): scores for 4 heads into two psum banks
        (scA = heads 0-2 main [128, 3L]; scB = head-3 main [128, L] + the
        4 stacked [32, L] m-tails), mask added via one identity matmul from
        a precomputed SBUF tile, one Exp per bank, then col-tiled AV into an
        O|D bank: rows 32c = head c, cols 0:L = o, L:2L = denominator.
        One reciprocal + one multiply normalize all 4 heads."""
        nc, cfg = self.nc, self.cfg
        E = cfg.E
        mask3_t, maskB_t, maskC_t = masks
        for s in range(nseq):
            for qd in range(2):
                scA = pools["sc"].tile([128, 3 * L], F32, tag="scA", name=_tn("scA"))
                scB = pools["sc"].tile([128, 2 * L], F32, tag="scB", name=_tn("scB"))
                scC = pools["sc"].tile([32, 3 * L], F32, tag="scC", name=_tn("scC"))
                nc.tensor.matmul(scA, self.ident, mask3_t, start=True,
                                 stop=False, skip_group_check=True)
                nc.tensor.matmul(scB, self.ident, maskB_t, start=True,
                                 stop=False, skip_group_check=True)
                nc.tensor.matmul(scC, self.ident[:, 0:32], maskC_t,
                                 start=True, stop=False,
                                 skip_group_check=True)
                base = qd * ntokmax + s * L
                for c in range(4):
                    ks = k_s[4 * qd + c]
                    qs = q_s[4 * qd + c][0:32, s * L: (s + 1) * L]
                    out_main = (scA[:, L * c:L * (c + 1)] if c < 3
                                else scB[:, 0:L])
                    nc.tensor.matmul(out_main,
                                     ks[0:32, s * L: s * L + 128], qs,
                                     start=False, stop=True,
                                     skip_group_check=True)
                    tdst = (scC[0:32, L * c:L * (c + 1)] if c < 3
                            else scB[0:32, L:2 * L])
                    nc.tensor.matmul(tdst,
                                     ks[0:32, s * L + 128: (s + 1) * L], qs,
                                     start=False, stop=True,
                                     skip_group_check=True)
                exA = pools["exp"].tile([128, 3 * L], BF16, tag="exA", name=_tn("exA"))
                nc.scalar.activation(exA, scA, AF.Exp)
                exB = pools["exp"].tile([128, 2 * L], BF16, tag="exB", name=_tn("exB"))
                nc.scalar.activation(exB, scB, AF.Exp)
                exC = pools["exp"].tile([32, 3 * L], BF16, tag="exC", name=_tn("exC"))
                nc.scalar.activation(exC, scC, AF.Exp)
                # AV + denominators reuse scB's bank: its scores are dead
                # once exB is taken, so no extra PSUM bank is needed.
                for c in range(4):
                    r0 = 32 * c
                    hg = 4 * qd + c
                    exm = exA[:, L * c:L * (c + 1)] if c < 3 else exB[:, 0:L]
                    ext = (exC[0:32, L * c:L * (c + 1)] if c < 3
                           else exB[0:32, L:2 * L])
                    vm = v_m[:, s * E + 32 * hg: s * E + 32 * hg + 32]
                    vt = v_t[0:32, s * E + 32 * hg: s * E + 32 * hg + 32]
                    nc.tensor.matmul(scB[r0:r0 + 32, 0:L], vm, exm,
                                     start=True, stop=False,
                                     tile_position=(0, r0))
                    nc.tensor.matmul(scB[r0:r0 + 32, 0:L], vt, ext,
                                     start=False, stop=True,
                                     tile_position=(0, r0))
                    nc.tensor.matmul(scB[r0:r0 + 32, L:2 * L],
                                     self.ones_att[0:128, 0:32], exm,
                                     start=True, stop=False,
                                     tile_position=(0, r0))
                    nc.tensor.matmul(scB[r0:r0 + 32, L:2 * L],
                                     self.ones_att[0:32, 0:32], ext,
                                     start=False, stop=True,
                                     tile_position=(0, r0))
                rec = pools["scratch"].tile([128, L], F32, tag="rec", name=_tn("rec"))
                nc.vector.reciprocal_approx_fast(rec, scB[:, L:2 * L])
                nc.vector.tensor_tensor(
                    out=o_bf[:, qd * ntok + s * L: qd * ntok + (s + 1) * L],
                    in0=scB[:, 0:L], in1=rec, op=OP.mult)

    # -- one transformer pass over one b-half ------------------------------
    def transformer_half(self, tc, pools, x_src_fn, masks, L, nseq_b,
                         evict_y_fn):
        """x_src_fn(c0, ntok) -> bf16 [128, ntok] input token tile (Win rhs).
        evict_y_fn(s_global_pair_index, nt0, n, psum) writes final y."""
        nc, cfg = self.nc, self.cfg
        E = cfg.E
        FT = E // 128
        CH = cfg.ch_seqs
        NTs = 3 * L            # token tile = 3 seqs (psum [128,480] f32 fits a bank)
        ntokmax = CH * L
        # q_s/k_s: one 32-row tile per head (base partition 0), cols = tokens.
        # Avoids row-tiled matmuls (broken on HW); K=32 score MMs all run at
        # array rows 0:32.
        q_s = [pools["qk"].tile([32, ntokmax], BF16, tag=f"q_s{h}",
                                name=_tn(f"q_s{h}")) for h in range(cfg.NH)]
        k_s = [pools["qk"].tile([32, ntokmax], BF16, tag=f"k_s{h}",
                                name=_tn(f"k_s{h}")) for h in range(cfg.NH)]
        for c0 in range(0, nseq_b, CH):
            ns = min(CH, nseq_b - c0)
            ntok = ns * L
            x_bf = x_src_fn(c0, ntok)
            tok_f = pools["big1"].tile([128, FT * ntok], F32, tag="tok_f", name=_tn("tok_f"))
            tok_bf = pools["big"].tile([128, FT * ntok], BF16, tag="tok_bf", name=_tn("tok_bf"))

            def ev_tok(mt, nt0, n, ps):
                nc.vector.tensor_copy(
                    out=tok_f[:, mt * ntok + nt0: mt * ntok + nt0 + n],
                    in_=ps[:, :n])
                nc.scalar.activation(
                    tok_bf[:, mt * ntok + nt0: mt * ntok + nt0 + n],
                    ps[:, :n], AF.Copy)

            self.gemm(pools["psum"], self.WinT, cfg.C, E,
                      lambda kt, nt0, n: x_bf[:, nt0:nt0 + n],
                      ntok, NTs, ev_tok, tag="gemm", name=_tn("gemm"))

            tn = pools["big"].tile([128, FT * ntok], BF16, tag="tn", name=_tn("tn"))
            if "ln" in self.ablate:
                nc.vector.tensor_copy(out=tn, in_=tok_bf)
            else:
                self.ln_chunk_v2(tc, pools, tok_f, self.lnw1, self.lnb1, tn,
                                 ntok, NTs, also_sq_src=tok_bf)

            def mk_ev(dst, use_act):
                def ev(mt, nt0, n, ps):
                    for c in range(4):
                        d = dst[4 * mt + c][0:32, nt0: nt0 + n]
                        if use_act:
                            nc.scalar.activation(
                                d, ps[32 * c:32 * c + 32, :n], AF.Copy)
                        else:
                            nc.vector.tensor_copy(
                                out=d, in_=ps[32 * c:32 * c + 32, :n])
                return ev

            rhs_tn = lambda kt, nt0, n: tn[:, kt * ntok + nt0: kt * ntok + nt0 + n]
            self.gemm(pools["psum"], self.WqT, E, E, rhs_tn, ntok, NTs,
                      mk_ev(q_s, True), tag="gemm", name=_tn("gemm"))
            self.gemm(pools["psum"], self.WkT, E, E, rhs_tn, ntok, NTs,
                      mk_ev(k_s, False), tag="gemm", name=_tn("gemm"))

            # V token-major [tok, E]
            v_m = pools["big"].tile([128, CH * E], BF16, tag="v_m", name=_tn("v_m"))
            has_tail = L > 128
            v_t = None
            if has_tail:
                v_t = pools["big"].tile([32, CH * E], BF16,
                                        tag="v_t", name=_tn("v_t"))
            Lm = min(128, L)
            for s in range(ns):
                ps = pools["psum"].tile([128, E], F32, tag="gemm", name=_tn("gemm"))
                for kt in range(FT):
                    nc.tensor.matmul(
                        ps[0:Lm, :],
                        tok_bf[:, kt * ntok + s * L: kt * ntok + s * L + Lm],
                        self.WvT[:, kt * E:(kt + 1) * E],
                        start=(kt == 0), stop=(kt == FT - 1))
                nc.vector.tensor_copy(
                    out=v_m[0:Lm, s * E:(s + 1) * E], in_=ps[0:Lm, :])
            if has_tail:
                tl = L - 128
                for s in range(ns):
                    ps = pools["psum"].tile([128, E], F32, tag="gemm", name=_tn("gemm"))
                    for kt in range(FT):
                        nc.tensor.matmul(
                            ps[0:tl, :],
                            tok_bf[:, kt * ntok + s * L + 128:
                                   kt * ntok + s * L + 128 + tl],
                            self.WvT[:, kt * E:(kt + 1) * E],
                            start=(kt == 0), stop=(kt == FT - 1))
                    nc.scalar.activation(
                        v_t[0:tl, s * E:(s + 1) * E], ps[0:tl, :], AF.Copy)

            o_bf = pools["big"].tile([128, FT * ntok], BF16, tag="o_bf", name=_tn("o_bf"))
            if "attn" in self.ablate:
                nc.vector.tensor_copy(out=o_bf, in_=tn)
            else:
                self.attention(pools, q_s, k_s, v_m, v_t, masks,
                               o_bf, ns, L, ntok, ntokmax)

            # out-proj + residual
            outp_f = pools["big1"].tile([128, FT * ntok], F32, tag="outp_f", name=_tn("outp_f"))
            outp_bf = pools["big"].tile([128, FT * ntok], BF16, tag="outp_bf", name=_tn("outp_bf"))

            def ev_outp(mt, nt0, n, ps):
                nc.vector.tensor_tensor(
                    out=outp_f[:, mt * ntok + nt0: mt * ntok + nt0 + n],
                    in0=ps[:, :n],
                    in1=tok_f[:, mt * ntok + nt0: mt * ntok + nt0 + n],
                    op=OP.add)
                nc.scalar.activation(
                    outp_bf[:, mt * ntok + nt0: mt * ntok + nt0 + n],
                    outp_f[:, mt * ntok + nt0: mt * ntok + nt0 + n], AF.Copy)

            rhs_o = lambda kt, nt0, n: o_bf[:, kt * ntok + nt0: kt * ntok + nt0 + n]
            self.gemm(pools["psum"], self.WoT, E, E, rhs_o, ntok, NTs,
                      ev_outp, tag="gemm", name=_tn("gemm"))

            tn2 = pools["big"].tile([128, FT * ntok], BF16, tag="tn2", name=_tn("tn2"))
            if "ln" in self.ablate:
                nc.vector.tensor_copy(out=tn2, in_=outp_bf)
            else:
                self.ln_chunk_v2(tc, pools, outp_f, self.lnw2, self.lnb2, tn2,
                                 ntok, NTs, also_sq_src=outp_bf)

            ffh = pools["big"].tile([128, 2 * FT * ntok], BF16, tag="ffh", name=_tn("ffh"))

            def ev_ffh(mt, nt0, n, ps):
                nc.vector.tensor_relu(
                    out=ffh[:, mt * ntok + nt0: mt * ntok + nt0 + n],
                    in_=ps[:, :n])

            rhs_tn2 = lambda kt, nt0, n: tn2[:, kt * ntok + nt0: kt * ntok + nt0 + n]
            self.gemm(pools["psum"], self.Wff1T, E, 2 * E, rhs_tn2, ntok, NTs,
                      ev_ffh, tag="gemm", name=_tn("gemm"))

            res2_f = pools["big1"].tile([128, FT * ntok], F32, tag="res2_f", name=_tn("res2_f"))
            res2_bf = pools["big"].tile([128, FT * ntok], BF16, tag="res2_bf", name=_tn("res2_bf"))

            def ev_ffo(mt, nt0, n, ps):
                nc.vector.tensor_tensor(
                    out=res2_f[:, mt * ntok + nt0: mt * ntok + nt0 + n],
                    in0=ps[:, :n],
                    in1=outp_f[:, mt * ntok + nt0: mt * ntok + nt0 + n],
                    op=OP.add)
                nc.scalar.activation(
                    res2_bf[:, mt * ntok + nt0: mt * ntok + nt0 + n],
                    res2_f[:, mt * ntok + nt0: mt * ntok + nt0 + n], AF.Copy)

            rhs_ffh = lambda kt, nt0, n: ffh[:, kt * ntok + nt0: kt * ntok + nt0 + n]
            self.gemm(pools["psum"], self.Wff2T, 2 * E, E, rhs_ffh, ntok, NTs,
                      ev_ffo, tag="gemm", name=_tn("gemm"))

            rhs_r2 = lambda kt, nt0, n: res2_bf[:, kt * ntok + nt0: kt * ntok + nt0 + n]
            self.gemm(pools["psum"], self.WoutT, E, cfg.C, rhs_r2, ntok, NTs,
                      lambda mt, nt0, n, ps: evict_y_fn(c0, nt0, n, ps),
                      tag="gemm", name=_tn("gemm"))

    # -- conv 3x3 over (h,w) for one b-half --------------------------------
    def conv_half(self, pools, cin, evict_fn, rows=None):
        """cin: sbuf [128, A*6*WPAD*A(v)] bf16 window tile (w-padded).
        out pixels (u, hl, w, v); evict_fn(u, hp, n, psum) with pixel tiles
        [128, 2*W*A(v)].  rows: list of (hp, nh) output-row groups; default
        covers all HL rows in pairs."""
        nc, cfg = self.nc, self.cfg
        A, W = cfg.A, cfg.W
        WP = W + 2
        if rows is None:
            rows = [(hp, min(2, cfg.HL - hp)) for hp in range(0, cfg.HL, 2)]
        cv = cin.rearrange("p (u h w v) -> p u h w v", u=A, h=6, w=WP)
        for u in range(A):
            for hp, nh in rows:
                ps = pools["psum"].tile([128, 2 * W * A], F32, tag="gemm", name=_tn("gemm"))
                first = True
                for dy in range(3):
                    for dx in range(3):
                        rhs = cv[:, u, hp + dy:hp + dy + nh, dx:dx + W, :]
                        nc.tensor.matmul(
                            ps[:, :nh * W * A],
                            self.taps[:, (dy * 3 + dx) * 128:(dy * 3 + dx + 1) * 128],
                            rhs,
                            start=first, stop=(dy == 2 and dx == 2))
                        first = False
                evict_fn(u, hp, nh, ps)

    # -- full graph --------------------------------------------------------
    def build(self):
        cfg = self.cfg
        nc = self.nc
        A, C, E, B, H, W = cfg.A, cfg.C, cfg.E, cfg.B, cfg.H, cfg.W
        L1, L2 = cfg.L1, cfg.L2
        FT = E // 128
        WL, HL, HP = cfg.WL, cfg.HL, cfg.HPAD
        NC = cfg.ncores
        WPAD = W + 2
        chunk1_cols = A * HP * WL * A          # y_perm cols (u hpad wl v)
        chunk2_cols = A * HL * W * A           # y2 cols (u hl w v)

        # ---- dram params
        P = {}
        def par(name, shape, dt):
            P[name] = nc.dram_tensor(name, shape, dt, kind="ExternalInput").ap()
        par("xtok1", [C, B * cfg.SEQ1 * L1], BF16)
        par("res1", [C, B * chunk2_cols], F32)
        par("WinT", [C, E], BF16)
        for n in ("WqT", "WkT", "WvT", "WoT"):
            par(n, [E, E], BF16)
        par("Wff1T", [E, 2 * E], BF16)
        par("Wff2T", [2 * E, E], BF16)
        par("WoutT", [E, C], BF16)
        par("tapT", [3, 3, C, C], BF16)
        par("mask3_1", [128, 480], BF16)
        par("maskB_1", [128, 320], BF16)
        par("maskC_1", [128, 480], BF16)
        par("mask3_2", [128, 480], BF16)
        par("maskB_2", [128, 320], BF16)
        par("maskC_2", [128, 480], BF16)
        par("ident", [128, 128], BF16)
        for n in ("lnw1", "lnb1", "lnw2", "lnb2"):
            par(n, [128, FT], F32)
        self.params = P
        out_ext = nc.dram_tensor("out", [C, B * chunk2_cols], F32,
                                 kind="ExternalOutput")

        with tile.TileContext(nc) as tc:
            import contextlib
            with contextlib.ExitStack() as ctx:
                pools = {}
                pools["const"] = ctx.enter_context(
                    tc.tile_pool(name="const", bufs=1))
                pools["big"] = ctx.enter_context(
                    tc.tile_pool(name="big", bufs=2))
                pools["big1"] = ctx.enter_context(
                    tc.tile_pool(name="big1", bufs=1))
                pools["qk"] = ctx.enter_context(
                    tc.tile_pool(name="qk", bufs=1))
                pools["scratch"] = ctx.enter_context(
                    tc.tile_pool(name="scratch", bufs=1))
                pools["io"] = ctx.enter_context(
                    tc.tile_pool(name="io", bufs=1))
                pools["psum"] = ctx.enter_context(
                    tc.tile_pool(name="psum", bufs=2, space="PSUM"))
                pools["sc"] = ctx.enter_context(
                    tc.tile_pool(name="sc", bufs=2, space="PSUM"))
                pools["exp"] = ctx.enter_context(
                    tc.tile_pool(name="exp", bufs=2))
                pools["dram"] = ctx.enter_context(
                    tc.tile_pool(name="dram", bufs=1, space="DRAM"))
                self._build_body(tc, pools, out_ext)
        nc.compile()
        return nc

    def _build_body(self, tc, pools, out_ext):
        nc, cfg = self.nc, self.cfg
        A, C, E, B, H, W = cfg.A, cfg.C, cfg.E, cfg.B, cfg.H, cfg.W
        L1, L2 = cfg.L1, cfg.L2
        FT = E // 128
        WL, HL, HP = cfg.WL, cfg.HL, cfg.HPAD
        NC = cfg.ncores
        WPAD = W + 2
        ch1 = A * HP * WL * A
        ch2 = A * HL * W * A
        dram = pools["dram"]

        self.load_weights(tc, pools["const"])
        import os as _os
        _simpid = _os.environ.get("KSIMPID")
        pid = int(_simpid) if _simpid else nc.partition_id()

        x1_chunk = [dram.tile([C, ch1], BF16, tag=f"x1c{b}", name=_tn(f"x1c{b}")) for b in range(B)]
        x1_full = [dram.tile([NC * C, ch1], BF16, addr_space="Shared",
                             tag=f"x1f{b}", name=_tn(f"x1f{b}")) for b in range(B)]
        bnd_cols = 2 * A * W * A
        bnd_snd = [dram.tile([C, bnd_cols], BF16, tag=f"bs{b}", name=_tn(f"bs{b}")) for b in range(B)]
        bnd_all = [dram.tile([NC * C, bnd_cols], BF16, tag=f"ba{b}", name=_tn(f"ba{b}")) for b in range(B)]
        sc2_dram = [dram.tile([C, ch2], F32, tag=f"s2d{b}", name=_tn(f"s2d{b}")) for b in range(B)]

        # ---------------- pass 1 (per b-half)
        sc2_f = []
        y2_sbs = []
        for b in range(B):
            y_perm = pools["io"].tile([C, ch1], BF16, tag="y_perm", name=_tn("y_perm"))
            # zero hpad rows 0 and HP-1:  cols (u, {0,HP-1}, wl, v)
            yv = y_perm.rearrange("p (u h w v) -> p u h w v", u=A, h=HP, w=WL)
            nc.vector.memset(yv[:, :, 0:1, :, :], 0.0)
            nc.vector.memset(yv[:, :, HP - 1:HP, :, :], 0.0)

            def x_src(c0, ntok, b=b):
                t = pools["big"].tile([128, ntok], BF16, tag="x_in", name=_tn("x_in"))
                nc.sync.dma_start(
                    t, self.params["xtok1"][:, b * cfg.SEQ1 * L1 + c0 * L1:
                                            b * cfg.SEQ1 * L1 + c0 * L1 + ntok])
                return t

            def ev_y(c0, nt0, n, ps, y_perm=y_perm):
                # psum [128, n] tokens of seqs starting at s0=c0+nt0//L1
                # seq (v, wl): v = s//WL, wl = s%WL ; token (u, h)
                # y_perm col = u*(HP*WL*A) + (h+1)*(WL*A) + wl*A + v
                s0 = c0 + nt0 // L1
                npair = n // L1
                for i in range(0, npair):
                    s = s0 + i
                    v, wl = s // WL, s % WL
                    dst = y_perm.rearrange(
                        "p (u h w vv) -> p u h w vv", u=A, h=HP, w=WL)
                    nc.vector.tensor_copy(
                        out=dst[:, :, 1:H + 1, wl, v],
                        in_=ps[:, i * L1:(i + 1) * L1].rearrange(
                            "p (u h) -> p u h", h=H))

            self.transformer_half(tc, pools, x_src,
                                  (self.mask3_1, self.maskB_1, self.maskC_1),
                                  L1, cfg.SEQ1, ev_y)
            nc.sync.dma_start(x1_chunk[b][:, :], y_perm)
            nc.gpsimd.collective_compute(
                "AllGather", mybir.AluOpType.bypass,
                ins=[x1_chunk[b].opt()], outs=[x1_full[b].opt()],
                replica_groups=[list(range(NC))])

        # ---------------- conv1 + residual -> sc2 ; then pass 2
        for b in range(B):
            cin = pools["io"].tile([C, A * 6 * WPAD * A], BF16, tag="cin", name=_tn("cin"))
            cinv = cin.rearrange("p (u h w v) -> p u h w v", u=A, h=6, w=WPAD)
            nc.vector.memset(cinv[:, :, :, 0:1, :], 0.0)
            nc.vector.memset(cinv[:, :, :, WPAD - 1:WPAD, :], 0.0)
            xf = x1_full[b].rearrange("(wc c) (u h w v) -> wc c u h w v",
                                      c=C, u=A, h=HP, w=WL)
            for wc in range(NC):
                nc.sync.dma_start(
                    cinv[:, :, :, 1 + wc * WL:1 + (wc + 1) * WL, :],
                    xf[wc, :, :, ds(pid * HL, 6), :, :])
            scf = pools["io"].tile([C, ch2], BF16, tag=f"sc2f{b}", name=_tn(f"sc2f{b}"))
            sc2_f.append(scf)

            res_cache = {}

            def ev_c1b(u, hp, nh, ps, b=b, scf=scf, res_cache=res_cache):
                col = u * (HL * W * A) + hp * (W * A)
                n = nh * W * A
                if u not in res_cache:
                    rt = pools["big"].tile([C, HL * W * A], F32, tag="res_u", name=_tn("res_u"))
                    ub = u * (HL * W * A)
                    nc.sync.dma_start(
                        rt, self.params["res1"][:, b * ch2 + ub:
                                                b * ch2 + ub + HL * W * A])
                    res_cache[u] = rt
                sct = pools["big"].tile([C, 2 * W * A], F32, tag="out_t", name=_tn("sc_t"))
                nc.vector.tensor_tensor(
                    out=sct[:, :n], in0=ps[:, :n],
                    in1=res_cache[u][:, hp * W * A: hp * W * A + n], op=OP.add)
                nc.scalar.activation(scf[:, col:col + n], sct[:, :n], AF.Copy)
                nc.sync.dma_start(sc2_dram[b][:, col:col + n], sct[:, :n])

            self.conv_half(pools, cin, ev_c1b)

            # ---- pass 2 on sc2 (boundary hl rows first so the halo
            # exchange overlaps interior compute)
            y2_sb = pools["io"].tile([C, ch2], BF16, tag=f"y2sb{b}", name=_tn(f"y2sb{b}"))
            y2_sbs.append(y2_sb)
            perm2 = ([u * HL for u in range(A)]
                     + [u * HL + HL - 1 for u in range(A)]
                     + [u * HL + hl for hl in range(1, HL - 1)
                        for u in range(A)])

            def x_src2(c0, ntok, scf=scf):
                t = pools["big"].tile([128, ntok], BF16, tag="x2g", name=_tn("x2g"))
                for i in range(ntok // L2):
                    st = perm2[c0 + i]
                    nc.vector.tensor_copy(
                        out=t[:, i * L2:(i + 1) * L2],
                        in_=scf[:, st * L2:(st + 1) * L2])
                return t

            def ev_y2(c0, nt0, n, ps, y2_sb=y2_sb):
                for i in range(n // L2):
                    st = perm2[c0 + nt0 // L2 + i]
                    nc.vector.tensor_copy(
                        out=y2_sb[:, st * L2:(st + 1) * L2],
                        in_=ps[:, i * L2:(i + 1) * L2])

            self.transformer_half(tc, pools, x_src2,
                                  (self.mask3_2, self.maskB_2, self.maskC_2),
                                  L2, cfg.SEQ2, ev_y2)
            ysb_v = y2_sb.rearrange("p (u hl wv) -> p u hl wv", u=A, hl=HL)
            nc.sync.dma_start(
                bnd_snd[b][:, 0:A * W * A].rearrange(
                    "p (u wv) -> p u wv", u=A),
                ysb_v[:, :, 0, :])
            nc.sync.dma_start(
                bnd_snd[b][:, A * W * A:2 * A * W * A].rearrange(
                    "p (u wv) -> p u wv", u=A),
                ysb_v[:, :, HL - 1, :])
            nc.gpsimd.collective_compute(
                "AllGather", mybir.AluOpType.bypass,
                ins=[bnd_snd[b].opt()],
                outs=[bnd_all[b].opt()],
                replica_groups=[list(range(NC))])

        # ---------------- conv2 + sc2 -> out
        for b in range(B):
            cin = pools["io"].tile([C, A * 6 * WPAD * A], BF16, tag="cin", name=_tn("cin"))
            cinv = cin.rearrange("p (u h w v) -> p u h w v", u=A, h=6, w=WPAD)
            nc.vector.memset(cinv[:, :, :, 0:1, :], 0.0)
            nc.vector.memset(cinv[:, :, :, WPAD - 1:WPAD, :], 0.0)
            ysv = y2_sbs[b].rearrange("p (u hl w v) -> p u hl w v",
                                      u=A, hl=HL, w=W)
            for u in range(A):
                nc.sync.dma_start(cinv[:, u, 1:1 + HL, 1:1 + W, :], ysv[:, u])
            # interior output rows need only local y2 — run before halo
            def ev_c2(u, hp, nh, ps, b=b):
                col = u * (HL * W * A) + hp * (W * A)
                n = nh * W * A
                rt = pools["big"].tile([C, 2 * W * A], F32, tag="res_u", name=_tn("res2_u"))
                nc.sync.dma_start(rt[:, :n], sc2_dram[b][:, col: col + n])
                ot = pools["big"].tile([128, 2 * W * A], F32, tag="out_t", name=_tn("out_t"))
                nc.vector.tensor_tensor(
                    out=ot[:, :n], in0=ps[:, :n], in1=rt[:, :n], op=OP.add)
                nc.sync.dma_start(
                    out_ext.ap()[:, b * ch2 + col: b * ch2 + col + n],
                    ot[:, :n])

            self.conv_half(pools, cin, ev_c2, rows=[(1, 2)])
            blkA = (pid + NC - 1) % NC
            blkC = (pid + 1) % NC
            bav_t = bnd_all[b][ds(blkA * C, C), :].rearrange(
                "c (e u w v) -> c e u w v", e=2, u=A, w=W)
            nc.sync.dma_start(cinv[:, :, 0, 1:1 + W, :], bav_t[:, 1])
            bav_b = bnd_all[b][ds(blkC * C, C), :].rearrange(
                "c (e u w v) -> c e u w v", e=2, u=A, w=W)
            nc.sync.dma_start(cinv[:, :, 5, 1:1 + W, :], bav_b[:, 0])
            if isinstance(pid, int):
                if pid < 1:
                    nc.vector.memset(cinv[:, :, 0:1, :, :], 0.0)
                if pid > NC - 2:
                    nc.vector.memset(cinv[:, :, 5:6, :, :], 0.0)
            else:
                with tc.If(pid < 1):
                    nc.vector.memset(cinv[:, :, 0:1, :, :], 0.0)
                with tc.If(pid > NC - 2):
                    nc.vector.memset(cinv[:, :, 5:6, :, :], 0.0)
            self.conv_half(pools, cin, ev_c2, rows=[(0, 1), (HL - 1, 1)])


# ---------------------------------------------------------------- entry point

_CACHE = {}


def kernel(**inputs):
    import numpy as np
    from concourse.bass_utils import run_bass_kernel_spmd
    import os as _os
    cfg = Cfg()
    if "nc" not in _CACHE:
        abl = tuple(x for x in _os.environ.get("KABL", "").split(",") if x)
        ker = Ker(cfg, ablate=abl)
        _CACHE["nc"] = ker.build()
    nc = _CACHE["nc"]
    in_maps = host_prep(cfg, inputs)
    res = run_bass_kernel_spmd(nc, in_maps, core_ids=list(range(cfg.ncores)),
                               trace=False)
    outs = [res.results[i]["out"] for i in range(cfg.ncores)]
    return host_unshard(cfg, outs).astype(np.float32)



# revision 59
# speedup vs baseline: 1.3687x; 1.0034x over previous
"""Bass kernel for nn_AltFilter: dual-pass windowed transformer + conv.

Sharding: pass-1 data-parallel over w (8 chunks of W/8), pass-2 over h.
Between passes: AllGather of pass-1 output (bf16), conv read from gathered.

Layouts (per core, per b-half):
  xtok1   [C, (v, wl, u, h)]        pass-1 token input (host prepped, bf16)
  y_perm  [C, (u, hpad34, wl, v)]   pass-1 output staged for AG (bf16)
  x1_full [8*C, (u, hpad34, wl, v)] AG result, wc-major blocks
  conv1in [C, (u, 6, Wpad, v)]      conv window (bf16) ; w = 4*wc+wl
  sc2     [C, (u, hl, w, v)]        conv1+res (f32/bf16) == pass-2 tokens
  y2      [C, (u, hl, w, v)]        pass-2 out chunk (bf16) -> AG (10-block padded)
  out     [C, (b, u, hl, w, v)]     final (f32)
"""

import numpy as np
from dataclasses import dataclass
from concourse import bass, bacc, tile, mybir
from concourse.bass import ds

BF16 = mybir.dt.bfloat16
F32 = mybir.dt.float32
AF = mybir.ActivationFunctionType
OP = mybir.AluOpType


@dataclass
class Cfg:
    A: int = 5
    C: int = 128
    E: int = 256
    NH: int = 8
    B: int = 2
    H: int = 32
    W: int = 32
    ncores: int = 8
    ch_seqs: int = 5      # sequences per processing chunk
    win: int = 5          # attention half-window (KW//2)

    @property
    def HD(self):
        return self.E // self.NH

    @property
    def WL(self):
        return self.W // self.ncores

    @property
    def HL(self):
        return self.H // self.ncores

    @property
    def L1(self):
        return self.A * self.H       # pass-1 tokens per seq (u, h)

    @property
    def L2(self):
        return self.W * self.A       # pass-2 tokens per seq (w, v)

    @property
    def SEQ1(self):
        return self.A * self.WL      # per-b seqs pass 1 (v, wl)

    @property
    def SEQ2(self):
        return self.A * self.HL      # per-b seqs pass 2 (u, hl)

    @property
    def HPAD(self):
        return self.H + 2



_tname_ctr = [0]


def _tn(tag):
    _tname_ctr[0] += 1
    return f"{tag}_{_tname_ctr[0]}"

def mchunks(L):
    out = []
    o = 0
    while o < L:
        sz = min(128, L - o)
        out.append((o, sz))
        o += sz
    return out


# ---------------------------------------------------------------- host prep

def host_prep(cfg, inputs):
    """Build per-core in_maps from full inputs. Returns list of dicts."""
    import ml_dtypes
    bf = ml_dtypes.bfloat16
    A, C, E, B, H, W = cfg.A, cfg.C, cfg.E, cfg.B, cfg.H, cfg.W
    NC = cfg.ncores
    WL, HL = cfg.WL, cfg.HL

    buf = np.asarray(inputs["buffer"], np.float32)
    buf6 = buf.reshape(B, C, A, A, H, W)                    # b c u v h w

    # pass-1 tokens: [c, b, v, w, u, h] -> core k takes w slice
    xt = np.ascontiguousarray(buf6.transpose(1, 0, 3, 5, 2, 4))  # c b v w u h
    # conv1 residual: [c, b, u, h, w, v] -> core k takes h slice
    rs = np.ascontiguousarray(buf6.transpose(1, 0, 2, 4, 5, 3))  # c b u h w v

    ip = np.asarray(inputs["in_proj"], np.float32)
    sc = 1.0 / np.sqrt(cfg.HD)
    WqT = (ip[0:E].T * sc).astype(bf)
    WkT = ip[E:2 * E].T.astype(bf)
    WvT = ip[2 * E:3 * E].T.astype(bf)
    WinT = np.asarray(inputs["Win"], np.float32).T.astype(bf)       # (C, E)
    WoT = np.asarray(inputs["attn_out_w"], np.float32).T.astype(bf)  # (E, E)
    Wff1T = np.asarray(inputs["ff1"], np.float32).T.astype(bf)       # (E, 2E)
    Wff2T = np.asarray(inputs["ff2"], np.float32).T.astype(bf)       # (2E, E)
    WoutT = np.asarray(inputs["Wout"], np.float32).T.astype(bf)      # (E, C)
    cw = np.asarray(inputs["conv_w"], np.float32)[:, :, 0]           # (O,I,3,3)
    tapT = np.ascontiguousarray(cw.transpose(2, 3, 1, 0)).astype(bf)  # ky kx I O

    def band_mask(L, blk, n_outer, outer_major):
        # tokens: outer_major=True -> l = outer*blk_count... build via indices
        # pass1: l = u*H + h, band over h ; pass2: l = w*A + v, band over w
        l = np.arange(L)
        if outer_major:
            pos = l % blk          # h = l % H  (u-major, h inner)
        else:
            pos = l // n_outer     # w = l // A (w-major, v inner)
        d = np.abs(pos[:, None] - pos[None, :])
        m = np.where(d <= cfg.win, 0.0, -30000.0).astype(np.float32)
        return m.astype(bf)

    mask1 = band_mask(cfg.L1, cfg.H, cfg.A, True)
    mask2 = band_mask(cfg.L2, cfg.W, cfg.A, False)

    # attention-psum mask-init tiles: scA [128, 3L] = 3 head replicas of
    # mask rows 0:128; scB [128, 2L] = head-3 main + tail (rows 0:32);
    # scC [128, 3L] = 3 tail replicas at rows 0:32 (rest zero).
    def mk_masks(mask):
        m_main = np.asarray(mask[0:128, :], np.float32)   # [128, 160]
        m_tail = np.asarray(mask[128:160, :], np.float32)  # [32, 160]
        m3 = np.concatenate([m_main] * 3, axis=1)          # [128, 480]
        tail_pad = np.concatenate(
            [m_tail, np.full((96, m_tail.shape[1]), -30000.0, np.float32)], 0)
        mB = np.concatenate([m_main, tail_pad], axis=1)    # [128, 320]
        tail_z = np.concatenate(
            [m_tail, np.zeros((96, m_tail.shape[1]), np.float32)], 0)
        mC = np.concatenate([tail_z] * 3, axis=1)          # [128, 480]
        return (np.ascontiguousarray(m3).astype(bf),
                np.ascontiguousarray(mB).astype(bf),
                np.ascontiguousarray(mC).astype(bf))

    mask3_1, maskB_1, maskC_1 = mk_masks(mask1)
    mask3_2, maskB_2, maskC_2 = mk_masks(mask2)
    ident = np.eye(128, dtype=np.float32).astype(bf)

    lnw1 = np.asarray(inputs["ln_w"], np.float32).reshape(E // 128, 128).T.copy()
    lnb1 = np.asarray(inputs["ln_b"], np.float32).reshape(E // 128, 128).T.copy()
    lnw2 = np.asarray(inputs["ffn_ln_w"], np.float32).reshape(E // 128, 128).T.copy()
    lnb2 = np.asarray(inputs["ffn_ln_b"], np.float32).reshape(E // 128, 128).T.copy()

    shared = dict(WinT=WinT, WqT=WqT, WkT=WkT, WvT=WvT, WoT=WoT,
                  Wff1T=Wff1T, Wff2T=Wff2T, WoutT=WoutT, tapT=tapT,
                  mask3_1=mask3_1, maskB_1=maskB_1, maskC_1=maskC_1,
                  mask3_2=mask3_2, maskB_2=maskB_2, maskC_2=maskC_2,
                  ident=ident,
                  lnw1=lnw1, lnb1=lnb1, lnw2=lnw2, lnb2=lnb2)

    maps = []
    for k in range(NC):
        xk = xt[:, :, :, k * WL:(k + 1) * WL]   # c b v wl u h
        xk = np.ascontiguousarray(xk).reshape(C, -1).astype(bf)
        rk = rs[:, :, :, k * HL:(k + 1) * HL]   # c b u hl w v
        rk = np.ascontiguousarray(rk).reshape(C, -1).astype(np.float32)
        m = dict(shared)
        m["xtok1"] = xk
        m["res1"] = rk
        maps.append(m)
    return maps


def host_unshard(cfg, outs):
    """outs: list of per-core 'out' arrays [C, B*A*HL*W*A] -> full output."""
    A, C, B, H, W = cfg.A, cfg.C, cfg.B, cfg.H, cfg.W
    o = np.stack(outs)  # j c b u hl w v
    o = o.reshape(cfg.ncores, C, B, A, cfg.HL, W, A)
    o = o.transpose(2, 1, 3, 6, 0, 4, 5)  # b c u v j hl w
    return np.ascontiguousarray(o).reshape(B, C, A * A, H, W)


# ---------------------------------------------------------------- builder

class Ker:
    """Holds nc + pools + weight tiles while building."""

    _tables_pinned = False

    @classmethod
    def _pin_act_tables(cls):
        import os as _os
        if cls._tables_pinned or _os.environ.get("KTAB", "pin") != "pin":
            return
        cls._tables_pinned = True
        import concourse.bacc as _baccmod
        from concourse.hw_specs import get_activation_tables as _gat

        def pinned(arch):
            tabs = _gat(arch)
            keep = "natural_log_exp_and_others"
            mine = {AF.Exp, AF.Ln, AF.Copy, AF.Identity}
            out = {}
            for name, s in tabs.items():
                out[name] = s if name == keep else (s - mine)
            return out

        _baccmod.get_activation_tables = pinned

    def __init__(self, cfg, stage=4, ablate=()):
        self._pin_act_tables()
        self.cfg = cfg
        self.stage = stage  # 1=pass1, 2=+conv1, 3=+pass2, 4=full
        self.ablate = set(ablate)
        self.nc = bacc.Bacc("TRN2", target_bir_lowering=False, debug=False,
                            num_devices=cfg.ncores)

    # -- weights to sbuf ---------------------------------------------------
    def load_weights(self, tc, pool):
        nc, cfg = self.nc, self.cfg
        E = cfg.E

        def wtile(name, K, M):
            p = self.params[name]
            t = pool.tile([128, (K // 128) * M], BF16, tag=name)
            for kt in range(K // 128):
                nc.sync.dma_start(t[:, kt * M:(kt + 1) * M],
                                  p[kt * 128:(kt + 1) * 128, :])
            return t

        self.WinT = wtile("WinT", cfg.C, E)
        self.WqT = wtile("WqT", E, E)
        self.WkT = wtile("WkT", E, E)
        self.WvT = wtile("WvT", E, E)
        self.WoT = wtile("WoT", E, E)
        self.Wff1T = wtile("Wff1T", E, 2 * E)
        self.Wff2T = wtile("Wff2T", 2 * E, E)
        self.WoutT = wtile("WoutT", E, cfg.C)

        tap = self.params["tapT"]  # (3,3,I,O)
        self.taps = pool.tile([128, 9 * 128], BF16, tag="taps", name=_tn("taps"))
        for ky in range(3):
            for kx in range(3):
                i = ky * 3 + kx
                nc.sync.dma_start(self.taps[:, i * 128:(i + 1) * 128],
                                  tap[ky, kx])


        def lnt(name):
            t = pool.tile([128, E // 128], F32, tag=name)
            nc.sync.dma_start(t, self.params[name])
            return t

        self.lnw1, self.lnb1 = lnt("lnw1"), lnt("lnb1")
        self.lnw2, self.lnb2 = lnt("lnw2"), lnt("lnb2")

        self.ones = pool.tile([128, 128], BF16, tag="ones", name=_tn("ones"))
        nc.vector.memset(self.ones[:], 1.0 / E)

        def bftile(name, shape):
            t = pool.tile(shape, BF16, tag=name)
            nc.sync.dma_start(t, self.params[name])
            return t

        self.mask3_1 = bftile("mask3_1", [128, 480])
        self.maskB_1 = bftile("maskB_1", [128, 320])
        self.maskC_1 = bftile("maskC_1", [128, 480])
        self.mask3_2 = bftile("mask3_2", [128, 480])
        self.maskB_2 = bftile("maskB_2", [128, 320])
        self.maskC_2 = bftile("maskC_2", [128, 480])
        self.ident = bftile("ident", [128, 128])
        self.ones_att = pool.tile([128, 32], BF16, tag="ones_att",
                                  name=_tn("ones_att"))
        nc.vector.memset(self.ones_att[:], 1.0)


    # -- generic GEMM over one token chunk --------------------------------
    def gemm(self, psum_pool, wt, K, M, rhs_fn, ntok, nt_sz, evict_fn,
             tag="gemm", name=_tn("gemm")):
        """out[mt] = sum_kt  wt[kt,mt].T @ rhs(kt, nt) ; evict per (mt, nt)."""
        nc = self.nc
        KT, MT = K // 128, M // 128
        for mt in range(MT):
            for nt0 in range(0, ntok, nt_sz):
                n = min(nt_sz, ntok - nt0)
                ps = psum_pool.tile([128, nt_sz], F32, tag=tag)
                for kt in range(KT):
                    nc.tensor.matmul(
                        ps[:, :n],
                        wt[:, kt * M + mt * 128: kt * M + mt * 128 + 128],
                        rhs_fn(kt, nt0, n),
                        start=(kt == 0), stop=(kt == KT - 1))
                evict_fn(mt, nt0, n, ps)

    # -- layernorm over one chunk -----------------------------------------
    def ln_chunk_v2(self, tc, pools, x_f32, lnw, lnb, out_bf, ntok, nt_sz,
                    also_sq_src=None):
        """Feature-major LN. Stats are computed via all-ones matmuls whose
        M=128 stationary replicates sum across all partitions, so no
        partition-broadcast is ever needed."""
        nc, cfg = self.nc, self.cfg
        FT = cfg.E // 128
        x_bf = also_sq_src
        sq = pools["scratch"].tile([128, FT * ntok], BF16, tag="lnsq", name=_tn("lnsq"))
        for ft in range(FT):
            nc.vector.tensor_tensor(
                out=sq[:, ft * ntok:(ft + 1) * ntok],
                in0=x_bf[:, ft * ntok:(ft + 1) * ntok],
                in1=x_bf[:, ft * ntok:(ft + 1) * ntok], op=OP.mult)
        mean = pools["scratch"].tile([128, ntok], BF16, tag="lnmean", name=_tn("lnmean"))
        rstd = pools["scratch"].tile([128, ntok], F32, tag="lnrstd", name=_tn("lnrstd"))
        rstd_bf = pools["scratch"].tile([128, ntok], BF16, tag="lnrstdb", name=_tn("lnrstdb"))
        for nt0 in range(0, ntok, nt_sz):
            n = min(nt_sz, ntok - nt0)
            ps_s = pools["psum"].tile([128, nt_sz], F32, tag="gemm", name=_tn("lnstat"))
            ps_q = pools["psum"].tile([128, nt_sz], F32, tag="gemm", name=_tn("lnstat"))
            for kt in range(FT):
                nc.tensor.matmul(ps_s[:, :n], self.ones,
                                 x_bf[:, kt * ntok + nt0: kt * ntok + nt0 + n],
                                 start=(kt == 0), stop=(kt == FT - 1))
            for kt in range(FT):
                nc.tensor.matmul(ps_q[:, :n], self.ones,
                                 sq[:, kt * ntok + nt0: kt * ntok + nt0 + n],
                                 start=(kt == 0), stop=(kt == FT - 1))
            nc.vector.tensor_copy(out=mean[:, nt0:nt0 + n], in_=ps_s[:, :n])
            nc.vector.tensor_copy(out=rstd[:, nt0:nt0 + n], in_=ps_q[:, :n])
        # rstd = (E[x^2] + eps - mean^2) ** -0.5
        msq = pools["scratch"].tile([128, ntok], BF16, tag="lnmsq", name=_tn("lnmsq"))
        nc.vector.tensor_tensor(out=msq[:], in0=mean[:], in1=mean[:],
                                op=OP.mult)
        nc.vector.scalar_tensor_tensor(
            out=rstd[:], in0=rstd[:], scalar=1e-5, in1=msq[:],
            op0=OP.add, op1=OP.subtract)
        # rstd = exp(-0.5*ln(var)); Ln+Exp share one ACT table so no
        # table reloads against the attention exps. The Exp writes bf16
        # directly so the apply TTs below run in the 2x DVE mode.
        if "rsqrt" not in self.ablate:
            nc.scalar.activation(rstd[:], rstd[:], AF.Ln)
            nc.scalar.activation(rstd_bf[:], rstd[:], AF.Exp, scale=-0.5)
        else:
            nc.vector.tensor_copy(out=rstd_bf[:], in_=rstd[:])
        t1 = pools["scratch"].tile([128, nt_sz], BF16, tag="lnt1", name=_tn("lnt1"))
        for ft in range(FT):
            for nt0 in range(0, ntok, nt_sz):
                n = min(nt_sz, ntok - nt0)
                nc.vector.tensor_tensor(
                    out=t1[:, :n],
                    in0=x_bf[:, ft * ntok + nt0: ft * ntok + nt0 + n],
                    in1=mean[:, nt0:nt0 + n], op=OP.subtract)
                nc.vector.tensor_tensor(
                    out=t1[:, :n], in0=t1[:, :n],
                    in1=rstd_bf[:, nt0:nt0 + n], op=OP.mult)
                if "lnapply" in self.ablate:
                    nc.scalar.activation(
                        out_bf[:, ft * ntok + nt0: ft * ntok + nt0 + n],
                        t1[:, :n], AF.Copy)
                else:
                    nc.scalar.activation(
                        out_bf[:, ft * ntok + nt0: ft * ntok + nt0 + n],
                        t1[:, :n], AF.Identity, bias=lnb[:, ft:ft + 1],
                        scale=lnw[:, ft:ft + 1])

    # -- attention for one chunk of sequences ------------------------------
    def attention(self, pools, q_s, k_s, v_m, v_t, masks, o_bf, nseq, L,
                  ntok, ntokmax):
        """Per (seq, head-quad): scores for 4 heads into two psum banks
        (scA = heads 0-2 main [128, 3L]; scB = head-3 main [128, L] + the
        4 stacked [32, L] m-tails), mask added via one identity matmul from
        a precomputed SBUF tile, one Exp per bank, then col-tiled AV into an
        O|D bank: rows 32c = head c, cols 0:L = o, L:2L = denominator.
        One reciprocal + one multiply normalize all 4 heads."""
        nc, cfg = self.nc, self.cfg
        E = cfg.E
        mask3_t, maskB_t, maskC_t = masks
        for s in range(nseq):
            for qd in range(2):
                scA = pools["sc"].tile([128, 3 * L], F32, tag="scA", name=_tn("scA"))
                scB = pools["sc"].tile([128, 2 * L], F32, tag="scB", name=_tn("scB"))
                scC = pools["sc"].tile([32, 3 * L], F32, tag="scC", name=_tn("scC"))
                nc.tensor.matmul(scA, self.ident, mask3_t, start=True,
                                 stop=False, skip_group_check=True)
                nc.tensor.matmul(scB, self.ident, maskB_t, start=True,
                                 stop=False, skip_group_check=True)
                nc.tensor.matmul(scC, self.ident[:, 0:32], maskC_t,
                                 start=True, stop=False,
                                 skip_group_check=True)
                base = qd * ntokmax + s * L
                for c in range(4):
                    ks = k_s[4 * qd + c]
                    qs = q_s[4 * qd + c][0:32, s * L: (s + 1) * L]
                    out_main = (scA[:, L * c:L * (c + 1)] if c < 3
                                else scB[:, 0:L])
                    nc.tensor.matmul(out_main,
                                     ks[0:32, s * L: s * L + 128], qs,
                                     start=False, stop=True,
                                     skip_group_check=True)
                    tdst = (scC[0:32, L * c:L * (c + 1)] if c < 3
                            else scB[0:32, L:2 * L])
                    nc.tensor.matmul(tdst,
                                     ks[0:32, s * L + 128: (s + 1) * L], qs,
                                     start=False, stop=True,
                                     skip_group_check=True)
                exA = pools["exp"].tile([128, 3 * L], BF16, tag="exA", name=_tn("exA"))
                nc.scalar.activation(exA, scA, AF.Exp)
                exB = pools["exp"].tile([128, 2 * L], BF16, tag="exB", name=_tn("exB"))
                nc.scalar.activation(exB, scB, AF.Exp)
                exC = pools["exp"].tile([32, 3 * L], BF16, tag="exC", name=_tn("exC"))
                nc.scalar.activation(exC, scC, AF.Exp)
                # AV + denominators reuse scB's bank: its scores are dead
                # once exB is taken, so no extra PSUM bank is needed.
                for c in range(4):
                    r0 = 32 * c
                    hg = 4 * qd + c
                    exm = exA[:, L * c:L * (c + 1)] if c < 3 else exB[:, 0:L]
                    ext = (exC[0:32, L * c:L * (c + 1)] if c < 3
                           else exB[0:32, L:2 * L])
                    vm = v_m[:, s * E + 32 * hg: s * E + 32 * hg + 32]
                    vt = v_t[0:32, s * E + 32 * hg: s * E + 32 * hg + 32]
                    nc.tensor.matmul(scB[r0:r0 + 32, 0:L], vm, exm,
                                     start=True, stop=False,
                                     tile_position=(0, r0))
                    nc.tensor.matmul(scB[r0:r0 + 32, 0:L], vt, ext,
                                     start=False, stop=True,
                                     tile_position=(0, r0))
                    nc.tensor.matmul(scB[r0:r0 + 32, L:2 * L],
                                     self.ones_att[0:128, 0:32], exm,
                                     start=True, stop=False,
                                     tile_position=(0, r0))
                    nc.tensor.matmul(scB[r0:r0 + 32, L:2 * L],
                                     self.ones_att[0:32, 0:32], ext,
                                     start=False, stop=True,
                                     tile_position=(0, r0))
                rec = pools["scratch"].tile([128, L], F32, tag="rec", name=_tn("rec"))
                nc.vector.reciprocal_approx_fast(rec, scB[:, L:2 * L])
                nc.vector.tensor_tensor(
                    out=o_bf[:, qd * ntok + s * L: qd * ntok + (s + 1) * L],
                    in0=scB[:, 0:L], in1=rec, op=OP.mult)

    # -- one transformer pass over one b-half ------------------------------
    def transformer_half(self, tc, pools, x_src_fn, masks, L, nseq_b,
                         evict_y_fn):
        """x_src_fn(c0, ntok) -> bf16 [128, ntok] input token tile (Win rhs).
        evict_y_fn(s_global_pair_index, nt0, n, psum) writes final y."""
        nc, cfg = self.nc, self.cfg
        E = cfg.E
        FT = E // 128
        CH = cfg.ch_seqs
        NTs = 3 * L            # token tile = 3 seqs (psum [128,480] f32 fits a bank)
        ntokmax = CH * L
        # q_s/k_s: one 32-row tile per head (base partition 0), cols = tokens.
        # Avoids row-tiled matmuls (broken on HW); K=32 score MMs all run at
        # array rows 0:32.
        q_s = [pools["qk"].tile([32, ntokmax], BF16, tag=f"q_s{h}",
                                name=_tn(f"q_s{h}")) for h in range(cfg.NH)]
        k_s = [pools["qk"].tile([32, ntokmax], BF16, tag=f"k_s{h}",
                                name=_tn(f"k_s{h}")) for h in range(cfg.NH)]
        for c0 in range(0, nseq_b, CH):
            ns = min(CH, nseq_b - c0)
            ntok = ns * L
            x_bf = x_src_fn(c0, ntok)
            tok_f = pools["big1"].tile([128, FT * ntok], F32, tag="tok_f", name=_tn("tok_f"))
            tok_bf = pools["big"].tile([128, FT * ntok], BF16, tag="tok_bf", name=_tn("tok_bf"))

            def ev_tok(mt, nt0, n, ps):
                nc.vector.tensor_copy(
                    out=tok_f[:, mt * ntok + nt0: mt * ntok + nt0 + n],
                    in_=ps[:, :n])
                nc.scalar.activation(
                    tok_bf[:, mt * ntok + nt0: mt * ntok + nt0 + n],
                    ps[:, :n], AF.Copy)

            self.gemm(pools["psum"], self.WinT, cfg.C, E,
                      lambda kt, nt0, n: x_bf[:, nt0:nt0 + n],
                      ntok, NTs, ev_tok, tag="gemm", name=_tn("gemm"))

            tn = pools["big"].tile([128, FT * ntok], BF16, tag="tn", name=_tn("tn"))
            if "ln" in self.ablate:
                nc.vector.tensor_copy(out=tn, in_=tok_bf)
            else:
                self.ln_chunk_v2(tc, pools, tok_f, self.lnw1, self.lnb1, tn,
                                 ntok, NTs, also_sq_src=tok_bf)

            def mk_ev(dst, use_act):
                def ev(mt, nt0, n, ps):
                    for c in range(4):
                        d = dst[4 * mt + c][0:32, nt0: nt0 + n]
                        if use_act:
                            nc.scalar.activation(
                                d, ps[32 * c:32 * c + 32, :n], AF.Copy)
                        else:
                            nc.vector.tensor_copy(
                                out=d, in_=ps[32 * c:32 * c + 32, :n])
                return ev

            rhs_tn = lambda kt, nt0, n: tn[:, kt * ntok + nt0: kt * ntok + nt0 + n]
            self.gemm(pools["psum"], self.WqT, E, E, rhs_tn, ntok, NTs,
                      mk_ev(q_s, True), tag="gemm", name=_tn("gemm"))
            self.gemm(pools["psum"], self.WkT, E, E, rhs_tn, ntok, NTs,
                      mk_ev(k_s, False), tag="gemm", name=_tn("gemm"))

            # V token-major [tok, E]
            v_m = pools["big"].tile([128, CH * E], BF16, tag="v_m", name=_tn("v_m"))
            has_tail = L > 128
            v_t = None
            if has_tail:
                v_t = pools["big"].tile([32, CH * E], BF16,
                                        tag="v_t", name=_tn("v_t"))
            Lm = min(128, L)
            for s in range(ns):
                ps = pools["psum"].tile([128, E], F32, tag="gemm", name=_tn("gemm"))
                for kt in range(FT):
                    nc.tensor.matmul(
                        ps[0:Lm, :],
                        tok_bf[:, kt * ntok + s * L: kt * ntok + s * L + Lm],
                        self.WvT[:, kt * E:(kt + 1) * E],
                        start=(kt == 0), stop=(kt == FT - 1))
                nc.vector.tensor_copy(
                    out=v_m[0:Lm, s * E:(s + 1) * E], in_=ps[0:Lm, :])
            if has_tail:
                tl = L - 128
                for s in range(ns):
                    ps = pools["psum"].tile([128, E], F32, tag="gemm", name=_tn("gemm"))
                    for kt in range(FT):
                        nc.tensor.matmul(
                            ps[0:tl, :],
                            tok_bf[:, kt * ntok + s * L + 128:
                                   kt * ntok + s * L + 128 + tl],
                            self.WvT[:, kt * E:(kt + 1) * E],
                            start=(kt == 0), stop=(kt == FT - 1))
                    nc.scalar.activation(
                        v_t[0:tl, s * E:(s + 1) * E], ps[0:tl, :], AF.Copy)

            o_bf = pools["big"].tile([128, FT * ntok], BF16, tag="o_bf", name=_tn("o_bf"))
            if "attn" in self.ablate:
                nc.vector.tensor_copy(out=o_bf, in_=tn)
            else:
                self.attention(pools, q_s, k_s, v_m, v_t, masks,
                               o_bf, ns, L, ntok, ntokmax)

            # out-proj + residual
            outp_f = pools["big1"].tile([128, FT * ntok], F32, tag="outp_f", name=_tn("outp_f"))
            outp_bf = pools["big"].tile([128, FT * ntok], BF16, tag="outp_bf", name=_tn("outp_bf"))

            def ev_outp(mt, nt0, n, ps):
                nc.vector.tensor_tensor(
                    out=outp_f[:, mt * ntok + nt0: mt * ntok + nt0 + n],
                    in0=ps[:, :n],
                    in1=tok_f[:, mt * ntok + nt0: mt * ntok + nt0 + n],
                    op=OP.add)
                nc.scalar.activation(
                    outp_bf[:, mt * ntok + nt0: mt * ntok + nt0 + n],
                    outp_f[:, mt * ntok + nt0: mt * ntok + nt0 + n], AF.Copy)

            rhs_o = lambda kt, nt0, n: o_bf[:, kt * ntok + nt0: kt * ntok + nt0 + n]
            self.gemm(pools["psum"], self.WoT, E, E, rhs_o, ntok, NTs,
                      ev_outp, tag="gemm", name=_tn("gemm"))

            tn2 = pools["big"].tile([128, FT * ntok], BF16, tag="tn2", name=_tn("tn2"))
            if "ln" in self.ablate:
                nc.vector.tensor_copy(out=tn2, in_=outp_bf)
            else:
                self.ln_chunk_v2(tc, pools, outp_f, self.lnw2, self.lnb2, tn2,
                                 ntok, NTs, also_sq_src=outp_bf)

            ffh = pools["big"].tile([128, 2 * FT * ntok], BF16, tag="ffh", name=_tn("ffh"))

            def ev_ffh(mt, nt0, n, ps):
                nc.vector.tensor_relu(
                    out=ffh[:, mt * ntok + nt0: mt * ntok + nt0 + n],
                    in_=ps[:, :n])

            rhs_tn2 = lambda kt, nt0, n: tn2[:, kt * ntok + nt0: kt * ntok + nt0 + n]
            self.gemm(pools["psum"], self.Wff1T, E, 2 * E, rhs_tn2, ntok, NTs,
                      ev_ffh, tag="gemm", name=_tn("gemm"))

            res2_f = pools["big1"].tile([128, FT * ntok], F32, tag="res2_f", name=_tn("res2_f"))
            res2_bf = pools["big"].tile([128, FT * ntok], BF16, tag="res2_bf", name=_tn("res2_bf"))

            def ev_ffo(mt, nt0, n, ps):
                nc.vector.tensor_tensor(
                    out=res2_f[:, mt * ntok + nt0: mt * ntok + nt0 + n],
                    in0=ps[:, :n],
                    in1=outp_f[:, mt * ntok + nt0: mt * ntok + nt0 + n],
                    op=OP.add)
                nc.scalar.activation(
                    res2_bf[:, mt * ntok + nt0: mt * ntok + nt0 + n],
                    res2_f[:, mt * ntok + nt0: mt * ntok + nt0 + n], AF.Copy)

            rhs_ffh = lambda kt, nt0, n: ffh[:, kt * ntok + nt0: kt * ntok + nt0 + n]
            self.gemm(pools["psum"], self.Wff2T, 2 * E, E, rhs_ffh, ntok, NTs,
                      ev_ffo, tag="gemm", name=_tn("gemm"))

            rhs_r2 = lambda kt, nt0, n: res2_bf[:, kt * ntok + nt0: kt * ntok + nt0 + n]
            self.gemm(pools["psum"], self.WoutT, E, cfg.C, rhs_r2, ntok, NTs,
                      lambda mt, nt0, n, ps: evict_y_fn(c0, nt0, n, ps),
                      tag="gemm", name=_tn("gemm"))

    # -- conv 3x3 over (h,w) for one b-half --------------------------------
    def conv_half(self, pools, cin, evict_fn, rows=None):
        """cin: sbuf [128, A*6*WPAD*A(v)] bf16 window tile (w-padded).
        out pixels (u, hl, w, v); evict_fn(u, hp, n, psum) with pixel tiles
        [128, 2*W*A(v)].  rows: list of (hp, nh) output-row groups; default
        covers all HL rows in pairs."""
        nc, cfg = self.nc, self.cfg
        A, W = cfg.A, cfg.W
        WP = W + 2
        if rows is None:
            rows = [(hp, min(2, cfg.HL - hp)) for hp in range(0, cfg.HL, 2)]
        cv = cin.rearrange("p (u h w v) -> p u h w v", u=A, h=6, w=WP)
        for u in range(A):
            for hp, nh in rows:
                ps = pools["psum"].tile([128, 2 * W * A], F32, tag="gemm", name=_tn("gemm"))
                first = True
                for dy in range(3):
                    for dx in range(3):
                        rhs = cv[:, u, hp + dy:hp + dy + nh, dx:dx + W, :]
                        nc.tensor.matmul(
                            ps[:, :nh * W * A],
                            self.taps[:, (dy * 3 + dx) * 128:(dy * 3 + dx + 1) * 128],
                            rhs,
                            start=first, stop=(dy == 2 and dx == 2))
                        first = False
                evict_fn(u, hp, nh, ps)

    # -- full graph --------------------------------------------------------
    def build(self):
        cfg = self.cfg
        nc = self.nc
        A, C, E, B, H, W = cfg.A, cfg.C, cfg.E, cfg.B, cfg.H, cfg.W
        L1, L2 = cfg.L1, cfg.L2
        FT = E // 128
        WL, HL, HP = cfg.WL, cfg.HL, cfg.HPAD
        NC = cfg.ncores
        WPAD = W + 2
        chunk1_cols = A * HP * WL * A          # y_perm cols (u hpad wl v)
        chunk2_cols = A * HL * W * A           # y2 cols (u hl w v)

        # ---- dram params
        P = {}
        def par(name, shape, dt):
            P[name] = nc.dram_tensor(name, shape, dt, kind="ExternalInput").ap()
        par("xtok1", [C, B * cfg.SEQ1 * L1], BF16)
        par("res1", [C, B * chunk2_cols], F32)
        par("WinT", [C, E], BF16)
        for n in ("WqT", "WkT", "WvT", "WoT"):
            par(n, [E, E], BF16)
        par("Wff1T", [E, 2 * E], BF16)
        par("Wff2T", [2 * E, E], BF16)
        par("WoutT", [E, C], BF16)
        par("tapT", [3, 3, C, C], BF16)
        par("mask3_1", [128, 480], BF16)
        par("maskB_1", [128, 320], BF16)
        par("maskC_1", [128, 480], BF16)
        par("mask3_2", [128, 480], BF16)
        par("maskB_2", [128, 320], BF16)
        par("maskC_2", [128, 480], BF16)
        par("ident", [128, 128], BF16)
        for n in ("lnw1", "lnb1", "lnw2", "lnb2"):
            par(n, [128, FT], F32)
        self.params = P
        out_ext = nc.dram_tensor("out", [C, B * chunk2_cols], F32,
                                 kind="ExternalOutput")

        with tile.TileContext(nc) as tc:
            import contextlib
            with contextlib.ExitStack() as ctx:
                pools = {}
                pools["const"] = ctx.enter_context(
                    tc.tile_pool(name="const", bufs=1))
                pools["big"] = ctx.enter_context(
                    tc.tile_pool(name="big", bufs=2))
                pools["big1"] = ctx.enter_context(
                    tc.tile_pool(name="big1", bufs=1))
                pools["qk"] = ctx.enter_context(
                    tc.tile_pool(name="qk", bufs=1))
                pools["scratch"] = ctx.enter_context(
                    tc.tile_pool(name="scratch", bufs=1))
                pools["io"] = ctx.enter_context(
                    tc.tile_pool(name="io", bufs=1))
                pools["psum"] = ctx.enter_context(
                    tc.tile_pool(name="psum", bufs=2, space="PSUM"))
                pools["sc"] = ctx.enter_context(
                    tc.tile_pool(name="sc", bufs=2, space="PSUM"))
                pools["exp"] = ctx.enter_context(
                    tc.tile_pool(name="exp", bufs=3))
                pools["dram"] = ctx.enter_context(
                    tc.tile_pool(name="dram", bufs=1, space="DRAM"))
                self._build_body(tc, pools, out_ext)
        nc.compile()
        return nc

    def _build_body(self, tc, pools, out_ext):
        nc, cfg = self.nc, self.cfg
        A, C, E, B, H, W = cfg.A, cfg.C, cfg.E, cfg.B, cfg.H, cfg.W
        L1, L2 = cfg.L1, cfg.L2
        FT = E // 128
        WL, HL, HP = cfg.WL, cfg.HL, cfg.HPAD
        NC = cfg.ncores
        WPAD = W + 2
        ch1 = A * HP * WL * A
        ch2 = A * HL * W * A
        dram = pools["dram"]

        self.load_weights(tc, pools["const"])
        import os as _os
        _simpid = _os.environ.get("KSIMPID")
        pid = int(_simpid) if _simpid else nc.partition_id()

        x1_chunk = [dram.tile([C, ch1], BF16, tag=f"x1c{b}", name=_tn(f"x1c{b}")) for b in range(B)]
        x1_full = [dram.tile([NC * C, ch1], BF16, addr_space="Shared",
                             tag=f"x1f{b}", name=_tn(f"x1f{b}")) for b in range(B)]
        bnd_cols = 2 * A * W * A
        bnd_snd = [dram.tile([C, bnd_cols], BF16, tag=f"bs{b}", name=_tn(f"bs{b}")) for b in range(B)]
        bnd_all = [dram.tile([NC * C, bnd_cols], BF16, tag=f"ba{b}", name=_tn(f"ba{b}")) for b in range(B)]
        sc2_dram = [dram.tile([C, ch2], F32, tag=f"s2d{b}", name=_tn(f"s2d{b}")) for b in range(B)]

        # ---------------- pass 1 (per b-half)
        sc2_f = []
        y2_sbs = []
        for b in range(B):
            y_perm = pools["io"].tile([C, ch1], BF16, tag="y_perm", name=_tn("y_perm"))
            # zero hpad rows 0 and HP-1:  cols (u, {0,HP-1}, wl, v)
            yv = y_perm.rearrange("p (u h w v) -> p u h w v", u=A, h=HP, w=WL)
            nc.vector.memset(yv[:, :, 0:1, :, :], 0.0)
            nc.vector.memset(yv[:, :, HP - 1:HP, :, :], 0.0)

            def x_src(c0, ntok, b=b):
                t = pools["big"].tile([128, ntok], BF16, tag="x_in", name=_tn("x_in"))
                nc.sync.dma_start(
                    t, self.params["xtok1"][:, b * cfg.SEQ1 * L1 + c0 * L1:
                                            b * cfg.SEQ1 * L1 + c0 * L1 + ntok])
                return t

            def ev_y(c0, nt0, n, ps, y_perm=y_perm):
                # psum [128, n] tokens of seqs starting at s0=c0+nt0//L1
                # seq (v, wl): v = s//WL, wl = s%WL ; token (u, h)
                # y_perm col = u*(HP*WL*A) + (h+1)*(WL*A) + wl*A + v
                s0 = c0 + nt0 // L1
                npair = n // L1
                for i in range(0, npair):
                    s = s0 + i
                    v, wl = s // WL, s % WL
                    dst = y_perm.rearrange(
                        "p (u h w vv) -> p u h w vv", u=A, h=HP, w=WL)
                    nc.vector.tensor_copy(
                        out=dst[:, :, 1:H + 1, wl, v],
                        in_=ps[:, i * L1:(i + 1) * L1].rearrange(
                            "p (u h) -> p u h", h=H))

            self.transformer_half(tc, pools, x_src,
                                  (self.mask3_1, self.maskB_1, self.maskC_1),
                                  L1, cfg.SEQ1, ev_y)
            nc.sync.dma_start(x1_chunk[b][:, :], y_perm)
            nc.gpsimd.collective_compute(
                "AllGather", mybir.AluOpType.bypass,
                ins=[x1_chunk[b].opt()], outs=[x1_full[b].opt()],
                replica_groups=[list(range(NC))])

        # ---------------- conv1 + residual -> sc2 ; then pass 2
        for b in range(B):
            cin = pools["io"].tile([C, A * 6 * WPAD * A], BF16, tag="cin", name=_tn("cin"))
            cinv = cin.rearrange("p (u h w v) -> p u h w v", u=A, h=6, w=WPAD)
            nc.vector.memset(cinv[:, :, :, 0:1, :], 0.0)
            nc.vector.memset(cinv[:, :, :, WPAD - 1:WPAD, :], 0.0)
            xf = x1_full[b].rearrange("(wc c) (u h w v) -> wc c u h w v",
                                      c=C, u=A, h=HP, w=WL)
            for wc in range(NC):
                nc.sync.dma_start(
                    cinv[:, :, :, 1 + wc * WL:1 + (wc + 1) * WL, :],
                    xf[wc, :, :, ds(pid * HL, 6), :, :])
            scf = pools["io"].tile([C, ch2], BF16, tag=f"sc2f{b}", name=_tn(f"sc2f{b}"))
            sc2_f.append(scf)

            res_cache = {}

            def ev_c1b(u, hp, nh, ps, b=b, scf=scf, res_cache=res_cache):
                col = u * (HL * W * A) + hp * (W * A)
                n = nh * W * A
                if u not in res_cache:
                    rt = pools["big"].tile([C, HL * W * A], F32, tag="res_u", name=_tn("res_u"))
                    ub = u * (HL * W * A)
                    nc.sync.dma_start(
                        rt, self.params["res1"][:, b * ch2 + ub:
                                                b * ch2 + ub + HL * W * A])
                    res_cache[u] = rt
                sct = pools["big"].tile([C, 2 * W * A], F32, tag="out_t", name=_tn("sc_t"))
                nc.vector.tensor_tensor(
                    out=sct[:, :n], in0=ps[:, :n],
                    in1=res_cache[u][:, hp * W * A: hp * W * A + n], op=OP.add)
                nc.scalar.activation(scf[:, col:col + n], sct[:, :n], AF.Copy)
                nc.sync.dma_start(sc2_dram[b][:, col:col + n], sct[:, :n])

            self.conv_half(pools, cin, ev_c1b)

            # ---- pass 2 on sc2 (boundary hl rows first so the halo
            # exchange overlaps interior compute)
            y2_sb = pools["io"].tile([C, ch2], BF16, tag=f"y2sb{b}", name=_tn(f"y2sb{b}"))
            y2_sbs.append(y2_sb)
            perm2 = ([u * HL for u in range(A)]
                     + [u * HL + HL - 1 for u in range(A)]
                     + [u * HL + hl for hl in range(1, HL - 1)
                        for u in range(A)])

            def x_src2(c0, ntok, scf=scf):
                t = pools["big"].tile([128, ntok], BF16, tag="x2g", name=_tn("x2g"))
                for i in range(ntok // L2):
                    st = perm2[c0 + i]
                    nc.vector.tensor_copy(
                        out=t[:, i * L2:(i + 1) * L2],
                        in_=scf[:, st * L2:(st + 1) * L2])
                return t

            def ev_y2(c0, nt0, n, ps, y2_sb=y2_sb):
                for i in range(n // L2):
                    st = perm2[c0 + nt0 // L2 + i]
                    nc.vector.tensor_copy(
                        out=y2_sb[:, st * L2:(st + 1) * L2],
                        in_=ps[:, i * L2:(i + 1) * L2])

            self.transformer_half(tc, pools, x_src2,
                                  (self.mask3_2, self.maskB_2, self.maskC_2),
                                  L2, cfg.SEQ2, ev_y2)
            ysb_v = y2_sb.rearrange("p (u hl wv) -> p u hl wv", u=A, hl=HL)
            nc.sync.dma_start(
                bnd_snd[b][:, 0:A * W * A].rearrange(
                    "p (u wv) -> p u wv", u=A),
                ysb_v[:, :, 0, :])
            nc.sync.dma_start(
                bnd_snd[b][:, A * W * A:2 * A * W * A].rearrange(
                    "p (u wv) -> p u wv", u=A),
                ysb_v[:, :, HL - 1, :])
            nc.gpsimd.collective_compute(
                "AllGather", mybir.AluOpType.bypass,
                ins=[bnd_snd[b].opt()],
                outs=[bnd_all[b].opt()],
                replica_groups=[list(range(NC))])

        # ---------------- conv2 + sc2 -> out
        for b in range(B):
            cin = pools["io"].tile([C, A * 6 * WPAD * A], BF16, tag="cin", name=_tn("cin"))
            cinv = cin.rearrange("p (u h w v) -> p u h w v", u=A, h=6, w=WPAD)
            nc.vector.memset(cinv[:, :, :, 0:1, :], 0.0)
            nc.vector.memset(cinv[:, :, :, WPAD - 1:WPAD, :], 0.0)
            ysv = y2_sbs[b].rearrange("p (u hl w v) -> p u hl w v",
                                      u=A, hl=HL, w=W)
            for u in range(A):
                nc.sync.dma_start(cinv[:, u, 1:1 + HL, 1:1 + W, :], ysv[:, u])
            # interior output rows need only local y2 — run before halo
            def ev_c2(u, hp, nh, ps, b=b):
                col = u * (HL * W * A) + hp * (W * A)
                n = nh * W * A
                rt = pools["big"].tile([C, 2 * W * A], F32, tag="res_u", name=_tn("res2_u"))
                nc.sync.dma_start(rt[:, :n], sc2_dram[b][:, col: col + n])
                ot = pools["big"].tile([128, 2 * W * A], F32, tag="out_t", name=_tn("out_t"))
                nc.vector.tensor_tensor(
                    out=ot[:, :n], in0=ps[:, :n], in1=rt[:, :n], op=OP.add)
                nc.sync.dma_start(
                    out_ext.ap()[:, b * ch2 + col: b * ch2 + col + n],
                    ot[:, :n])

            self.conv_half(pools, cin, ev_c2, rows=[(1, 2)])
            blkA = (pid + NC - 1) % NC
            blkC = (pid + 1) % NC
            bav_t = bnd_all[b][ds(blkA * C, C), :].rearrange(
                "c (e u w v) -> c e u w v", e=2, u=A, w=W)
            nc.sync.dma_start(cinv[:, :, 0, 1:1 + W, :], bav_t[:, 1])
            bav_b = bnd_all[b][ds(blkC * C, C), :].rearrange(
                "c (e u w v) -> c e u w v", e=2, u=A, w=W)
            nc.sync.dma_start(cinv[:, :, 5, 1:1 + W, :], bav_b[:, 0])
            if isinstance(pid, int):
                if pid < 1:
                    nc.vector.memset(cinv[:, :, 0:1, :, :], 0.0)
                if pid > NC - 2:
                    nc.vector.memset(cinv[:, :, 5:6, :, :], 0.0)
            else:
                with tc.If(pid < 1):
                    nc.vector.memset(cinv[:, :, 0:1, :, :], 0.0)
                with tc.If(pid > NC - 2):
                    nc.vector.memset(cinv[:, :, 5:6, :, :], 0.0)
            self.conv_half(pools, cin, ev_c2, rows=[(0, 1), (HL - 1, 1)])


# ---------------------------------------------------------------- entry point

_CACHE = {}


def kernel(**inputs):
    import numpy as np
    from concourse.bass_utils import run_bass_kernel_spmd
    import os as _os
    cfg = Cfg()
    if "nc" not in _CACHE:
        abl = tuple(x for x in _os.environ.get("KABL", "").split(",") if x)
        ker = Ker(cfg, ablate=abl)
        _CACHE["nc"] = ker.build()
    nc = _CACHE["nc"]
    in_maps = host_prep(cfg, inputs)
    res = run_bass_kernel_spmd(nc, in_maps, core_ids=list(range(cfg.ncores)),
                               trace=False)
    outs = [res.results[i]["out"] for i in range(cfg.ncores)]
    return host_unshard(cfg, outs).astype(np.float32)



# revision 67
# speedup vs baseline: 1.3964x; 1.0202x over previous
"""Bass kernel for nn_AltFilter: dual-pass windowed transformer + conv.

Sharding: pass-1 data-parallel over w (8 chunks of W/8), pass-2 over h.
Between passes: AllGather of pass-1 output (bf16), conv read from gathered.

Layouts (per core, per b-half):
  xtok1   [C, (v, wl, u, h)]        pass-1 token input (host prepped, bf16)
  y_perm  [C, (u, hpad34, wl, v)]   pass-1 output staged for AG (bf16)
  x1_full [8*C, (u, hpad34, wl, v)] AG result, wc-major blocks
  conv1in [C, (u, 6, Wpad, v)]      conv window (bf16) ; w = 4*wc+wl
  sc2     [C, (u, hl, w, v)]        conv1+res (f32/bf16) == pass-2 tokens
  y2      [C, (u, hl, w, v)]        pass-2 out chunk (bf16) -> AG (10-block padded)
  out     [C, (b, u, hl, w, v)]     final (f32)
"""

import numpy as np
from dataclasses import dataclass
from concourse import bass, bacc, tile, mybir
from concourse.bass import ds

BF16 = mybir.dt.bfloat16
F32 = mybir.dt.float32
AF = mybir.ActivationFunctionType
OP = mybir.AluOpType


@dataclass
class Cfg:
    A: int = 5
    C: int = 128
    E: int = 256
    NH: int = 8
    B: int = 2
    H: int = 32
    W: int = 32
    ncores: int = 8
    ch_seqs: int = 5      # sequences per processing chunk
    win: int = 5          # attention half-window (KW//2)

    @property
    def HD(self):
        return self.E // self.NH

    @property
    def WL(self):
        return self.W // self.ncores

    @property
    def HL(self):
        return self.H // self.ncores

    @property
    def L1(self):
        return self.A * self.H       # pass-1 tokens per seq (u, h)

    @property
    def L2(self):
        return self.W * self.A       # pass-2 tokens per seq (w, v)

    @property
    def SEQ1(self):
        return self.A * self.WL      # per-b seqs pass 1 (v, wl)

    @property
    def SEQ2(self):
        return self.A * self.HL      # per-b seqs pass 2 (u, hl)

    @property
    def HPAD(self):
        return self.H + 2



_tname_ctr = [0]


def _tn(tag):
    _tname_ctr[0] += 1
    return f"{tag}_{_tname_ctr[0]}"

def mchunks(L):
    out = []
    o = 0
    while o < L:
        sz = min(128, L - o)
        out.append((o, sz))
        o += sz
    return out


# ---------------------------------------------------------------- host prep

def host_prep(cfg, inputs):
    """Build per-core in_maps from full inputs. Returns list of dicts."""
    import ml_dtypes
    bf = ml_dtypes.bfloat16
    A, C, E, B, H, W = cfg.A, cfg.C, cfg.E, cfg.B, cfg.H, cfg.W
    NC = cfg.ncores
    WL, HL = cfg.WL, cfg.HL

    buf = np.asarray(inputs["buffer"], np.float32)
    buf6 = buf.reshape(B, C, A, A, H, W)                    # b c u v h w

    # pass-1 tokens: [c, b, v, w, u, h] -> core k takes w slice
    xt = np.ascontiguousarray(buf6.transpose(1, 0, 3, 5, 2, 4))  # c b v w u h
    # conv1 residual: [c, b, u, h, w, v] -> core k takes h slice
    rs = np.ascontiguousarray(buf6.transpose(1, 0, 2, 4, 5, 3))  # c b u h w v

    ip = np.asarray(inputs["in_proj"], np.float32)
    sc = 1.0 / np.sqrt(cfg.HD)
    WqT = (ip[0:E].T * sc).astype(bf)
    WkT = ip[E:2 * E].T.astype(bf)
    WvT = ip[2 * E:3 * E].T.astype(bf)
    WinT = np.asarray(inputs["Win"], np.float32).T.astype(bf)       # (C, E)
    WoT = np.asarray(inputs["attn_out_w"], np.float32).T.astype(bf)  # (E, E)
    Wff1T = np.asarray(inputs["ff1"], np.float32).T.astype(bf)       # (E, 2E)
    Wff2T = np.asarray(inputs["ff2"], np.float32).T.astype(bf)       # (2E, E)
    WoutT = np.asarray(inputs["Wout"], np.float32).T.astype(bf)      # (E, C)
    cw = np.asarray(inputs["conv_w"], np.float32)[:, :, 0]           # (O,I,3,3)
    tapT = np.ascontiguousarray(cw.transpose(2, 3, 1, 0)).astype(bf)  # ky kx I O

    def band_mask(L, blk, n_outer, outer_major):
        # tokens: outer_major=True -> l = outer*blk_count... build via indices
        # pass1: l = u*H + h, band over h ; pass2: l = w*A + v, band over w
        l = np.arange(L)
        if outer_major:
            pos = l % blk          # h = l % H  (u-major, h inner)
        else:
            pos = l // n_outer     # w = l // A (w-major, v inner)
        d = np.abs(pos[:, None] - pos[None, :])
        m = np.where(d <= cfg.win, 0.0, -30000.0).astype(np.float32)
        return m.astype(bf)

    mask1 = band_mask(cfg.L1, cfg.H, cfg.A, True)
    mask2 = band_mask(cfg.L2, cfg.W, cfg.A, False)

    # attention-psum mask-init tiles: scA [128, 3L] = 3 head replicas of
    # mask rows 0:128; scB [128, 2L] = head-3 main + tail (rows 0:32);
    # scC [128, 3L] = 3 tail replicas at rows 0:32 (rest zero).
    def mk_masks(mask):
        m_main = np.asarray(mask[0:128, :], np.float32)   # [128, 160]
        m_tail = np.asarray(mask[128:160, :], np.float32)  # [32, 160]
        m3 = np.concatenate([m_main] * 3, axis=1)          # [128, 480]
        tail_pad = np.concatenate(
            [m_tail, np.full((96, m_tail.shape[1]), -30000.0, np.float32)], 0)
        mB = np.concatenate([m_main, tail_pad], axis=1)    # [128, 320]
        tail_z = np.concatenate(
            [m_tail, np.zeros((96, m_tail.shape[1]), np.float32)], 0)
        mC = np.concatenate([tail_z] * 3, axis=1)          # [128, 480]
        return (np.ascontiguousarray(m3).astype(bf),
                np.ascontiguousarray(mB).astype(bf),
                np.ascontiguousarray(mC).astype(bf))

    mask3_1, maskB_1, maskC_1 = mk_masks(mask1)
    mask3_2, maskB_2, maskC_2 = mk_masks(mask2)
    ident = np.eye(128, dtype=np.float32).astype(bf)

    lnw1 = np.asarray(inputs["ln_w"], np.float32).reshape(E // 128, 128).T.copy()
    lnb1 = np.asarray(inputs["ln_b"], np.float32).reshape(E // 128, 128).T.copy()
    lnw2 = np.asarray(inputs["ffn_ln_w"], np.float32).reshape(E // 128, 128).T.copy()
    lnb2 = np.asarray(inputs["ffn_ln_b"], np.float32).reshape(E // 128, 128).T.copy()

    shared = dict(WinT=WinT, WqT=WqT, WkT=WkT, WvT=WvT, WoT=WoT,
                  Wff1T=Wff1T, Wff2T=Wff2T, WoutT=WoutT, tapT=tapT,
                  mask3_1=mask3_1, maskB_1=maskB_1, maskC_1=maskC_1,
                  mask3_2=mask3_2, maskB_2=maskB_2, maskC_2=maskC_2,
                  ident=ident,
                  lnw1=lnw1, lnb1=lnb1, lnw2=lnw2, lnb2=lnb2)

    maps = []
    for k in range(NC):
        xk = xt[:, :, :, k * WL:(k + 1) * WL]   # c b v wl u h
        xk = np.ascontiguousarray(xk).reshape(C, -1).astype(bf)
        rk = rs[:, :, :, k * HL:(k + 1) * HL]   # c b u hl w v
        rk = np.ascontiguousarray(rk).reshape(C, -1).astype(np.float32)
        m = dict(shared)
        m["xtok1"] = xk
        m["res1"] = rk
        maps.append(m)
    return maps


def host_unshard(cfg, outs):
    """outs: list of per-core 'out' arrays [C, B*A*HL*W*A] -> full output."""
    A, C, B, H, W = cfg.A, cfg.C, cfg.B, cfg.H, cfg.W
    o = np.stack(outs)  # j c b u hl w v
    o = o.reshape(cfg.ncores, C, B, A, cfg.HL, W, A)
    o = o.transpose(2, 1, 3, 6, 0, 4, 5)  # b c u v j hl w
    return np.ascontiguousarray(o).reshape(B, C, A * A, H, W)


# ---------------------------------------------------------------- builder

class Ker:
    """Holds nc + pools + weight tiles while building."""

    _tables_pinned = False

    @classmethod
    def _pin_act_tables(cls):
        import os as _os
        if cls._tables_pinned or _os.environ.get("KTAB", "pin") != "pin":
            return
        cls._tables_pinned = True
        import concourse.bacc as _baccmod
        from concourse.hw_specs import get_activation_tables as _gat

        def pinned(arch):
            tabs = _gat(arch)
            keep = "natural_log_exp_and_others"
            mine = {AF.Exp, AF.Ln, AF.Copy, AF.Identity}
            out = {}
            for name, s in tabs.items():
                out[name] = s if name == keep else (s - mine)
            return out

        _baccmod.get_activation_tables = pinned

    def __init__(self, cfg, stage=4, ablate=()):
        self._pin_act_tables()
        self.cfg = cfg
        self.stage = stage  # 1=pass1, 2=+conv1, 3=+pass2, 4=full
        self.ablate = set(ablate)
        self.nc = bacc.Bacc("TRN2", target_bir_lowering=False, debug=False,
                            num_devices=cfg.ncores)

    # -- weights to sbuf ---------------------------------------------------
    def load_weights(self, tc, pool):
        nc, cfg = self.nc, self.cfg
        E = cfg.E

        def wtile(name, K, M):
            p = self.params[name]
            t = pool.tile([128, (K // 128) * M], BF16, tag=name)
            for kt in range(K // 128):
                nc.sync.dma_start(t[:, kt * M:(kt + 1) * M],
                                  p[kt * 128:(kt + 1) * 128, :])
            return t

        self.WinT = wtile("WinT", cfg.C, E)
        self.WqT = wtile("WqT", E, E)
        self.WkT = wtile("WkT", E, E)
        self.WvT = wtile("WvT", E, E)
        self.WoT = wtile("WoT", E, E)
        self.Wff1T = wtile("Wff1T", E, 2 * E)
        self.Wff2T = wtile("Wff2T", 2 * E, E)
        self.WoutT = wtile("WoutT", E, cfg.C)

        tap = self.params["tapT"]  # (3,3,I,O)
        self.taps = pool.tile([128, 9 * 128], BF16, tag="taps", name=_tn("taps"))
        for ky in range(3):
            for kx in range(3):
                i = ky * 3 + kx
                nc.sync.dma_start(self.taps[:, i * 128:(i + 1) * 128],
                                  tap[ky, kx])


        def lnt(name):
            t = pool.tile([128, E // 128], F32, tag=name)
            nc.sync.dma_start(t, self.params[name])
            return t

        self.lnw1, self.lnb1 = lnt("lnw1"), lnt("lnb1")
        self.lnw2, self.lnb2 = lnt("lnw2"), lnt("lnb2")

        self.ones = pool.tile([128, 128], BF16, tag="ones", name=_tn("ones"))
        nc.vector.memset(self.ones[:], 1.0 / E)

        def bftile(name, shape):
            t = pool.tile(shape, BF16, tag=name)
            nc.sync.dma_start(t, self.params[name])
            return t

        self.mask3_1 = bftile("mask3_1", [128, 480])
        self.maskB_1 = bftile("maskB_1", [128, 320])
        self.maskC_1 = bftile("maskC_1", [128, 480])
        self.mask3_2 = bftile("mask3_2", [128, 480])
        self.maskB_2 = bftile("maskB_2", [128, 320])
        self.maskC_2 = bftile("maskC_2", [128, 480])
        self.ident = bftile("ident", [128, 128])
        self.ones_att = pool.tile([128, 32], BF16, tag="ones_att",
                                  name=_tn("ones_att"))
        nc.vector.memset(self.ones_att[:], 1.0)


    # -- generic GEMM over one token chunk --------------------------------
    def gemm(self, psum_pool, wt, K, M, rhs_fn, ntok, nt_sz, evict_fn,
             tag="gemm", name=_tn("gemm")):
        """out[mt] = sum_kt  wt[kt,mt].T @ rhs(kt, nt) ; evict per (mt, nt)."""
        nc = self.nc
        KT, MT = K // 128, M // 128
        for mt in range(MT):
            for nt0 in range(0, ntok, nt_sz):
                n = min(nt_sz, ntok - nt0)
                ps = psum_pool.tile([128, nt_sz], F32, tag=tag)
                for kt in range(KT):
                    nc.tensor.matmul(
                        ps[:, :n],
                        wt[:, kt * M + mt * 128: kt * M + mt * 128 + 128],
                        rhs_fn(kt, nt0, n),
                        start=(kt == 0), stop=(kt == KT - 1))
                evict_fn(mt, nt0, n, ps)

    # -- layernorm over one chunk -----------------------------------------
    def ln_chunk_v2(self, tc, pools, x_f32, lnw, lnb, out_bf, ntok, nt_sz,
                    also_sq_src=None):
        """Feature-major LN. Stats are computed via all-ones matmuls whose
        M=128 stationary replicates sum across all partitions, so no
        partition-broadcast is ever needed."""
        nc, cfg = self.nc, self.cfg
        FT = cfg.E // 128
        x_bf = also_sq_src
        sq = pools["scratch"].tile([128, FT * ntok], BF16, tag="lnsq", name=_tn("lnsq"))
        for ft in range(FT):
            nc.vector.tensor_tensor(
                out=sq[:, ft * ntok:(ft + 1) * ntok],
                in0=x_bf[:, ft * ntok:(ft + 1) * ntok],
                in1=x_bf[:, ft * ntok:(ft + 1) * ntok], op=OP.mult)
        mean = pools["scratch"].tile([128, ntok], BF16, tag="lnmean", name=_tn("lnmean"))
        rstd = pools["scratch"].tile([128, ntok], F32, tag="lnrstd", name=_tn("lnrstd"))
        rstd_bf = pools["scratch"].tile([128, ntok], BF16, tag="lnrstdb", name=_tn("lnrstdb"))
        for nt0 in range(0, ntok, nt_sz):
            n = min(nt_sz, ntok - nt0)
            # own 1-bank pool: s and q rotate through the same bank
            # sequentially, keeping the gemm pool free for other chunks.
            ps_s = pools["stat"].tile([128, nt_sz], F32, tag="lnstat", name=_tn("lnstat"))
            ps_q = pools["stat"].tile([128, nt_sz], F32, tag="lnstat", name=_tn("lnstat"))
            for kt in range(FT):
                nc.tensor.matmul(ps_s[:, :n], self.ones,
                                 x_bf[:, kt * ntok + nt0: kt * ntok + nt0 + n],
                                 start=(kt == 0), stop=(kt == FT - 1))
            for kt in range(FT):
                nc.tensor.matmul(ps_q[:, :n], self.ones,
                                 sq[:, kt * ntok + nt0: kt * ntok + nt0 + n],
                                 start=(kt == 0), stop=(kt == FT - 1))
            nc.vector.tensor_copy(out=mean[:, nt0:nt0 + n], in_=ps_s[:, :n])
            nc.vector.tensor_copy(out=rstd[:, nt0:nt0 + n], in_=ps_q[:, :n])
        # rstd = (E[x^2] + eps - mean^2) ** -0.5
        msq = pools["scratch"].tile([128, ntok], BF16, tag="lnmsq", name=_tn("lnmsq"))
        nc.vector.tensor_tensor(out=msq[:], in0=mean[:], in1=mean[:],
                                op=OP.mult)
        nc.vector.scalar_tensor_tensor(
            out=rstd[:], in0=rstd[:], scalar=1e-5, in1=msq[:],
            op0=OP.add, op1=OP.subtract)
        # rstd = exp(-0.5*ln(var)); Ln+Exp share one ACT table so no
        # table reloads against the attention exps. The Exp writes bf16
        # directly so the apply TTs below run in the 2x DVE mode.
        if "rsqrt" not in self.ablate:
            nc.scalar.activation(rstd[:], rstd[:], AF.Ln)
            nc.scalar.activation(rstd_bf[:], rstd[:], AF.Exp, scale=-0.5)
        else:
            nc.vector.tensor_copy(out=rstd_bf[:], in_=rstd[:])
        t1 = pools["scratch"].tile([128, nt_sz], BF16, tag="lnt1", name=_tn("lnt1"))
        for ft in range(FT):
            for nt0 in range(0, ntok, nt_sz):
                n = min(nt_sz, ntok - nt0)
                nc.vector.tensor_tensor(
                    out=t1[:, :n],
                    in0=x_bf[:, ft * ntok + nt0: ft * ntok + nt0 + n],
                    in1=mean[:, nt0:nt0 + n], op=OP.subtract)
                nc.vector.tensor_tensor(
                    out=t1[:, :n], in0=t1[:, :n],
                    in1=rstd_bf[:, nt0:nt0 + n], op=OP.mult)
                if "lnapply" in self.ablate:
                    nc.scalar.activation(
                        out_bf[:, ft * ntok + nt0: ft * ntok + nt0 + n],
                        t1[:, :n], AF.Copy)
                else:
                    nc.scalar.activation(
                        out_bf[:, ft * ntok + nt0: ft * ntok + nt0 + n],
                        t1[:, :n], AF.Identity, bias=lnb[:, ft:ft + 1],
                        scale=lnw[:, ft:ft + 1])

    # -- attention for one chunk of sequences ------------------------------
    def attention(self, pools, q_s, k_s, v_m, v_t, masks, o_bf, nseq, L,
                  ntok, ntokmax):
        """Per (seq, head-quad): scores for 4 heads into two psum banks
        (scA = heads 0-2 main [128, 3L]; scB = head-3 main [128, L] + the
        4 stacked [32, L] m-tails), mask added via one identity matmul from
        a precomputed SBUF tile, one Exp per bank, then col-tiled AV into an
        O|D bank: rows 32c = head c, cols 0:L = o, L:2L = denominator.
        One reciprocal + one multiply normalize all 4 heads."""
        nc, cfg = self.nc, self.cfg
        E = cfg.E
        mask3_t, maskB_t, maskC_t = masks
        for s in range(nseq):
            for qd in range(2):
                scA = pools["sc"].tile([128, 3 * L], F32, tag="scA", name=_tn("scA"))
                scB = pools["sc"].tile([128, 2 * L], F32, tag="scB", name=_tn("scB"))
                scC = pools["sc"].tile([32, 3 * L], F32, tag="scC",
                                       name=_tn("scC"), bufs=1)
                nc.tensor.matmul(scA, self.ident, mask3_t, start=True,
                                 stop=False, skip_group_check=True)
                nc.tensor.matmul(scB, self.ident, maskB_t, start=True,
                                 stop=False, skip_group_check=True)
                nc.tensor.matmul(scC, self.ident[:, 0:32], maskC_t,
                                 start=True, stop=False,
                                 skip_group_check=True)
                base = qd * ntokmax + s * L
                for c in range(4):
                    ks = k_s[4 * qd + c]
                    qs = q_s[4 * qd + c][0:32, s * L: (s + 1) * L]
                    out_main = (scA[:, L * c:L * (c + 1)] if c < 3
                                else scB[:, 0:L])
                    nc.tensor.matmul(out_main,
                                     ks[0:32, s * L: s * L + 128], qs,
                                     start=False, stop=True,
                                     skip_group_check=True)
                    tdst = (scC[0:32, L * c:L * (c + 1)] if c < 3
                            else scB[0:32, L:2 * L])
                    nc.tensor.matmul(tdst,
                                     ks[0:32, s * L + 128: (s + 1) * L], qs,
                                     start=False, stop=True,
                                     skip_group_check=True)
                exA = pools["exp"].tile([128, 3 * L], BF16, tag="exA", name=_tn("exA"))
                nc.scalar.activation(exA, scA, AF.Exp)
                exB = pools["exp"].tile([128, 2 * L], BF16, tag="exB", name=_tn("exB"))
                nc.scalar.activation(exB, scB, AF.Exp)
                exC = pools["exp"].tile([32, 3 * L], BF16, tag="exC", name=_tn("exC"))
                nc.scalar.activation(exC, scC, AF.Exp)
                # AV + denominators reuse scB's bank: its scores are dead
                # once exB is taken, so no extra PSUM bank is needed.
                for c in range(4):
                    r0 = 32 * c
                    hg = 4 * qd + c
                    exm = exA[:, L * c:L * (c + 1)] if c < 3 else exB[:, 0:L]
                    ext = (exC[0:32, L * c:L * (c + 1)] if c < 3
                           else exB[0:32, L:2 * L])
                    vm = v_m[:, s * E + 32 * hg: s * E + 32 * hg + 32]
                    vt = v_t[0:32, s * E + 32 * hg: s * E + 32 * hg + 32]
                    nc.tensor.matmul(scB[r0:r0 + 32, 0:L], vm, exm,
                                     start=True, stop=False,
                                     tile_position=(0, r0))
                    nc.tensor.matmul(scB[r0:r0 + 32, 0:L], vt, ext,
                                     start=False, stop=True,
                                     tile_position=(0, r0))
                    nc.tensor.matmul(scB[r0:r0 + 32, L:2 * L],
                                     self.ones_att[0:128, 0:32], exm,
                                     start=True, stop=False,
                                     tile_position=(0, r0))
                    nc.tensor.matmul(scB[r0:r0 + 32, L:2 * L],
                                     self.ones_att[0:32, 0:32], ext,
                                     start=False, stop=True,
                                     tile_position=(0, r0))
                rec = pools["scratch"].tile([128, L], F32, tag="rec", name=_tn("rec"))
                nc.vector.reciprocal_approx_fast(rec, scB[:, L:2 * L])
                nc.vector.tensor_tensor(
                    out=o_bf[:, qd * ntok + s * L: qd * ntok + (s + 1) * L],
                    in0=scB[:, 0:L], in1=rec, op=OP.mult)

    # -- one transformer pass over one b-half ------------------------------
    def transformer_half(self, tc, pools, x_src_fn, masks, L, nseq_b,
                         evict_y_fn):
        """x_src_fn(c0, ntok) -> bf16 [128, ntok] input token tile (Win rhs).
        evict_y_fn(s_global_pair_index, nt0, n, psum) writes final y."""
        nc, cfg = self.nc, self.cfg
        E = cfg.E
        FT = E // 128
        CH = cfg.ch_seqs
        NTs = 3 * L            # token tile = 3 seqs (psum [128,480] f32 fits a bank)
        ntokmax = CH * L
        # q_s/k_s: one 32-row tile per head (base partition 0), cols = tokens.
        # Avoids row-tiled matmuls (broken on HW); K=32 score MMs all run at
        # array rows 0:32.
        q_s = [pools["qk"].tile([32, ntokmax], BF16, tag=f"q_s{h}",
                                name=_tn(f"q_s{h}")) for h in range(cfg.NH)]
        k_s = [pools["qk"].tile([32, ntokmax], BF16, tag=f"k_s{h}",
                                name=_tn(f"k_s{h}")) for h in range(cfg.NH)]
        for c0 in range(0, nseq_b, CH):
            ns = min(CH, nseq_b - c0)
            ntok = ns * L
            x_bf = x_src_fn(c0, ntok)
            tok_f = pools["big1"].tile([128, FT * ntok], F32, tag="tok_f", name=_tn("tok_f"))
            tok_bf = pools["big"].tile([128, FT * ntok], BF16, tag="tok_bf", name=_tn("tok_bf"))

            def ev_tok(mt, nt0, n, ps):
                nc.vector.tensor_copy(
                    out=tok_f[:, mt * ntok + nt0: mt * ntok + nt0 + n],
                    in_=ps[:, :n])
                nc.scalar.activation(
                    tok_bf[:, mt * ntok + nt0: mt * ntok + nt0 + n],
                    ps[:, :n], AF.Copy)

            self.gemm(pools["psum"], self.WinT, cfg.C, E,
                      lambda kt, nt0, n: x_bf[:, nt0:nt0 + n],
                      ntok, NTs, ev_tok, tag="gemm", name=_tn("gemm"))

            tn = pools["big"].tile([128, FT * ntok], BF16, tag="tn", name=_tn("tn"))
            if "ln" in self.ablate:
                nc.vector.tensor_copy(out=tn, in_=tok_bf)
            else:
                self.ln_chunk_v2(tc, pools, tok_f, self.lnw1, self.lnb1, tn,
                                 ntok, NTs, also_sq_src=tok_bf)

            def mk_ev(dst, flip):
                # split the 4 head copies across ACT and DVE so the psum
                # frees in ~half the time (evicts gate the Q/K gemm stream)
                def ev(mt, nt0, n, ps):
                    for c in range(4):
                        d = dst[4 * mt + c][0:32, nt0: nt0 + n]
                        if (c % 2 == 0) != flip:
                            nc.scalar.activation(
                                d, ps[32 * c:32 * c + 32, :n], AF.Copy)
                        else:
                            nc.vector.tensor_copy(
                                out=d, in_=ps[32 * c:32 * c + 32, :n])
                return ev

            rhs_tn = lambda kt, nt0, n: tn[:, kt * ntok + nt0: kt * ntok + nt0 + n]
            self.gemm(pools["psum"], self.WqT, E, E, rhs_tn, ntok, NTs,
                      mk_ev(q_s, False), tag="gemm", name=_tn("gemm"))
            self.gemm(pools["psum"], self.WkT, E, E, rhs_tn, ntok, NTs,
                      mk_ev(k_s, True), tag="gemm", name=_tn("gemm"))

            # V token-major [tok, E]
            v_m = pools["big"].tile([128, CH * E], BF16, tag="v_m", name=_tn("v_m"))
            has_tail = L > 128
            v_t = None
            if has_tail:
                v_t = pools["big"].tile([32, CH * E], BF16,
                                        tag="v_t", name=_tn("v_t"))
            Lm = min(128, L)
            for s in range(ns):
                ps = pools["psum"].tile([128, E], F32, tag="gemm", name=_tn("gemm"))
                for kt in range(FT):
                    nc.tensor.matmul(
                        ps[0:Lm, :],
                        tok_bf[:, kt * ntok + s * L: kt * ntok + s * L + Lm],
                        self.WvT[:, kt * E:(kt + 1) * E],
                        start=(kt == 0), stop=(kt == FT - 1))
                nc.vector.tensor_copy(
                    out=v_m[0:Lm, s * E:(s + 1) * E], in_=ps[0:Lm, :])
            if has_tail:
                tl = L - 128
                for s in range(ns):
                    ps = pools["psum"].tile([128, E], F32, tag="gemm", name=_tn("gemm"))
                    for kt in range(FT):
                        nc.tensor.matmul(
                            ps[0:tl, :],
                            tok_bf[:, kt * ntok + s * L + 128:
                                   kt * ntok + s * L + 128 + tl],
                            self.WvT[:, kt * E:(kt + 1) * E],
                            start=(kt == 0), stop=(kt == FT - 1))
                    nc.scalar.activation(
                        v_t[0:tl, s * E:(s + 1) * E], ps[0:tl, :], AF.Copy)

            o_bf = pools["big"].tile([128, FT * ntok], BF16, tag="o_bf", name=_tn("o_bf"))
            if "attn" in self.ablate:
                nc.vector.tensor_copy(out=o_bf, in_=tn)
            else:
                self.attention(pools, q_s, k_s, v_m, v_t, masks,
                               o_bf, ns, L, ntok, ntokmax)

            # out-proj + residual
            outp_f = pools["big1"].tile([128, FT * ntok], F32, tag="outp_f", name=_tn("outp_f"))
            outp_bf = pools["big"].tile([128, FT * ntok], BF16, tag="outp_bf", name=_tn("outp_bf"))

            def ev_outp(mt, nt0, n, ps):
                nc.vector.tensor_tensor(
                    out=outp_f[:, mt * ntok + nt0: mt * ntok + nt0 + n],
                    in0=ps[:, :n],
                    in1=tok_f[:, mt * ntok + nt0: mt * ntok + nt0 + n],
                    op=OP.add)
                nc.scalar.activation(
                    outp_bf[:, mt * ntok + nt0: mt * ntok + nt0 + n],
                    outp_f[:, mt * ntok + nt0: mt * ntok + nt0 + n], AF.Copy)

            rhs_o = lambda kt, nt0, n: o_bf[:, kt * ntok + nt0: kt * ntok + nt0 + n]
            self.gemm(pools["psum"], self.WoT, E, E, rhs_o, ntok, NTs,
                      ev_outp, tag="gemm", name=_tn("gemm"))

            tn2 = pools["big"].tile([128, FT * ntok], BF16, tag="tn2", name=_tn("tn2"))
            if "ln" in self.ablate:
                nc.vector.tensor_copy(out=tn2, in_=outp_bf)
            else:
                self.ln_chunk_v2(tc, pools, outp_f, self.lnw2, self.lnb2, tn2,
                                 ntok, NTs, also_sq_src=outp_bf)

            ffh = pools["big"].tile([128, 2 * FT * ntok], BF16, tag="ffh", name=_tn("ffh"))

            def ev_ffh(mt, nt0, n, ps):
                nc.vector.tensor_relu(
                    out=ffh[:, mt * ntok + nt0: mt * ntok + nt0 + n],
                    in_=ps[:, :n])

            rhs_tn2 = lambda kt, nt0, n: tn2[:, kt * ntok + nt0: kt * ntok + nt0 + n]
            self.gemm(pools["psum"], self.Wff1T, E, 2 * E, rhs_tn2, ntok, NTs,
                      ev_ffh, tag="gemm", name=_tn("gemm"))

            res2_bf = pools["big"].tile([128, FT * ntok], BF16, tag="res2_bf", name=_tn("res2_bf"))

            def ev_ffo(mt, nt0, n, ps):
                # res2 is only consumed as bf16 (Wout rhs): add straight
                # into bf16, no f32 staging tile.
                nc.vector.tensor_tensor(
                    out=res2_bf[:, mt * ntok + nt0: mt * ntok + nt0 + n],
                    in0=ps[:, :n],
                    in1=outp_f[:, mt * ntok + nt0: mt * ntok + nt0 + n],
                    op=OP.add)

            rhs_ffh = lambda kt, nt0, n: ffh[:, kt * ntok + nt0: kt * ntok + nt0 + n]
            self.gemm(pools["psum"], self.Wff2T, 2 * E, E, rhs_ffh, ntok, NTs,
                      ev_ffo, tag="gemm", name=_tn("gemm"))

            rhs_r2 = lambda kt, nt0, n: res2_bf[:, kt * ntok + nt0: kt * ntok + nt0 + n]
            self.gemm(pools["psum"], self.WoutT, E, cfg.C, rhs_r2, ntok, NTs,
                      lambda mt, nt0, n, ps: evict_y_fn(c0, nt0, n, ps),
                      tag="gemm", name=_tn("gemm"))

    # -- conv 3x3 over (h,w) for one b-half --------------------------------
    def conv_half(self, pools, cin, evict_fn, rows=None):
        """cin: sbuf [128, A*6*WPAD*A(v)] bf16 window tile (w-padded).
        out pixels (u, hl, w, v); evict_fn(u, hp, n, psum) with pixel tiles
        [128, 2*W*A(v)].  rows: list of (hp, nh) output-row groups; default
        covers all HL rows in pairs."""
        nc, cfg = self.nc, self.cfg
        A, W = cfg.A, cfg.W
        WP = W + 2
        if rows is None:
            rows = [(hp, min(2, cfg.HL - hp)) for hp in range(0, cfg.HL, 2)]
        cv = cin.rearrange("p (u h w v) -> p u h w v", u=A, h=6, w=WP)
        for u in range(A):
            for hp, nh in rows:
                ps = pools["psum"].tile([128, 2 * W * A], F32, tag="gemm", name=_tn("gemm"))
                first = True
                for dy in range(3):
                    for dx in range(3):
                        rhs = cv[:, u, hp + dy:hp + dy + nh, dx:dx + W, :]
                        nc.tensor.matmul(
                            ps[:, :nh * W * A],
                            self.taps[:, (dy * 3 + dx) * 128:(dy * 3 + dx + 1) * 128],
                            rhs,
                            start=first, stop=(dy == 2 and dx == 2))
                        first = False
                evict_fn(u, hp, nh, ps)

    # -- full graph --------------------------------------------------------
    def build(self):
        cfg = self.cfg
        nc = self.nc
        A, C, E, B, H, W = cfg.A, cfg.C, cfg.E, cfg.B, cfg.H, cfg.W
        L1, L2 = cfg.L1, cfg.L2
        FT = E // 128
        WL, HL, HP = cfg.WL, cfg.HL, cfg.HPAD
        NC = cfg.ncores
        WPAD = W + 2
        chunk1_cols = A * HP * WL * A          # y_perm cols (u hpad wl v)
        chunk2_cols = A * HL * W * A           # y2 cols (u hl w v)

        # ---- dram params
        P = {}
        def par(name, shape, dt):
            P[name] = nc.dram_tensor(name, shape, dt, kind="ExternalInput").ap()
        par("xtok1", [C, B * cfg.SEQ1 * L1], BF16)
        par("res1", [C, B * chunk2_cols], F32)
        par("WinT", [C, E], BF16)
        for n in ("WqT", "WkT", "WvT", "WoT"):
            par(n, [E, E], BF16)
        par("Wff1T", [E, 2 * E], BF16)
        par("Wff2T", [2 * E, E], BF16)
        par("WoutT", [E, C], BF16)
        par("tapT", [3, 3, C, C], BF16)
        par("mask3_1", [128, 480], BF16)
        par("maskB_1", [128, 320], BF16)
        par("maskC_1", [128, 480], BF16)
        par("mask3_2", [128, 480], BF16)
        par("maskB_2", [128, 320], BF16)
        par("maskC_2", [128, 480], BF16)
        par("ident", [128, 128], BF16)
        for n in ("lnw1", "lnb1", "lnw2", "lnb2"):
            par(n, [128, FT], F32)
        self.params = P
        out_ext = nc.dram_tensor("out", [C, B * chunk2_cols], F32,
                                 kind="ExternalOutput")

        with tile.TileContext(nc) as tc:
            import contextlib
            with contextlib.ExitStack() as ctx:
                pools = {}
                pools["const"] = ctx.enter_context(
                    tc.tile_pool(name="const", bufs=1))
                pools["big"] = ctx.enter_context(
                    tc.tile_pool(name="big", bufs=2))
                pools["big1"] = ctx.enter_context(
                    tc.tile_pool(name="big1", bufs=2))
                pools["qk"] = ctx.enter_context(
                    tc.tile_pool(name="qk", bufs=1))
                pools["scratch"] = ctx.enter_context(
                    tc.tile_pool(name="scratch", bufs=1))
                pools["io"] = ctx.enter_context(
                    tc.tile_pool(name="io", bufs=1))
                pools["psum"] = ctx.enter_context(
                    tc.tile_pool(name="psum", bufs=2, space="PSUM"))
                pools["stat"] = ctx.enter_context(
                    tc.tile_pool(name="stat", bufs=1, space="PSUM"))
                pools["sc"] = ctx.enter_context(
                    tc.tile_pool(name="sc", bufs=2, space="PSUM"))
                pools["exp"] = ctx.enter_context(
                    tc.tile_pool(name="exp", bufs=3))
                pools["dram"] = ctx.enter_context(
                    tc.tile_pool(name="dram", bufs=1, space="DRAM"))
                self._build_body(tc, pools, out_ext)
        nc.compile()
        return nc

    def _build_body(self, tc, pools, out_ext):
        nc, cfg = self.nc, self.cfg
        A, C, E, B, H, W = cfg.A, cfg.C, cfg.E, cfg.B, cfg.H, cfg.W
        L1, L2 = cfg.L1, cfg.L2
        FT = E // 128
        WL, HL, HP = cfg.WL, cfg.HL, cfg.HPAD
        NC = cfg.ncores
        WPAD = W + 2
        ch1 = A * HP * WL * A
        ch2 = A * HL * W * A
        dram = pools["dram"]

        self.load_weights(tc, pools["const"])
        import os as _os
        _simpid = _os.environ.get("KSIMPID")
        pid = int(_simpid) if _simpid else nc.partition_id()

        x1_chunk = [dram.tile([C, ch1], BF16, tag=f"x1c{b}", name=_tn(f"x1c{b}")) for b in range(B)]
        x1_full = [dram.tile([NC * C, ch1], BF16, addr_space="Shared",
                             tag=f"x1f{b}", name=_tn(f"x1f{b}")) for b in range(B)]
        bnd_cols = 2 * A * W * A
        bnd_snd = [dram.tile([C, bnd_cols], BF16, tag=f"bs{b}", name=_tn(f"bs{b}")) for b in range(B)]
        bnd_all = [dram.tile([NC * C, bnd_cols], BF16, tag=f"ba{b}", name=_tn(f"ba{b}")) for b in range(B)]
        sc2_dram = [dram.tile([C, ch2], F32, tag=f"s2d{b}", name=_tn(f"s2d{b}")) for b in range(B)]

        # ---------------- pass 1 (per b-half)
        sc2_f = []
        y2_sbs = []
        for b in range(B):
            y_perm = pools["io"].tile([C, ch1], BF16, tag="y_perm", name=_tn("y_perm"))
            # zero hpad rows 0 and HP-1:  cols (u, {0,HP-1}, wl, v)
            yv = y_perm.rearrange("p (u h w v) -> p u h w v", u=A, h=HP, w=WL)
            nc.vector.memset(yv[:, :, 0:1, :, :], 0.0)
            nc.vector.memset(yv[:, :, HP - 1:HP, :, :], 0.0)

            def x_src(c0, ntok, b=b):
                t = pools["big"].tile([128, ntok], BF16, tag="x_in", name=_tn("x_in"))
                nc.sync.dma_start(
                    t, self.params["xtok1"][:, b * cfg.SEQ1 * L1 + c0 * L1:
                                            b * cfg.SEQ1 * L1 + c0 * L1 + ntok])
                return t

            def ev_y(c0, nt0, n, ps, y_perm=y_perm):
                # psum [128, n] tokens of seqs starting at s0=c0+nt0//L1
                # seq (v, wl): v = s//WL, wl = s%WL ; token (u, h)
                # y_perm col = u*(HP*WL*A) + (h+1)*(WL*A) + wl*A + v
                s0 = c0 + nt0 // L1
                npair = n // L1
                for i in range(0, npair):
                    s = s0 + i
                    v, wl = s // WL, s % WL
                    dst = y_perm.rearrange(
                        "p (u h w vv) -> p u h w vv", u=A, h=HP, w=WL)
                    nc.vector.tensor_copy(
                        out=dst[:, :, 1:H + 1, wl, v],
                        in_=ps[:, i * L1:(i + 1) * L1].rearrange(
                            "p (u h) -> p u h", h=H))

            self.transformer_half(tc, pools, x_src,
                                  (self.mask3_1, self.maskB_1, self.maskC_1),
                                  L1, cfg.SEQ1, ev_y)
            nc.sync.dma_start(x1_chunk[b][:, :], y_perm)
            nc.gpsimd.collective_compute(
                "AllGather", mybir.AluOpType.bypass,
                ins=[x1_chunk[b].opt()], outs=[x1_full[b].opt()],
                replica_groups=[list(range(NC))])

        # ---------------- conv1 + residual -> sc2 ; then pass 2
        for b in range(B):
            cin = pools["io"].tile([C, A * 6 * WPAD * A], BF16, tag="cin", name=_tn("cin"))
            cinv = cin.rearrange("p (u h w v) -> p u h w v", u=A, h=6, w=WPAD)
            nc.vector.memset(cinv[:, :, :, 0:1, :], 0.0)
            nc.vector.memset(cinv[:, :, :, WPAD - 1:WPAD, :], 0.0)
            xf = x1_full[b].rearrange("(wc c) (u h w v) -> wc c u h w v",
                                      c=C, u=A, h=HP, w=WL)
            for wc in range(NC):
                nc.sync.dma_start(
                    cinv[:, :, :, 1 + wc * WL:1 + (wc + 1) * WL, :],
                    xf[wc, :, :, ds(pid * HL, 6), :, :])
            scf = pools["io"].tile([C, ch2], BF16, tag=f"sc2f{b}", name=_tn(f"sc2f{b}"))
            sc2_f.append(scf)

            res_cache = {}

            def ev_c1b(u, hp, nh, ps, b=b, scf=scf, res_cache=res_cache):
                col = u * (HL * W * A) + hp * (W * A)
                n = nh * W * A
                if u not in res_cache:
                    rt = pools["big"].tile([C, HL * W * A], F32, tag="res_u", name=_tn("res_u"))
                    ub = u * (HL * W * A)
                    nc.sync.dma_start(
                        rt, self.params["res1"][:, b * ch2 + ub:
                                                b * ch2 + ub + HL * W * A])
                    res_cache[u] = rt
                sct = pools["big"].tile([C, 2 * W * A], F32, tag="out_t", name=_tn("sc_t"))
                nc.vector.tensor_tensor(
                    out=sct[:, :n], in0=ps[:, :n],
                    in1=res_cache[u][:, hp * W * A: hp * W * A + n], op=OP.add)
                nc.scalar.activation(scf[:, col:col + n], sct[:, :n], AF.Copy)
                nc.sync.dma_start(sc2_dram[b][:, col:col + n], sct[:, :n])

            self.conv_half(pools, cin, ev_c1b)

            # ---- pass 2 on sc2 (boundary hl rows first so the halo
            # exchange overlaps interior compute)
            y2_sb = pools["io"].tile([C, ch2], BF16, tag=f"y2sb{b}", name=_tn(f"y2sb{b}"))
            y2_sbs.append(y2_sb)
            perm2 = ([u * HL for u in range(A)]
                     + [u * HL + HL - 1 for u in range(A)]
                     + [u * HL + hl for hl in range(1, HL - 1)
                        for u in range(A)])

            def x_src2(c0, ntok, scf=scf):
                t = pools["big"].tile([128, ntok], BF16, tag="x2g", name=_tn("x2g"))
                for i in range(ntok // L2):
                    st = perm2[c0 + i]
                    nc.vector.tensor_copy(
                        out=t[:, i * L2:(i + 1) * L2],
                        in_=scf[:, st * L2:(st + 1) * L2])
                return t

            def ev_y2(c0, nt0, n, ps, y2_sb=y2_sb):
                for i in range(n // L2):
                    st = perm2[c0 + nt0 // L2 + i]
                    nc.vector.tensor_copy(
                        out=y2_sb[:, st * L2:(st + 1) * L2],
                        in_=ps[:, i * L2:(i + 1) * L2])

            self.transformer_half(tc, pools, x_src2,
                                  (self.mask3_2, self.maskB_2, self.maskC_2),
                                  L2, cfg.SEQ2, ev_y2)
            ysb_v = y2_sb.rearrange("p (u hl wv) -> p u hl wv", u=A, hl=HL)
            nc.sync.dma_start(
                bnd_snd[b][:, 0:A * W * A].rearrange(
                    "p (u wv) -> p u wv", u=A),
                ysb_v[:, :, 0, :])
            nc.sync.dma_start(
                bnd_snd[b][:, A * W * A:2 * A * W * A].rearrange(
                    "p (u wv) -> p u wv", u=A),
                ysb_v[:, :, HL - 1, :])
            nc.gpsimd.collective_compute(
                "AllGather", mybir.AluOpType.bypass,
                ins=[bnd_snd[b].opt()],
                outs=[bnd_all[b].opt()],
                replica_groups=[list(range(NC))])

        # ---------------- conv2 + sc2 -> out
        for b in range(B):
            cin = pools["io"].tile([C, A * 6 * WPAD * A], BF16, tag="cin", name=_tn("cin"))
            cinv = cin.rearrange("p (u h w v) -> p u h w v", u=A, h=6, w=WPAD)
            nc.vector.memset(cinv[:, :, :, 0:1, :], 0.0)
            nc.vector.memset(cinv[:, :, :, WPAD - 1:WPAD, :], 0.0)
            ysv = y2_sbs[b].rearrange("p (u hl w v) -> p u hl w v",
                                      u=A, hl=HL, w=W)
            for u in range(A):
                nc.sync.dma_start(cinv[:, u, 1:1 + HL, 1:1 + W, :], ysv[:, u])
            # interior output rows need only local y2 — run before halo
            def ev_c2(u, hp, nh, ps, b=b):
                col = u * (HL * W * A) + hp * (W * A)
                n = nh * W * A
                rt = pools["big"].tile([C, 2 * W * A], F32, tag="res_u", name=_tn("res2_u"))
                nc.sync.dma_start(rt[:, :n], sc2_dram[b][:, col: col + n])
                ot = pools["big"].tile([128, 2 * W * A], F32, tag="out_t", name=_tn("out_t"))
                nc.vector.tensor_tensor(
                    out=ot[:, :n], in0=ps[:, :n], in1=rt[:, :n], op=OP.add)
                nc.sync.dma_start(
                    out_ext.ap()[:, b * ch2 + col: b * ch2 + col + n],
                    ot[:, :n])

            self.conv_half(pools, cin, ev_c2, rows=[(1, 2)])
            blkA = (pid + NC - 1) % NC
            blkC = (pid + 1) % NC
            bav_t = bnd_all[b][ds(blkA * C, C), :].rearrange(
                "c (e u w v) -> c e u w v", e=2, u=A, w=W)
            nc.sync.dma_start(cinv[:, :, 0, 1:1 + W, :], bav_t[:, 1])
            bav_b = bnd_all[b][ds(blkC * C, C), :].rearrange(
                "c (e u w v) -> c e u w v", e=2, u=A, w=W)
            nc.sync.dma_start(cinv[:, :, 5, 1:1 + W, :], bav_b[:, 0])
            if isinstance(pid, int):
                if pid < 1:
                    nc.vector.memset(cinv[:, :, 0:1, :, :], 0.0)
                if pid > NC - 2:
                    nc.vector.memset(cinv[:, :, 5:6, :, :], 0.0)
            else:
                with tc.If(pid < 1):
                    nc.vector.memset(cinv[:, :, 0:1, :, :], 0.0)
                with tc.If(pid > NC - 2):
                    nc.vector.memset(cinv[:, :, 5:6, :, :], 0.0)
            self.conv_half(pools, cin, ev_c2, rows=[(0, 1), (HL - 1, 1)])


# ---------------------------------------------------------------- entry point

_CACHE = {}


def kernel(**inputs):
    import numpy as np
    from concourse.bass_utils import run_bass_kernel_spmd
    import os as _os
    cfg = Cfg()
    if "nc" not in _CACHE:
        abl = tuple(x for x in _os.environ.get("KABL", "").split(",") if x)
        ker = Ker(cfg, ablate=abl)
        _CACHE["nc"] = ker.build()
    nc = _CACHE["nc"]
    in_maps = host_prep(cfg, inputs)
    res = run_bass_kernel_spmd(nc, in_maps, core_ids=list(range(cfg.ncores)),
                               trace=False)
    outs = [res.results[i]["out"] for i in range(cfg.ncores)]
    return host_unshard(cfg, outs).astype(np.float32)

